# revision 59
# baseline (speedup 1.0000x reference)
"""Qwen3.5 vision attention (S=4096, H=16x80, block-diag mask) on 8 trn2 cores.

Sharding: tensor-parallel over heads (2 heads/core). Each core computes
qkv projection for its heads, rotary, block-sparse attention, and a partial
output projection (RowParallel); the host sums the 8 partials (all-reduce)
and adds proj_b.

v2 (390 -> 238 us): bf16 everywhere off the PSUM paths, on-chip softmax
normalization (no DRAM round-trip), on-chip V ones-column, staged output
stores.

v3 (238 -> ~210 us) — PE (tensor engine) is the bottleneck at ~85% busy;
every change targets PE stalls and the HAM clock gate (PE runs at 1.2GHz
until ~3.4us of sustained activity, and re-throttles after idle windows):
  - rotary is split into per-(token-half, q/k, head) units; the half-0
    units interleave with qkv matmuls mid-phase, and at the block seam
    the half-1 DVE work is queued ahead of the next block's first qkv
    matmuls so the (in-order) PE always has filler in front of the
    rotary-dependent transposes. Seam transposes use the scores PSUM
    bank, which is idle exactly then.
  - x for block b+1 prefetches during block b's qkv phase (the 2.6MB
    DMA no longer starves the attention-phase filler).
  - attention chains run scores two chunks ahead of PV in the PE queue:
    PV_i waits exp_i while score_{i+2} is bounded by the scps WAR on
    exp_i, so the chain advances at the scalar exp rate with the scalar
    engine saturated instead of ping-ponging at ~2x that period.
  - sh1 projs of blocks 0-2 are deferred into the final block's seam and
    attention phase as per-chunk pair filler (the only dense PE work
    available there), and the final block's own sh0 proj pairs join the
    queue mid-phase. Late projs alternate between the proj and (idle)
    qkv PSUM pools so 4 matmul pairs run ahead of the psum->sbuf copies.
  - softmax denominator broadcast moved off the PE (rank-1 matmul) to
    gpsimd.partition_broadcast.
  - startup: weight chunks split across sync+scalar DMA queues ahead of /
    parallel with block-0 x; cos/sin host-pretransposed for contiguous
    DMA; self-pacing warmup matmuls cover the preamble+DMA window so the
    first real matmul runs at full clock.
"""

import os
from contextlib import ExitStack

import numpy as np

S = 4096
HID = 1280
D = 80
NB = 4
BS = 1024
NHL = 2  # heads per core
NCORES = 8
SCALING = float(D) ** -0.5
NEG_THRESH = -1e8

_CACHE = {}


def _build(allowed, mask_add, qkv_dt_name="bfloat16", attn_dt_name="bfloat16",
           out_dt_name="bfloat16"):
    """Build + compile the per-core bass module.

    allowed: tuple over qb of tuple of kb blocks attended to.
    mask_add: frozenset of (qb, kb) needing an additive mask tile.
    """
    import concourse.bass as bass
    import concourse.mybir as mybir
    import concourse.tile as tile
    from concourse import bacc
    from concourse.masks import make_identity

    f32 = mybir.dt.float32
    f32r = mybir.dt.float32r
    dt_qkv = getattr(mybir.dt, qkv_dt_name)
    dt_a = getattr(mybir.dt, attn_dt_name)
    out_dt = getattr(mybir.dt, out_dt_name)
    use_mask = len(mask_add) > 0

    nc = bacc.Bacc(
        "TRN2", target_bir_lowering=False, debug=False, num_devices=NCORES
    )
    xt = nc.dram_tensor("xt", [HID, S], dt_qkv, kind="ExternalInput").ap()
    wt = nc.dram_tensor("wt", [HID, 480], dt_qkv, kind="ExternalInput").ap()
    bqkv = nc.dram_tensor("bqkv", [1, 480], f32, kind="ExternalInput").ap()
    # host pre-rearranged to [p, c, d] tiles so the DMA is contiguous
    cosd = nc.dram_tensor("cosd", [NB * 128, 8 * D], dt_a, kind="ExternalInput").ap()
    sind = nc.dram_tensor("sind", [NB * 128, 8 * D], dt_a, kind="ExternalInput").ap()
    pw = nc.dram_tensor("pw", [2, D, HID], dt_a, kind="ExternalInput").ap()
    if use_mask:
        maskt = nc.dram_tensor("maskt", [S, S], f32, kind="ExternalInput").ap()
    outp = nc.dram_tensor("outp", [HID, S], out_dt, kind="ExternalOutput").ap()

    EXP = mybir.ActivationFunctionType.Exp
    interleave = all(tuple(allowed[b]) == (b,) for b in range(NB))

    with ExitStack() as ctx:
        tc = ctx.enter_context(tile.TileContext(nc))

        # ---- constants ----
        cpool = ctx.enter_context(tc.tile_pool(name="cpool", bufs=1))
        wt_sb = cpool.tile([128, 10, 480], dt_qkv, tag="wt_sb", name="wt_sb")
        wt_r = wt.rearrange("(kk p) c -> p kk c", p=128)
        # first two contraction chunks go ahead of everything else on the
        # sync queue; block 0's x tiles follow immediately so the first
        # qkv k-loop starts as soon as x_0 lands (~6.5us). The remaining
        # weight chunks, bias, and proj weights stream on the gpsimd queue
        # in parallel.
        nc.sync.dma_start(out=wt_sb[:, 0:2, :], in_=wt_r[:, 0:2, :])
        bias_bc = cpool.tile([128, 480], f32, tag="bias_bc", name="bias_bc")
        ident = cpool.tile([128, 128], dt_a, tag="ident", name="ident")
        make_identity(nc, ident)
        pw_sb = cpool.tile([D, 2, HID], dt_a, tag="pw_sb", name="pw_sb")

        def c_wrest():
            nc.scalar.dma_start(out=wt_sb[:, 2:6, :], in_=wt_r[:, 2:6, :])
            nc.scalar.dma_start(out=wt_sb[:, 6:10, :], in_=wt_r[:, 6:10, :])
            nc.gpsimd.dma_start(
                out=bias_bc, in_=bqkv[0:1, :].to_broadcast((128, 480))
            )

        def c_const():
            # deferred: not needed until the first attention/proj phase
            nc.gpsimd.dma_start(out=pw_sb, in_=pw.rearrange("h d o -> d h o"))

        kT_sb = [
            cpool.tile([D, S], dt_a, tag=f"kT{h}_sb", name=f"kT{h}_sb")
            for h in range(NHL)
        ]

        # ---- pools ----
        xtp = ctx.enter_context(tc.tile_pool(name="xtp", bufs=20))
        trig = ctx.enter_context(tc.tile_pool(name="trig", bufs=2))
        qkp = ctx.enter_context(tc.tile_pool(name="qkp", bufs=2))
        t2p = ctx.enter_context(tc.tile_pool(name="t2p", bufs=2))
        vp = ctx.enter_context(tc.tile_pool(name="vp", bufs=1))
        qtp = ctx.enter_context(
            tc.tile_pool(name="qtp", bufs=4 if interleave else 8)
        )
        expp = ctx.enter_context(tc.tile_pool(name="expp", bufs=4))
        # ot tiles of blocks NB-3/NB-2 stay alive until their deferred
        # sh1 proj runs during the final block -> all 4 blocks' ot tiles
        # can be live at once.
        otp = ctx.enter_context(tc.tile_pool(name="otp", bufs=8))
        ddp = ctx.enter_context(tc.tile_pool(name="ddp", bufs=2))
        rbp = ctx.enter_context(tc.tile_pool(name="rbp", bufs=2))
        stg = ctx.enter_context(tc.tile_pool(name="stg", bufs=20))
        if use_mask:
            mtp = ctx.enter_context(tc.tile_pool(name="mtp", bufs=4))

        # PSUM: 8 banks total. qkv accumulation and the rotary transposes
        # share one tag (they alternate within the qkv phase).
        qkvps = ctx.enter_context(tc.tile_pool(name="qkvps", bufs=2, space="PSUM"))
        scps = ctx.enter_context(tc.tile_pool(name="scps", bufs=2, space="PSUM"))
        pvps = ctx.enter_context(tc.tile_pool(name="pvps", bufs=2, space="PSUM"))
        prps = ctx.enter_context(tc.tile_pool(name="prps", bufs=2, space="PSUM"))

        # PE warmup: the HAM clock gate only releases full rate after ~3.4us
        # of sustained matmul activity, and the first real matmul waits on
        # the whole block-0 x DMA watermark (~13us). These throwaway
        # matmuls on a zeroed tile cover the 8..13us window so the real
        # qkv starts at full clock.
        scratch = cpool.tile([128, 512], dt_a, tag="scratch", name="scratch")
        nc.gpsimd.memset(scratch, 0.0)
        for w in range(6):
            wp = scps.tile([128, 128], f32, tag="sc", name=f"warm_{w}")
            nc.tensor.matmul(out=wp, lhsT=scratch[:, 0:128], rhs=scratch[:, 0:128])
        # wide warmups self-pace at ~640ns through the sem chain, covering
        # the stretch until the block-0 x watermark (~17us) with few
        # instructions.
        for w in range(12):
            wp = scps.tile([128, 512], f32, tag="sc", name=f"warmw_{w}")
            nc.tensor.matmul(out=wp, lhsT=scratch[:, 0:128], rhs=scratch)

        v_tiles = {}
        qT_tiles = {}
        xts_map = {}

        def load_x(b):
            # x of block b must be resident BEFORE block b-1's attention
            # begins (its qkv matmuls are the filler there), so this is
            # emitted at the start of block b-1's qkv phase: the 10 DMAs
            # (~7us) overlap that whole phase instead of starving the
            # attention filler. For block 0 the remaining weight chunks are
            # woven between the early x tiles on the same queue so the
            # first k-loop never stalls on a weight arriving behind all
            # of x.
            xts = []
            for k in range(10):
                xt_t = xtp.tile([128, BS], dt_qkv, tag="xt", name=f"xt_{b}_{k}")
                nc.sync.dma_start(
                    out=xt_t,
                    in_=xt[k * 128 : (k + 1) * 128, b * BS : (b + 1) * BS],
                )
                xts.append(xt_t)
            xts_map[b] = xts

        def qkv_chunks(b):
            """Emission chunks for the qkv+rotary+transpose phase of block b.

            Returned as closures so they can be zipped between the previous
            block's attention chunks: alternating bass_priority makes the
            tile scheduler fill the exp-bound attention stretches with qkv
            matmuls.
            """
            st = {}
            # qk tile allocated at qkv_chunks() call time (seam start) so
            # the first c_mm does not depend on the trig DMA emission.
            st["qk"] = qkp.tile([128, 8, 320], dt_a, tag="qk", name=f"qk_{b}")

            def c_load():
                cos_t = trig.tile([128, 8, D], dt_a, tag="cos", name=f"cos_{b}")
                nc.sync.dma_start(
                    out=cos_t,
                    in_=cosd[b * 128 : (b + 1) * 128, :].rearrange(
                        "p (c d) -> p c d", c=8
                    ),
                )
                sin_t = trig.tile([128, 8, D], dt_a, tag="sin", name=f"sin_{b}")
                nc.sync.dma_start(
                    out=sin_t,
                    in_=sind[b * 128 : (b + 1) * 128, :].rearrange(
                        "p (c d) -> p c d", c=8
                    ),
                )
                st["cos"] = cos_t
                st["sin"] = sin_t

            def c_mm(m):
                def run():
                    qk_blk = st["qk"]
                    ps = qkvps.tile(
                        [128, 480], f32, tag="qkvps", name=f"qkvps_{b}_{m}"
                    )
                    for k in range(10):
                        nc.tensor.matmul(
                            out=ps,
                            lhsT=xts_map[b][k][:, m * 128 : (m + 1) * 128],
                            rhs=wt_sb[:, k, :],
                            start=(k == 0),
                            stop=(k == 9),
                        )
                    nc.vector.tensor_add(
                        out=qk_blk[:, m, :], in0=ps[:, 0:320], in1=bias_bc[:, 0:320]
                    )
                    v_t = vp.tile(
                        [128, 2, 97], dt_a, tag="v", name=f"v_{b}_{m}",
                        bufs=16 if interleave else 32,
                    )
                    nc.vector.tensor_add(
                        out=v_t[:, :, 0:D],
                        in0=ps[:, 320:480].rearrange("p (h d) -> p h d", h=2),
                        in1=bias_bc[:, 320:480].rearrange("p (h d) -> p h d", h=2),
                    )
                    nc.gpsimd.memset(v_t[:, :, D:97], 1.0)
                    v_tiles[(b, m)] = v_t

                return run

            def rot_dve(half, tau, h):
                # rotary multiply-adds for one (token-half, q/k, head) unit.
                # Split from the transposes so the DVE work can be queued
                # ahead of PE filler matmuls at the block seam: the PE is
                # strictly in-order, so a transpose waiting on DVE rotary
                # blocks every later matmul in its queue.
                def run():
                    qk_blk = st["qk"]
                    sin_t = st["sin"]
                    cos_t = st["cos"]
                    base = tau * 160 + h * D
                    ms = slice(half * 4, half * 4 + 4)
                    sl = qk_blk[:, ms, base : base + D]
                    t2 = t2p.tile(
                        [128, 4, D], dt_a, tag="t2", name=f"t2_{b}_{half}_{tau}_{h}"
                    )
                    nc.vector.tensor_mul(
                        out=t2[:, :, 0:40],
                        in0=qk_blk[:, ms, base + 40 : base + D],
                        in1=sin_t[:, ms, 0:40],
                    )
                    nc.vector.tensor_mul(
                        out=t2[:, :, 40:D],
                        in0=qk_blk[:, ms, base : base + 40],
                        in1=sin_t[:, ms, 40:D],
                    )
                    nc.vector.tensor_mul(out=sl, in0=sl, in1=cos_t[:, ms, :])
                    nc.vector.tensor_add(out=sl, in0=sl, in1=t2)

                return run

            def rot_pe(half, tau, h):
                def run():
                    qk_blk = st["qk"]
                    base = tau * 160 + h * D
                    g = half
                    if tau == 0 and half == 0:
                        dst_t = qtp.tile([D, BS], dt_a, tag="qt", name=f"qT_{b}_{h}")
                        qT_tiles[(b, h)] = dst_t
                    # half-1 transposes run at the seam where the scores
                    # PSUM bank is idle; half-0 shares the qkv bank.
                    pool = qkvps if half == 0 else scps
                    tag = "qkvps" if half == 0 else "sc"
                    tp = pool.tile(
                        [D, 512], dt_a, tag=tag, name=f"tr_{b}_{half}_{tau}_{h}"
                    )
                    for j in range(4):
                        m = g * 4 + j
                        nc.tensor.matmul(
                            out=tp[:, j * 128 : (j + 1) * 128],
                            lhsT=qk_blk[:, m, base : base + D],
                            rhs=ident,
                            is_transpose=True,
                            start=(j == 0),
                            stop=(j == 3),
                        )
                    if tau == 0:
                        dst = qT_tiles[(b, h)][:, g * 512 : (g + 1) * 512]
                    else:
                        dst = kT_sb[h][
                            :, b * BS + g * 512 : b * BS + (g + 1) * 512
                        ]
                    nc.vector.tensor_copy(out=dst, in_=tp)

                return run

            return {
                "load": c_load,
                "mm": [c_mm(m) for m in range(8)],
                "rdve": {
                    (hf, tau, h): rot_dve(hf, tau, h)
                    for hf in range(2)
                    for tau in range(2)
                    for h in range(NHL)
                },
                "rpe": {
                    (hf, tau, h): rot_pe(hf, tau, h)
                    for hf in range(2)
                    for tau in range(2)
                    for h in range(NHL)
                },
            }

        def attn_chunks(b):
            kbs = allowed[b]
            nmm = len(kbs) * 8
            st = {}

            def c_alloc():
                st["ots"] = [
                    otp.tile([D, BS], dt_a, tag="ot", name=f"ot_{b}_{h}")
                    for h in range(NHL)
                ]

            def c_attn(sh, h, fillq=None):
                def run():
                    qT_t = qT_tiles[(b, h)]
                    ot_t = st["ots"][h]
                    pv = pvps.tile(
                        [97, 512], f32, tag="pv", name=f"pv_{b}_{h}_{sh}"
                    )
                    items = [(kb, t) for kb in kbs for t in range(8)]

                    def score(i):
                        kb, t = items[i]
                        scp = scps.tile(
                            [128, 512], f32, tag="sc",
                            name=f"sc_{b}_{h}_{sh}_{kb}_{t}",
                        )
                        nc.tensor.matmul(
                            out=scp,
                            lhsT=kT_sb[h][
                                :, kb * BS + t * 128 : kb * BS + (t + 1) * 128
                            ],
                            rhs=qT_t[:, sh * 512 : (sh + 1) * 512],
                            start=True,
                            stop=True,
                        )
                        if (b, kb) in mask_add:
                            mk = mtp.tile(
                                [128, 512], f32, tag="mk",
                                name=f"mk_{b}_{h}_{sh}_{kb}_{t}",
                            )
                            nc.sync.dma_start(
                                out=mk,
                                in_=maskt[
                                    kb * BS + t * 128 : kb * BS + (t + 1) * 128,
                                    b * BS + sh * 512 : b * BS + (sh + 1) * 512,
                                ],
                            )
                            nc.vector.tensor_add(out=scp, in0=scp, in1=mk)
                        ep = expp.tile(
                            [128, 512], dt_a, tag="exp",
                            name=f"ep_{b}_{h}_{sh}_{kb}_{t}",
                        )
                        nc.scalar.activation(
                            out=ep, in_=scp, func=EXP, scale=SCALING
                        )
                        return ep

                    def pvmm(i, ep):
                        kb, t = items[i]
                        nc.tensor.matmul(
                            out=pv,
                            lhsT=v_tiles[(kb, t)][:, h, :],
                            rhs=ep,
                            start=(i == 0),
                            stop=(i == nmm - 1),
                        )

                    # scores run two chunks ahead of pv in the (in-order) PE
                    # queue: pv_i waits on exp_i, and score_{i+2} waits on
                    # exp_i via the scps WAR, so the chain advances at the
                    # scalar exp rate with the scalar engine saturated.
                    # fillq items (deferred proj pairs) slot in after each
                    # pv to absorb the remaining exp slack.
                    pend = []
                    for i in range(len(items)):
                        ep = score(i)
                        pend.append((i, ep))
                        if i >= 2:
                            j, epj = pend.pop(0)
                            pvmm(j, epj)
                            if fillq:
                                fillq.pop(0)()
                    while pend:
                        j, epj = pend.pop(0)
                        pvmm(j, epj)
                        if fillq:
                            fillq.pop(0)()
                    # normalize: dd = denom row (scalar copy, psum->sbuf),
                    # broadcast across the 80 partitions on the (idle)
                    # gpsimd engine, invert with the fast custom-DVE
                    # reciprocal, ot = pv * rb.
                    dd = ddp.tile(
                        [1, 512], f32, tag="dd", name=f"dd_{b}_{h}_{sh}"
                    )
                    nc.scalar.copy(out=dd, in_=pv[96:97, :])
                    rbc = rbp.tile(
                        [D, 512], f32, tag="rbc", name=f"rbc_{b}_{h}_{sh}"
                    )
                    nc.gpsimd.partition_broadcast(rbc, dd)
                    rb = rbp.tile([D, 512], f32, tag="rb", name=f"rb_{b}_{h}_{sh}")
                    nc.vector.reciprocal_approx_fast(out=rb, in_=rbc)
                    nc.vector.tensor_mul(
                        out=ot_t[:, sh * 512 : (sh + 1) * 512],
                        in0=pv[0:D, :],
                        in1=rb,
                    )

                return run

            def emit_proj_o(sh, o, late, copy_scalar):
                ots = st["ots"]
                sts_t = stg.tile(
                    [128, 512], out_dt, tag="st", name=f"st_{b}_{sh}_{o}"
                )
                if copy_scalar and o % 2:
                    # scalar path (exp-idle stretches): fp32 accumulate +
                    # ACT copy, keeping the work off the busy DVE.
                    pp = qkvps.tile(
                        [128, 512], f32, tag="qkvps", name=f"pr_{b}_{sh}_{o}"
                    )
                    for hh in range(2):
                        nc.tensor.matmul(
                            out=pp,
                            lhsT=pw_sb[:, hh, o * 128 : (o + 1) * 128],
                            rhs=ots[hh][:, sh * 512 : (sh + 1) * 512],
                            start=(hh == 0),
                            stop=(hh == 1),
                        )
                    nc.scalar.copy(out=sts_t, in_=pp)
                    if b == NB - 1:
                        wp = pvps.tile(
                            [128, 128], f32, tag="pv", name=f"tw_{b}_{sh}_{o}"
                        )
                        nc.tensor.matmul(
                            out=wp,
                            lhsT=scratch[:, 0:128],
                            rhs=scratch[:, 0:128],
                        )
                else:
                    # late projs run when qkv is idle: alternate the idle
                    # qkv PSUM banks in so 4 matmul pairs can run ahead of
                    # the psum->sbuf copies.
                    if (late or b == NB - 1) and o % 2:
                        pp = qkvps.tile(
                            [128, 512], f32, tag="qkvps", name=f"pr_{b}_{sh}_{o}"
                        )
                    else:
                        pp = prps.tile(
                            [128, 512], f32, tag="pr", name=f"pr_{b}_{sh}_{o}"
                        )
                    for hh in range(2):
                        nc.tensor.matmul(
                            out=pp,
                            lhsT=pw_sb[:, hh, o * 128 : (o + 1) * 128],
                            rhs=ots[hh][:, sh * 512 : (sh + 1) * 512],
                            start=(hh == 0),
                            stop=(hh == 1),
                        )
                    nc.vector.tensor_copy(out=sts_t, in_=pp)
                if copy_scalar:
                    eng = (nc.sync, nc.gpsimd, nc.scalar)[o % 3]
                else:
                    eng = nc.gpsimd if o % 2 else nc.sync
                eng.dma_start(
                    out=outp[
                        o * 128 : (o + 1) * 128,
                        b * BS + sh * 512 : b * BS + (sh + 1) * 512,
                    ],
                    in_=sts_t,
                )

            def c_proj(sh, o_lo=0, o_hi=10, late=False, copy_scalar=False):
                def run():
                    for o in range(o_lo, o_hi):
                        emit_proj_o(sh, o, late, copy_scalar)

                return run

            def proj_pair(sh, o, late=False):
                def run():
                    emit_proj_o(sh, o, late, False)

                return run

            if b == NB - 1:
                # final block: deferred proj pairs of blocks NB-3/NB-2 are
                # consumed one pair per attention chunk inside the units
                # (fillq), absorbing the per-chunk exp slack; the block's
                # own sh0 proj pairs join the queue once both sh0 units are
                # done. sh1 proj runs at the very end with scalar copies
                # (exp is finished by then).
                chunks = [c_alloc]
                for h in range(NHL):
                    r = c_attn(0, h, fillq=deferred_pairs)
                    r.is_attn = True
                    chunks.append(r)
                chunks.append(
                    lambda: deferred_pairs.extend(
                        proj_pair(0, o) for o in range(10)
                    )
                )
                for h in range(NHL):
                    r = c_attn(1, h, fillq=deferred_pairs)
                    r.is_attn = True
                    chunks.append(r)
                chunks.append(lambda: [f() for f in deferred_pairs])
                chunks.append(lambda: deferred_pairs.clear())
                chunks.append(c_proj(1, copy_scalar=True))
                return chunks

            chunks = [c_alloc]
            for sh in range(2):
                for h in range(NHL):
                    r = c_attn(sh, h)
                    r.is_attn = True
                    chunks.append(r)
                # sh1 projs of blocks NB-4..NB-2 are deferred into the last
                # block's seam + attention phase: they are the only dense PE
                # filler available there (no next-block qkv remains), keeping
                # the HAM clock warm through the tail. Block NB-4's unit is
                # split in half to zip with the seam transposes; its copies
                # go to the scalar engine, which is exp-idle at the seam.
                if sh == 1 and b == NB - 4:
                    deferred.append(c_proj(sh, 0, 5, late=True, copy_scalar=True))
                    deferred.append(c_proj(sh, 5, 10, late=True, copy_scalar=True))
                elif sh == 1 and b in (NB - 3, NB - 2):
                    deferred_pairs.extend(
                        proj_pair(sh, o, late=True) for o in range(10)
                    )
                else:
                    # the last two o-chunks of each sh0 proj also go to the
                    # final block's fill queue: 30 pairs cover only 30 of
                    # its 32 attention-chunk slots, so the last unit runs
                    # dry without these.
                    chunks.append(c_proj(sh, 0, 7))
                    deferred_pairs.extend(
                        proj_pair(sh, o, late=True) for o in range(7, 10)
                    )
            return chunks

        # rotary unit order: k heads first (scores need the full kT block),
        # q heads after (only the matching sh-half of qT is needed early)
        ROT0 = [(0, 1, 0), (0, 1, 1), (0, 0, 0), (0, 0, 1)]
        ROT1 = [(1, 1, 0), (1, 1, 1), (1, 0, 0), (1, 0, 1)]

        def mid_list(q):
            # qkv mid-phase of a block: mm4..7 with the half-0 rotary units
            # woven in so each transpose lands behind a dense mm stretch.
            return [
                q["mm"][4], q["rdve"][ROT0[0]], q["mm"][5], q["rpe"][ROT0[0]],
                q["rdve"][ROT0[1]], q["mm"][6], q["rpe"][ROT0[1]],
                q["rdve"][ROT0[2]], q["mm"][7], q["rpe"][ROT0[2]],
                q["rdve"][ROT0[3]], q["rpe"][ROT0[3]],
            ]

        if interleave:
            # Software-pipelined emission. Per block b:
            #   seam:      half-1 rotary DVE queued first, its transposes
            #              zipped between block b+1's first qkv matmuls
            #              (in-order PE filler for the DVE wait)
            #   attention: zipped with block b+1's qkv mid-phase
            # The deferred sh1 projs of blocks NB-3/NB-2 play the role of
            # the "next qkv" for the final block.
            deferred = []
            deferred_pairs = []
            load_x(0)
            cur = qkv_chunks(0)
            c_wrest()
            for m in range(2):
                cur["mm"][m]()
            cur["load"]()
            c_const()
            for m in range(2, 4):
                cur["mm"][m]()
            load_x(1)
            for c in mid_list(cur):
                c()
            for b in range(NB):
                at = attn_chunks(b)
                if b + 1 < NB:
                    nxt = qkv_chunks(b + 1)
                    nxt["load"]()
                    if b + 2 < NB:
                        load_x(b + 2)
                    seam_fill = [nxt["mm"][m] for m in range(4)]
                else:
                    nxt = None
                    seam_fill = deferred[:2]
                for u in ROT1:
                    cur["rdve"][u]()
                if nxt is not None:
                    for f, u in zip(seam_fill, ROT1):
                        f()
                        cur["rpe"][u]()
                else:
                    seam_fill[0]()
                    cur["rpe"][ROT1[0]]()
                    cur["rpe"][ROT1[1]]()
                    seam_fill[1]()
                    cur["rpe"][ROT1[2]]()
                    cur["rpe"][ROT1[3]]()
                filler = mid_list(nxt) if nxt is not None else deferred[2:]
                # distribute filler only after the exp-bound attention
                # chunks (proj chunks are already tensor-dense)
                attn_pos = [
                    i for i, a in enumerate(at) if getattr(a, "is_attn", False)
                ]
                k = 0
                j = 0
                for i, a in enumerate(at):
                    a()
                    if i in attn_pos:
                        j += 1
                        take = (len(filler) * j) // max(1, len(attn_pos)) - k
                        for _ in range(take):
                            filler[k]()
                            k += 1
                while k < len(filler):
                    filler[k]()
                    k += 1
                cur = nxt
        else:
            deferred = []
            deferred_pairs = []
            c_wrest()
            c_const()
            load_x(0)
            for b in range(NB):
                if b + 1 < NB:
                    load_x(b + 1)
                q = qkv_chunks(b)
                q["load"]()
                for m in range(8):
                    q["mm"][m]()
                for u in ROT0 + ROT1:
                    q["rdve"][u]()
                    q["rpe"][u]()
            for b in range(NB):
                for c in attn_chunks(b):
                    c()
            for c in deferred:
                c()
            for c in deferred_pairs:
                c()

    nc.compile()
    return nc


def _build_v4(qkv_dt_name="bfloat16", attn_dt_name="bfloat16",
              out_dt_name="bfloat16"):
    """Block-sharded build: core c handles sequence block c//2 (1024
    tokens) and head-half c%2 (8 heads, 4 head-groups of 2). Attention is
    identical per head-group, but the output projection now contracts over
    640 dims (5 full 128-chunks) instead of 2x80 -> 10x2x5x512 cycles for
    a quarter of the output area, saving ~12.8us of PE streaming per core,
    and the kernel ends in a dense exp-free proj GEMM.
    Only valid for the exact block-diagonal mask (no mask adds).
    """
    import concourse.mybir as mybir
    import concourse.tile as tile
    from concourse import bacc
    from concourse.masks import make_identity

    f32 = mybir.dt.float32
    dt_qkv = getattr(mybir.dt, qkv_dt_name)
    dt_a = getattr(mybir.dt, attn_dt_name)
    out_dt = getattr(mybir.dt, out_dt_name)
    NHG = 4  # head groups of 2 per core

    nc = bacc.Bacc(
        "TRN2", target_bir_lowering=False, debug=False, num_devices=NCORES
    )
    xt = nc.dram_tensor("xt", [HID, BS], dt_qkv, kind="ExternalInput").ap()
    # host pre-arranged to the SBUF tile layout so every weight DMA is
    # contiguous per partition (strided weight loads run at ~50GB/s and
    # starve the x stream)
    wt = nc.dram_tensor(
        "wt", [NHG, 128, 10 * 480], dt_qkv, kind="ExternalInput"
    ).ap()
    bqkv = nc.dram_tensor("bqkv", [1, NHG * 480], f32, kind="ExternalInput").ap()
    cosd = nc.dram_tensor("cosd", [128, 8 * D], dt_a, kind="ExternalInput").ap()
    sind = nc.dram_tensor("sind", [128, 8 * D], dt_a, kind="ExternalInput").ap()
    pw = nc.dram_tensor("pw", [5, 128, HID], dt_a, kind="ExternalInput").ap()
    outp = nc.dram_tensor("outp", [HID, BS], out_dt, kind="ExternalOutput").ap()

    EXP = mybir.ActivationFunctionType.Exp

    with ExitStack() as ctx:
        tc = ctx.enter_context(tile.TileContext(nc))

        cpool = ctx.enter_context(tc.tile_pool(name="cpool", bufs=1))
        wt_sb = cpool.tile([128, NHG, 10, 480], dt_qkv, tag="wt_sb", name="wt_sb")
        # hg0 weights ahead of x on sync/scalar; hg1-3 behind on gpsimd
        wt_r0 = wt[0].rearrange("p (kk c) -> p kk c", kk=10)
        nc.sync.dma_start(out=wt_sb[:, 0, 0:2, :], in_=wt_r0[:, 0:2, :])
        bias_bc = cpool.tile([128, NHG * 480], f32, tag="bias_bc", name="bias_bc")
        ident = cpool.tile([128, 128], dt_a, tag="ident", name="ident")
        make_identity(nc, ident)
        pw_sb = cpool.tile([128, 5, HID], dt_a, tag="pw_sb", name="pw_sb")
        ot_stack = [
            cpool.tile([128, BS], dt_a, tag=f"ot{c}", name=f"ot{c}")
            for c in range(5)
        ]

        def c_wrest():
            nc.scalar.dma_start(out=wt_sb[:, 0, 2:10, :], in_=wt_r0[:, 2:10, :])
            nc.gpsimd.dma_start(
                out=bias_bc[:, 0:480],
                in_=bqkv[0:1, 0:480].to_broadcast((128, 480)),
            )

        def c_wnext(hg):
            # one head-group of weights (1.2MB), loaded one phase before
            # its qkv so the DMA never competes with block-0 x at startup
            def run():
                nc.gpsimd.dma_start(
                    out=wt_sb[:, hg, :, :],
                    in_=wt[hg].rearrange("p (kk c) -> p kk c", kk=10),
                )
                nc.gpsimd.dma_start(
                    out=bias_bc[:, hg * 480 : (hg + 1) * 480],
                    in_=bqkv[0:1, hg * 480 : (hg + 1) * 480].to_broadcast(
                        (128, 480)
                    ),
                )

            return run

        def c_const():
            nc.gpsimd.dma_start(out=pw_sb, in_=pw.rearrange("c p o -> p c o"))

        # ---- pools ----
        xtp = ctx.enter_context(tc.tile_pool(name="xtp", bufs=10))
        trig = ctx.enter_context(tc.tile_pool(name="trig", bufs=1))
        qkp = ctx.enter_context(tc.tile_pool(name="qkp", bufs=2))
        t2p = ctx.enter_context(tc.tile_pool(name="t2p", bufs=2))
        vp = ctx.enter_context(tc.tile_pool(name="vp", bufs=1))
        qtp = ctx.enter_context(tc.tile_pool(name="qtp", bufs=4))
        ktp = ctx.enter_context(tc.tile_pool(name="ktp", bufs=4))
        expp = ctx.enter_context(tc.tile_pool(name="expp", bufs=4))
        ddp = ctx.enter_context(tc.tile_pool(name="ddp", bufs=2))
        rbp = ctx.enter_context(tc.tile_pool(name="rbp", bufs=2))
        stg = ctx.enter_context(tc.tile_pool(name="stg", bufs=20))
        otmp = ctx.enter_context(tc.tile_pool(name="otmp", bufs=3))

        qkvps = ctx.enter_context(tc.tile_pool(name="qkvps", bufs=2, space="PSUM"))
        scps = ctx.enter_context(tc.tile_pool(name="scps", bufs=2, space="PSUM"))
        pvps = ctx.enter_context(tc.tile_pool(name="pvps", bufs=2, space="PSUM"))
        prps = ctx.enter_context(tc.tile_pool(name="prps", bufs=2, space="PSUM"))

        # PE warmup (see _build)
        scratch = cpool.tile([128, 512], dt_a, tag="scratch", name="scratch")
        nc.gpsimd.memset(scratch, 0.0)
        for w in range(6):
            wp = scps.tile([128, 128], f32, tag="sc", name=f"warm_{w}")
            nc.tensor.matmul(out=wp, lhsT=scratch[:, 0:128], rhs=scratch[:, 0:128])
        for w in range(12):
            wp = scps.tile([128, 512], f32, tag="sc", name=f"warmw_{w}")
            nc.tensor.matmul(out=wp, lhsT=scratch[:, 0:128], rhs=scratch)

        v_tiles = {}
        qT_tiles = {}
        kT_tiles = {}
        xts = []
        st_trig = {}

        def load_x():
            # startup is single-queue at full HBM share: hg0's remaining
            # weight chunks are woven into the x stream (splitting them to
            # other queues costs x two-thirds of the bandwidth).
            for k in range(10):
                xt_t = xtp.tile([128, BS], dt_qkv, tag="xt", name=f"xt_{k}")
                nc.sync.dma_start(out=xt_t, in_=xt[k * 128 : (k + 1) * 128, :])
                xts.append(xt_t)

        def c_trig():
            cos_t = trig.tile([128, 8, D], dt_a, tag="cos", name="cos")
            nc.sync.dma_start(
                out=cos_t, in_=cosd.rearrange("p (c d) -> p c d", c=8)
            )
            sin_t = trig.tile([128, 8, D], dt_a, tag="sin", name="sin")
            nc.sync.dma_start(
                out=sin_t, in_=sind.rearrange("p (c d) -> p c d", c=8)
            )
            st_trig["cos"] = cos_t
            st_trig["sin"] = sin_t

        def qkv_chunks(hg):
            st = {}
            st["qk"] = qkp.tile([128, 8, 320], dt_a, tag="qk", name=f"qk_{hg}")

            def c_mm(m):
                def run():
                    qk_blk = st["qk"]
                    ps = qkvps.tile(
                        [128, 480], f32, tag="qkvps", name=f"qkvps_{hg}_{m}"
                    )
                    for k in range(10):
                        nc.tensor.matmul(
                            out=ps,
                            lhsT=xts[k][:, m * 128 : (m + 1) * 128],
                            rhs=wt_sb[:, hg, k, :],
                            start=(k == 0),
                            stop=(k == 9),
                        )
                    bb = bias_bc[:, hg * 480 : (hg + 1) * 480]
                    nc.vector.tensor_add(
                        out=qk_blk[:, m, :], in0=ps[:, 0:320], in1=bb[:, 0:320]
                    )
                    v_t = vp.tile(
                        [128, 2, 97], dt_a, tag="v", name=f"v_{hg}_{m}", bufs=16
                    )
                    nc.vector.tensor_add(
                        out=v_t[:, :, 0:D],
                        in0=ps[:, 320:480].rearrange("p (h d) -> p h d", h=2),
                        in1=bb[:, 320:480].rearrange("p (h d) -> p h d", h=2),
                    )
                    nc.gpsimd.memset(v_t[:, :, D:97], 1.0)
                    v_tiles[(hg, m)] = v_t

                return run

            def rot_dve(half, tau, h):
                def run():
                    qk_blk = st["qk"]
                    sin_t = st_trig["sin"]
                    cos_t = st_trig["cos"]
                    base = tau * 160 + h * D
                    ms = slice(half * 4, half * 4 + 4)
                    sl = qk_blk[:, ms, base : base + D]
                    t2 = t2p.tile(
                        [128, 4, D], dt_a, tag="t2",
                        name=f"t2_{hg}_{half}_{tau}_{h}",
                    )
                    nc.vector.tensor_mul(
                        out=t2[:, :, 0:40],
                        in0=qk_blk[:, ms, base + 40 : base + D],
                        in1=sin_t[:, ms, 0:40],
                    )
                    nc.vector.tensor_mul(
                        out=t2[:, :, 40:D],
                        in0=qk_blk[:, ms, base : base + 40],
                        in1=sin_t[:, ms, 40:D],
                    )
                    nc.vector.tensor_mul(out=sl, in0=sl, in1=cos_t[:, ms, :])
                    nc.vector.tensor_add(out=sl, in0=sl, in1=t2)

                return run

            def rot_pe(half, tau, h):
                def run():
                    qk_blk = st["qk"]
                    base = tau * 160 + h * D
                    g = half
                    if half == 0:
                        if tau == 0:
                            qT_tiles[(hg, h)] = qtp.tile(
                                [D, BS], dt_a, tag="qt", name=f"qT_{hg}_{h}"
                            )
                        else:
                            kT_tiles[(hg, h)] = ktp.tile(
                                [D, BS], dt_a, tag="kt", name=f"kT_{hg}_{h}"
                            )
                    pool = qkvps if half == 0 else scps
                    tag = "qkvps" if half == 0 else "sc"
                    tp = pool.tile(
                        [D, 512], dt_a, tag=tag, name=f"tr_{hg}_{half}_{tau}_{h}"
                    )
                    for j in range(4):
                        m = g * 4 + j
                        nc.tensor.matmul(
                            out=tp[:, j * 128 : (j + 1) * 128],
                            lhsT=qk_blk[:, m, base : base + D],
                            rhs=ident,
                            is_transpose=True,
                            start=(j == 0),
                            stop=(j == 3),
                        )
                    tiles = qT_tiles if tau == 0 else kT_tiles
                    dst = tiles[(hg, h)][:, g * 512 : (g + 1) * 512]
                    nc.vector.tensor_copy(out=dst, in_=tp)

                return run

            return {
                "mm": [c_mm(m) for m in range(8)],
                "rdve": {
                    (hf, tau, h): rot_dve(hf, tau, h)
                    for hf in range(2)
                    for tau in range(2)
                    for h in range(NHL)
                },
                "rpe": {
                    (hf, tau, h): rot_pe(hf, tau, h)
                    for hf in range(2)
                    for tau in range(2)
                    for h in range(NHL)
                },
            }

        def emit_proj_o(sh, o, copy_scalar):
            sts_t = stg.tile([128, 512], out_dt, tag="st", name=f"st_{sh}_{o}")
            use_scalar = copy_scalar and o % 2
            pool, tag = (qkvps, "qkvps") if o % 2 else (prps, "pr")
            pp = pool.tile([128, 512], f32, tag=tag, name=f"pr_{sh}_{o}")
            for c in range(5):
                nc.tensor.matmul(
                    out=pp,
                    lhsT=pw_sb[:, c, o * 128 : (o + 1) * 128],
                    rhs=ot_stack[c][:, sh * 512 : (sh + 1) * 512],
                    start=(c == 0),
                    stop=(c == 4),
                )
            if use_scalar:
                nc.scalar.copy(out=sts_t, in_=pp)
            else:
                nc.vector.tensor_copy(out=sts_t, in_=pp)
            eng = (nc.sync, nc.gpsimd, nc.scalar)[o % 3] if copy_scalar else (
                nc.gpsimd if o % 2 else nc.sync
            )
            eng.dma_start(
                out=outp[
                    o * 128 : (o + 1) * 128, sh * 512 : (sh + 1) * 512
                ],
                in_=sts_t,
            )

        def proj_pair(sh, o, copy_scalar=False):
            def run():
                emit_proj_o(sh, o, copy_scalar)

            return run

        fillq = []

        def attn_chunks(hg):
            def c_attn(sh, h, use_fill=False):
                def run():
                    qT_t = qT_tiles[(hg, h)]
                    kT_t = kT_tiles[(hg, h)]
                    pv = pvps.tile(
                        [97, 512], f32, tag="pv", name=f"pv_{hg}_{h}_{sh}"
                    )

                    def score(i):
                        scp = scps.tile(
                            [128, 512], f32, tag="sc", name=f"sc_{hg}_{h}_{sh}_{i}"
                        )
                        nc.tensor.matmul(
                            out=scp,
                            lhsT=kT_t[:, i * 128 : (i + 1) * 128],
                            rhs=qT_t[:, sh * 512 : (sh + 1) * 512],
                            start=True,
                            stop=True,
                        )
                        ep = expp.tile(
                            [128, 512], dt_a, tag="exp",
                            name=f"ep_{hg}_{h}_{sh}_{i}",
                        )
                        nc.scalar.activation(
                            out=ep, in_=scp, func=EXP, scale=SCALING
                        )
                        return ep

                    def pvmm(i, ep):
                        nc.tensor.matmul(
                            out=pv,
                            lhsT=v_tiles[(hg, i)][:, h, :],
                            rhs=ep,
                            start=(i == 0),
                            stop=(i == 7),
                        )

                    pend = []
                    for i in range(8):
                        ep = score(i)
                        pend.append((i, ep))
                        if i >= 2:
                            j, epj = pend.pop(0)
                            pvmm(j, epj)
                            if use_fill and fillq:
                                fillq.pop(0)()
                    while pend:
                        j, epj = pend.pop(0)
                        pvmm(j, epj)
                        if use_fill and fillq:
                            fillq.pop(0)()
                    dd = ddp.tile([1, 512], f32, tag="dd", name=f"dd_{hg}_{h}_{sh}")
                    nc.scalar.copy(out=dd, in_=pv[96:97, :])
                    rbc = rbp.tile(
                        [D, 512], f32, tag="rbc", name=f"rbc_{hg}_{h}_{sh}"
                    )
                    nc.gpsimd.partition_broadcast(rbc, dd)
                    rb = rbp.tile([D, 512], f32, tag="rb", name=f"rb_{hg}_{h}_{sh}")
                    nc.vector.reciprocal_approx_fast(out=rb, in_=rbc)
                    # place into the stacked [128, BS] ot tiles at the head
                    # global row offset. DVE lanes are hard-wired to
                    # partitions, so a partition-shifted placement needs an
                    # SBUF->SBUF DMA hop (split across two tiles if the
                    # head straddles a 128 boundary).
                    g = hg * 2 + h
                    r0 = (g * D) % 128
                    t0 = (g * D) // 128
                    cs = slice(sh * 512, (sh + 1) * 512)
                    if r0 == 0:
                        nc.vector.tensor_mul(
                            out=ot_stack[t0][0:D, cs], in0=pv[0:D, :], in1=rb
                        )
                    else:
                        ott = otmp.tile(
                            [D, 512], dt_a, tag="otm", name=f"otm_{hg}_{h}_{sh}"
                        )
                        nc.vector.tensor_mul(out=ott, in0=pv[0:D, :], in1=rb)
                        n1 = min(128 - r0, D)
                        eng = nc.gpsimd if (g + sh) % 2 else nc.sync
                        eng.dma_start(
                            out=ot_stack[t0][r0 : r0 + n1, cs], in_=ott[0:n1, :]
                        )
                        if n1 < D:
                            eng.dma_start(
                                out=ot_stack[t0 + 1][0 : D - n1, cs],
                                in_=ott[n1:D, :],
                            )

                return run

            if hg == NHG - 1:
                chunks = []
                for h in range(NHL):
                    r = c_attn(0, h)
                    r.is_attn = True
                    chunks.append(r)
                chunks.append(
                    lambda: fillq.extend(proj_pair(0, o) for o in range(10))
                )
                for h in range(NHL):
                    r = c_attn(1, h, use_fill=True)
                    r.is_attn = True
                    chunks.append(r)
                chunks.append(lambda: [f() for f in fillq])
                chunks.append(lambda: fillq.clear())
                chunks.append(
                    lambda: [
                        emit_proj_o(1, o, copy_scalar=True) for o in range(10)
                    ]
                )
                return chunks
            chunks = []
            for sh in range(2):
                for h in range(NHL):
                    r = c_attn(sh, h)
                    r.is_attn = True
                    chunks.append(r)
            return chunks

        ROT0 = [(0, 1, 0), (0, 1, 1), (0, 0, 0), (0, 0, 1)]
        ROT1 = [(1, 1, 0), (1, 1, 1), (1, 0, 0), (1, 0, 1)]

        def mid_list(q):
            return [
                q["mm"][4], q["rdve"][ROT0[0]], q["mm"][5], q["rpe"][ROT0[0]],
                q["rdve"][ROT0[1]], q["mm"][6], q["rpe"][ROT0[1]],
                q["rdve"][ROT0[2]], q["mm"][7], q["rpe"][ROT0[2]],
                q["rdve"][ROT0[3]], q["rpe"][ROT0[3]],
            ]

        def seam_dummy():
            wp = prps.tile([128, 512], f32, tag="pr", name=f"sd_{len(sdn)}")
            sdn.append(0)
            nc.tensor.matmul(out=wp, lhsT=scratch[:, 0:128], rhs=scratch)

        sdn = []
        load_x()
        cur = qkv_chunks(0)
        c_wrest()
        for m in range(2):
            cur["mm"][m]()
        c_trig()
        for m in range(2, 4):
            cur["mm"][m]()
        mids = mid_list(cur)
        mids[0]()
        c_wnext(1)()
        for c in mids[1:]:
            c()
        for hg in range(NHG):
            at = attn_chunks(hg)
            if hg + 1 < NHG:
                nxt = qkv_chunks(hg + 1)
                seam_fill = [nxt["mm"][m] for m in range(4)]
            else:
                nxt = None
                seam_fill = [seam_dummy, seam_dummy, seam_dummy, seam_dummy]
            for u in ROT1:
                cur["rdve"][u]()
            for f, u in zip(seam_fill, ROT1):
                f()
                cur["rpe"][u]()
            filler = mid_list(nxt) if nxt is not None else []
            if hg == 0:
                filler = [c_wnext(2)] + filler
            elif hg == 1:
                filler = [c_wnext(3), c_const] + filler
            attn_pos = [
                i for i, a in enumerate(at) if getattr(a, "is_attn", False)
            ]
            k = 0
            j = 0
            for i, a in enumerate(at):
                a()
                if i in attn_pos:
                    j += 1
                    take = (len(filler) * j) // max(1, len(attn_pos)) - k
                    for _ in range(take):
                        filler[k]()
                        k += 1
            while k < len(filler):
                filler[k]()
                k += 1
            cur = nxt

    nc.compile()
    return nc


def _analyze_mask(mask):
    m = np.asarray(mask).reshape(S, S)
    allowed = []
    mask_add = set()
    for qb in range(NB):
        row = []
        for kb in range(NB):
            t = m[qb * BS : (qb + 1) * BS, kb * BS : (kb + 1) * BS]
            if np.all(t <= NEG_THRESH):
                continue
            row.append(kb)
            if not np.all(t == 0.0):
                mask_add.add((qb, kb))
        if not row:
            raise NotImplementedError("fully masked query block")
        allowed.append(tuple(row))
    return tuple(allowed), frozenset(mask_add)


def _np_dt(name):
    if name == "bfloat16":
        import ml_dtypes

        return ml_dtypes.bfloat16
    return np.float32


def kernel(
    hidden_states, attention_mask, cos, sin, qkv_w, qkv_b, proj_w, proj_b
):
    from concourse import bass_utils

    qkv_dt = os.environ.get("KERNEL_QKV_DT", "bfloat16")
    attn_dt = os.environ.get("KERNEL_ATTN_DT", "bfloat16")
    out_dt = os.environ.get("KERNEL_OUT_DT", "bfloat16")
    trace = bool(int(os.environ.get("KERNEL_TRACE", "0")))

    global LAST_RESULT

    X = np.ascontiguousarray(np.asarray(hidden_states, dtype=np.float32))
    allowed, mask_add = _analyze_mask(attention_mask)

    key = (qkv_dt, attn_dt, out_dt, allowed, mask_add)
    if key not in _CACHE:
        _CACHE[key] = _build(
            allowed, mask_add, qkv_dt_name=qkv_dt, attn_dt_name=attn_dt,
            out_dt_name=out_dt,
        )
    nc = _CACHE[key]

    np_qkv = _np_dt(qkv_dt)
    np_attn = _np_dt(attn_dt)

    XT = np.ascontiguousarray(X.T).astype(np_qkv)
    cos = np.ascontiguousarray(np.asarray(cos, dtype=np.float32))
    sin = np.asarray(sin, dtype=np.float32)
    sinh = np.ascontiguousarray(
        np.concatenate([-sin[:, : D // 2], sin[:, D // 2 :]], axis=1)
    )
    qkv_w = np.asarray(qkv_w, dtype=np.float32)

    def trig_prep(a):
        # [S, D] -> [NB*128, 8*D]: tile (b, p, c, d) = a[b*1024 + c*128 + p, d]
        return np.ascontiguousarray(
            a.reshape(NB, 8, 128, D).transpose(0, 2, 1, 3).reshape(NB * 128, 8 * D)
        )
    qkv_b = np.asarray(qkv_b, dtype=np.float32)
    proj_w = np.asarray(proj_w, dtype=np.float32)
    proj_b = np.asarray(proj_b, dtype=np.float32)

    diag = (
        all(tuple(allowed[b]) == (b,) for b in range(NB)) and not mask_add
    )
    if diag and os.environ.get("KERNEL_V4", "0") == "1":
        key = ("v4", qkv_dt, attn_dt, out_dt)
        if key not in _CACHE:
            _CACHE[key] = _build_v4(
                qkv_dt_name=qkv_dt, attn_dt_name=attn_dt, out_dt_name=out_dt
            )
        nc4 = _CACHE[key]
        cos_p = trig_prep(cos.astype(np_attn))
        sin_p = trig_prep(sinh.astype(np_attn))
        in_maps = []
        for c in range(NCORES):
            b = c // 2
            hh = c % 2
            wgs, bgs = [], []
            for hg in range(4):
                j0 = (hh * 8 + hg * 2) * D
                sl = slice(j0, j0 + 2 * D)
                Wg = np.concatenate(
                    [qkv_w[sl], qkv_w[HID:][sl], qkv_w[2 * HID :][sl]], axis=0
                )
                # [1280, 480] -> SBUF layout [128, 10*480]
                wgs.append(
                    Wg.T.reshape(10, 128, 480).transpose(1, 0, 2).reshape(
                        128, 4800
                    )
                )
                bgs.append(
                    np.concatenate(
                        [qkv_b[sl], qkv_b[HID:][sl], qkv_b[2 * HID :][sl]]
                    )
                )
            pwT = proj_w[:, hh * 640 : (hh + 1) * 640].T
            in_maps.append({
                "xt": np.ascontiguousarray(XT[:, b * BS : (b + 1) * BS]),
                "wt": np.ascontiguousarray(np.stack(wgs)).astype(np_qkv),
                "bqkv": np.ascontiguousarray(
                    np.concatenate(bgs)[None, :].astype(np.float32)
                ),
                "cosd": np.ascontiguousarray(cos_p[b * 128 : (b + 1) * 128]),
                "sind": np.ascontiguousarray(sin_p[b * 128 : (b + 1) * 128]),
                "pw": np.ascontiguousarray(pwT.reshape(5, 128, HID)).astype(
                    np_attn
                ),
            })
        res = bass_utils.run_bass_kernel_spmd(
            nc4, in_maps, core_ids=list(range(NCORES)), trace=trace
        )
        LAST_RESULT = res
        acc = np.zeros((HID, S), dtype=np.float64)
        for c in range(NCORES):
            b = c // 2
            acc[:, b * BS : (b + 1) * BS] += np.asarray(
                res.results[c]["outp"], dtype=np.float64
            )
        out = acc.T + proj_b.astype(np.float64)[None, :]
        return out.astype(np.float32)

    in_maps = []
    for c in range(NCORES):
        j0 = c * NHL * D
        sl = slice(j0, j0 + NHL * D)
        Wc = np.concatenate(
            [qkv_w[sl], qkv_w[HID:][sl], qkv_w[2 * HID :][sl]], axis=0
        )
        m = {
            "xt": XT,
            "wt": np.ascontiguousarray(Wc.T).astype(np_qkv),
            "bqkv": np.ascontiguousarray(
                np.concatenate([qkv_b[sl], qkv_b[HID:][sl], qkv_b[2 * HID :][sl]])[
                    None, :
                ]
            ),
            "cosd": trig_prep(cos.astype(np_attn)),
            "sind": trig_prep(sinh.astype(np_attn)),
            "pw": np.ascontiguousarray(
                np.stack(
                    [
                        proj_w[:, j0 : j0 + D].T,
                        proj_w[:, j0 + D : j0 + 2 * D].T,
                    ]
                )
            ).astype(np_attn),
        }
        if mask_add:
            m["maskt"] = np.ascontiguousarray(
                (np.asarray(attention_mask).reshape(S, S).T / SCALING).astype(
                    np.float32
                )
            )
        in_maps.append(m)

    res = bass_utils.run_bass_kernel_spmd(
        nc, in_maps, core_ids=list(range(NCORES)), trace=trace
    )
    LAST_RESULT = res

    acc = np.zeros((HID, S), dtype=np.float64)
    for c in range(NCORES):
        acc += np.asarray(res.results[c]["outp"], dtype=np.float64)
    out = acc.T + proj_b.astype(np.float64)[None, :]
    return out.astype(np.float32)


LAST_RESULT = None



# revision 60
# speedup vs baseline: 1.0180x; 1.0180x over previous
"""Qwen3.5 vision attention (S=4096, H=16x80, block-diag mask) on 8 trn2 cores.

Sharding: tensor-parallel over heads (2 heads/core). Each core computes
qkv projection for its heads, rotary, block-sparse attention, and a partial
output projection (RowParallel); the host sums the 8 partials (all-reduce)
and adds proj_b.

v2 (390 -> 238 us): bf16 everywhere off the PSUM paths, on-chip softmax
normalization (no DRAM round-trip), on-chip V ones-column, staged output
stores.

v3 (238 -> ~210 us) — PE (tensor engine) is the bottleneck at ~85% busy;
every change targets PE stalls and the HAM clock gate (PE runs at 1.2GHz
until ~3.4us of sustained activity, and re-throttles after idle windows):
  - rotary is split into per-(token-half, q/k, head) units; the half-0
    units interleave with qkv matmuls mid-phase, and at the block seam
    the half-1 DVE work is queued ahead of the next block's first qkv
    matmuls so the (in-order) PE always has filler in front of the
    rotary-dependent transposes. Seam transposes use the scores PSUM
    bank, which is idle exactly then.
  - x for block b+1 prefetches during block b's qkv phase (the 2.6MB
    DMA no longer starves the attention-phase filler).
  - attention chains run scores two chunks ahead of PV in the PE queue:
    PV_i waits exp_i while score_{i+2} is bounded by the scps WAR on
    exp_i, so the chain advances at the scalar exp rate with the scalar
    engine saturated instead of ping-ponging at ~2x that period.
  - sh1 projs of blocks 0-2 are deferred into the final block's seam and
    attention phase as per-chunk pair filler (the only dense PE work
    available there), and the final block's own sh0 proj pairs join the
    queue mid-phase. Late projs alternate between the proj and (idle)
    qkv PSUM pools so 4 matmul pairs run ahead of the psum->sbuf copies.
  - softmax denominator broadcast moved off the PE (rank-1 matmul) to
    gpsimd.partition_broadcast.
  - startup: weight chunks split across sync+scalar DMA queues ahead of /
    parallel with block-0 x; cos/sin host-pretransposed for contiguous
    DMA; self-pacing warmup matmuls cover the preamble+DMA window so the
    first real matmul runs at full clock.
"""

import os
from contextlib import ExitStack

import numpy as np

S = 4096
HID = 1280
D = 80
NB = 4
BS = 1024
NHL = 2  # heads per core
NCORES = 8
SCALING = float(D) ** -0.5
NEG_THRESH = -1e8

_CACHE = {}


def _build(allowed, mask_add, qkv_dt_name="bfloat16", attn_dt_name="bfloat16",
           out_dt_name="bfloat16"):
    """Build + compile the per-core bass module.

    allowed: tuple over qb of tuple of kb blocks attended to.
    mask_add: frozenset of (qb, kb) needing an additive mask tile.
    """
    import concourse.bass as bass
    import concourse.mybir as mybir
    import concourse.tile as tile
    from concourse import bacc
    from concourse.masks import make_identity

    f32 = mybir.dt.float32
    f32r = mybir.dt.float32r
    dt_qkv = getattr(mybir.dt, qkv_dt_name)
    dt_a = getattr(mybir.dt, attn_dt_name)
    out_dt = getattr(mybir.dt, out_dt_name)
    use_mask = len(mask_add) > 0

    nc = bacc.Bacc(
        "TRN2", target_bir_lowering=False, debug=False, num_devices=NCORES
    )
    xt = nc.dram_tensor("xt", [HID, S], dt_qkv, kind="ExternalInput").ap()
    wt = nc.dram_tensor("wt", [HID, 480], dt_qkv, kind="ExternalInput").ap()
    bqkv = nc.dram_tensor("bqkv", [1, 480], f32, kind="ExternalInput").ap()
    # host pre-rearranged to [p, c, d] tiles so the DMA is contiguous
    cosd = nc.dram_tensor("cosd", [NB * 128, 8 * D], dt_a, kind="ExternalInput").ap()
    sind = nc.dram_tensor("sind", [NB * 128, 8 * D], dt_a, kind="ExternalInput").ap()
    pw = nc.dram_tensor("pw", [2, D, HID], dt_a, kind="ExternalInput").ap()
    if use_mask:
        maskt = nc.dram_tensor("maskt", [S, S], f32, kind="ExternalInput").ap()
    outp = nc.dram_tensor("outp", [HID, S], out_dt, kind="ExternalOutput").ap()

    EXP = mybir.ActivationFunctionType.Exp
    interleave = all(tuple(allowed[b]) == (b,) for b in range(NB))

    with ExitStack() as ctx:
        tc = ctx.enter_context(tile.TileContext(nc))

        # ---- constants ----
        cpool = ctx.enter_context(tc.tile_pool(name="cpool", bufs=1))
        wt_sb = cpool.tile([128, 10, 480], dt_qkv, tag="wt_sb", name="wt_sb")
        wt_r = wt.rearrange("(kk p) c -> p kk c", p=128)
        # first two contraction chunks go ahead of everything else on the
        # sync queue; block 0's x tiles follow immediately so the first
        # qkv k-loop starts as soon as x_0 lands (~6.5us). The remaining
        # weight chunks, bias, and proj weights stream on the gpsimd queue
        # in parallel.
        nc.sync.dma_start(out=wt_sb[:, 0:2, :], in_=wt_r[:, 0:2, :])
        bias_bc = cpool.tile([128, 480], f32, tag="bias_bc", name="bias_bc")
        ident = cpool.tile([128, 128], dt_a, tag="ident", name="ident")
        make_identity(nc, ident)
        pw_sb = cpool.tile([D, 2, HID], dt_a, tag="pw_sb", name="pw_sb")

        def c_wrest():
            nc.scalar.dma_start(out=wt_sb[:, 2:6, :], in_=wt_r[:, 2:6, :])
            nc.scalar.dma_start(out=wt_sb[:, 6:10, :], in_=wt_r[:, 6:10, :])
            nc.gpsimd.dma_start(
                out=bias_bc, in_=bqkv[0:1, :].to_broadcast((128, 480))
            )

        def c_const():
            # deferred: not needed until the first attention/proj phase
            nc.gpsimd.dma_start(out=pw_sb, in_=pw.rearrange("h d o -> d h o"))

        kT_sb = [
            cpool.tile([D, S], dt_a, tag=f"kT{h}_sb", name=f"kT{h}_sb")
            for h in range(NHL)
        ]

        # ---- pools ----
        xtp = ctx.enter_context(tc.tile_pool(name="xtp", bufs=20))
        trig = ctx.enter_context(tc.tile_pool(name="trig", bufs=2))
        qkp = ctx.enter_context(tc.tile_pool(name="qkp", bufs=2))
        t2p = ctx.enter_context(tc.tile_pool(name="t2p", bufs=2))
        vp = ctx.enter_context(tc.tile_pool(name="vp", bufs=1))
        qtp = ctx.enter_context(
            tc.tile_pool(name="qtp", bufs=4 if interleave else 8)
        )
        expp = ctx.enter_context(tc.tile_pool(name="expp", bufs=4))
        # ot tiles of blocks NB-3/NB-2 stay alive until their deferred
        # sh1 proj runs during the final block -> all 4 blocks' ot tiles
        # can be live at once.
        otp = ctx.enter_context(tc.tile_pool(name="otp", bufs=8))
        ddp = ctx.enter_context(tc.tile_pool(name="ddp", bufs=2))
        rbp = ctx.enter_context(tc.tile_pool(name="rbp", bufs=2))
        stg = ctx.enter_context(tc.tile_pool(name="stg", bufs=20))
        if use_mask:
            mtp = ctx.enter_context(tc.tile_pool(name="mtp", bufs=4))

        # PSUM: 8 banks total. qkv accumulation and the rotary transposes
        # share one tag (they alternate within the qkv phase).
        qkvps = ctx.enter_context(tc.tile_pool(name="qkvps", bufs=2, space="PSUM"))
        scps = ctx.enter_context(tc.tile_pool(name="scps", bufs=2, space="PSUM"))
        pvps = ctx.enter_context(tc.tile_pool(name="pvps", bufs=2, space="PSUM"))
        prps = ctx.enter_context(tc.tile_pool(name="prps", bufs=2, space="PSUM"))

        # PE warmup: the HAM clock gate only releases full rate after ~3.4us
        # of sustained matmul activity, and the first real matmul waits on
        # the whole block-0 x DMA watermark (~13us). These throwaway
        # matmuls on a zeroed tile cover the 8..13us window so the real
        # qkv starts at full clock.
        scratch = cpool.tile([128, 512], dt_a, tag="scratch", name="scratch")
        nc.gpsimd.memset(scratch, 0.0)
        for w in range(6):
            wp = scps.tile([128, 128], f32, tag="sc", name=f"warm_{w}")
            nc.tensor.matmul(out=wp, lhsT=scratch[:, 0:128], rhs=scratch[:, 0:128])
        # wide warmups self-pace at ~640ns through the sem chain, covering
        # the stretch until the block-0 x watermark (~17us) with few
        # instructions.
        for w in range(12):
            wp = scps.tile([128, 512], f32, tag="sc", name=f"warmw_{w}")
            nc.tensor.matmul(out=wp, lhsT=scratch[:, 0:128], rhs=scratch)

        v_tiles = {}
        qT_tiles = {}
        xts_map = {}

        def load_x(b):
            # x of block b must be resident BEFORE block b-1's attention
            # begins (its qkv matmuls are the filler there), so this is
            # emitted at the start of block b-1's qkv phase: the 10 DMAs
            # (~7us) overlap that whole phase instead of starving the
            # attention filler. For block 0 the remaining weight chunks are
            # woven between the early x tiles on the same queue so the
            # first k-loop never stalls on a weight arriving behind all
            # of x.
            xts = []
            for k in range(10):
                xt_t = xtp.tile([128, BS], dt_qkv, tag="xt", name=f"xt_{b}_{k}")
                nc.sync.dma_start(
                    out=xt_t,
                    in_=xt[k * 128 : (k + 1) * 128, b * BS : (b + 1) * BS],
                )
                xts.append(xt_t)
            xts_map[b] = xts

        def qkv_chunks(b):
            """Emission chunks for the qkv+rotary+transpose phase of block b.

            Returned as closures so they can be zipped between the previous
            block's attention chunks: alternating bass_priority makes the
            tile scheduler fill the exp-bound attention stretches with qkv
            matmuls.
            """
            st = {}
            # qk tile allocated at qkv_chunks() call time (seam start) so
            # the first c_mm does not depend on the trig DMA emission.
            st["qk"] = qkp.tile([128, 8, 320], dt_a, tag="qk", name=f"qk_{b}")

            def c_load():
                cos_t = trig.tile([128, 8, D], dt_a, tag="cos", name=f"cos_{b}")
                nc.sync.dma_start(
                    out=cos_t,
                    in_=cosd[b * 128 : (b + 1) * 128, :].rearrange(
                        "p (c d) -> p c d", c=8
                    ),
                )
                sin_t = trig.tile([128, 8, D], dt_a, tag="sin", name=f"sin_{b}")
                nc.sync.dma_start(
                    out=sin_t,
                    in_=sind[b * 128 : (b + 1) * 128, :].rearrange(
                        "p (c d) -> p c d", c=8
                    ),
                )
                st["cos"] = cos_t
                st["sin"] = sin_t

            def c_mm(m):
                def run():
                    qk_blk = st["qk"]
                    ps = qkvps.tile(
                        [128, 480], f32, tag="qkvps", name=f"qkvps_{b}_{m}"
                    )
                    for k in range(10):
                        nc.tensor.matmul(
                            out=ps,
                            lhsT=xts_map[b][k][:, m * 128 : (m + 1) * 128],
                            rhs=wt_sb[:, k, :],
                            start=(k == 0),
                            stop=(k == 9),
                        )
                    nc.vector.tensor_add(
                        out=qk_blk[:, m, :], in0=ps[:, 0:320], in1=bias_bc[:, 0:320]
                    )
                    v_t = vp.tile(
                        [128, 2, 97], dt_a, tag="v", name=f"v_{b}_{m}",
                        bufs=16 if interleave else 32,
                    )
                    nc.vector.tensor_add(
                        out=v_t[:, :, 0:D],
                        in0=ps[:, 320:480].rearrange("p (h d) -> p h d", h=2),
                        in1=bias_bc[:, 320:480].rearrange("p (h d) -> p h d", h=2),
                    )
                    nc.gpsimd.memset(v_t[:, :, D:97], 1.0)
                    v_tiles[(b, m)] = v_t

                return run

            def rot_dve(half, tau, h):
                # rotary multiply-adds for one (token-half, q/k, head) unit.
                # Split from the transposes so the DVE work can be queued
                # ahead of PE filler matmuls at the block seam: the PE is
                # strictly in-order, so a transpose waiting on DVE rotary
                # blocks every later matmul in its queue.
                def run():
                    qk_blk = st["qk"]
                    sin_t = st["sin"]
                    cos_t = st["cos"]
                    base = tau * 160 + h * D
                    ms = slice(half * 4, half * 4 + 4)
                    sl = qk_blk[:, ms, base : base + D]
                    t2 = t2p.tile(
                        [128, 4, D], dt_a, tag="t2", name=f"t2_{b}_{half}_{tau}_{h}"
                    )
                    nc.vector.tensor_mul(
                        out=t2[:, :, 0:40],
                        in0=qk_blk[:, ms, base + 40 : base + D],
                        in1=sin_t[:, ms, 0:40],
                    )
                    nc.vector.tensor_mul(
                        out=t2[:, :, 40:D],
                        in0=qk_blk[:, ms, base : base + 40],
                        in1=sin_t[:, ms, 40:D],
                    )
                    nc.vector.tensor_mul(out=sl, in0=sl, in1=cos_t[:, ms, :])
                    nc.vector.tensor_add(out=sl, in0=sl, in1=t2)

                return run

            def rot_pe(half, tau, h):
                def run():
                    qk_blk = st["qk"]
                    base = tau * 160 + h * D
                    g = half
                    if tau == 0 and half == 0:
                        dst_t = qtp.tile([D, BS], dt_a, tag="qt", name=f"qT_{b}_{h}")
                        qT_tiles[(b, h)] = dst_t
                    # half-1 transposes run at the seam where the scores
                    # PSUM bank is idle; half-0 shares the qkv bank.
                    pool = qkvps if half == 0 else scps
                    tag = "qkvps" if half == 0 else "sc"
                    tp = pool.tile(
                        [D, 512], dt_a, tag=tag, name=f"tr_{b}_{half}_{tau}_{h}"
                    )
                    for j in range(4):
                        m = g * 4 + j
                        nc.tensor.matmul(
                            out=tp[:, j * 128 : (j + 1) * 128],
                            lhsT=qk_blk[:, m, base : base + D],
                            rhs=ident,
                            is_transpose=True,
                            start=(j == 0),
                            stop=(j == 3),
                        )
                    if tau == 0:
                        dst = qT_tiles[(b, h)][:, g * 512 : (g + 1) * 512]
                    else:
                        dst = kT_sb[h][
                            :, b * BS + g * 512 : b * BS + (g + 1) * 512
                        ]
                    nc.vector.tensor_copy(out=dst, in_=tp)

                return run

            return {
                "load": c_load,
                "mm": [c_mm(m) for m in range(8)],
                "rdve": {
                    (hf, tau, h): rot_dve(hf, tau, h)
                    for hf in range(2)
                    for tau in range(2)
                    for h in range(NHL)
                },
                "rpe": {
                    (hf, tau, h): rot_pe(hf, tau, h)
                    for hf in range(2)
                    for tau in range(2)
                    for h in range(NHL)
                },
            }

        def attn_chunks(b):
            kbs = allowed[b]
            nmm = len(kbs) * 8
            st = {}

            def c_alloc():
                st["ots"] = [
                    otp.tile([D, BS], dt_a, tag="ot", name=f"ot_{b}_{h}")
                    for h in range(NHL)
                ]

            def c_attn(sh, h, fillq=None):
                def run():
                    qT_t = qT_tiles[(b, h)]
                    ot_t = st["ots"][h]
                    pv = pvps.tile(
                        [97, 512], f32, tag="pv", name=f"pv_{b}_{h}_{sh}"
                    )
                    items = [(kb, t) for kb in kbs for t in range(8)]

                    def score(i):
                        kb, t = items[i]
                        scp = scps.tile(
                            [128, 512], f32, tag="sc",
                            name=f"sc_{b}_{h}_{sh}_{kb}_{t}",
                        )
                        nc.tensor.matmul(
                            out=scp,
                            lhsT=kT_sb[h][
                                :, kb * BS + t * 128 : kb * BS + (t + 1) * 128
                            ],
                            rhs=qT_t[:, sh * 512 : (sh + 1) * 512],
                            start=True,
                            stop=True,
                        )
                        if (b, kb) in mask_add:
                            mk = mtp.tile(
                                [128, 512], f32, tag="mk",
                                name=f"mk_{b}_{h}_{sh}_{kb}_{t}",
                            )
                            nc.sync.dma_start(
                                out=mk,
                                in_=maskt[
                                    kb * BS + t * 128 : kb * BS + (t + 1) * 128,
                                    b * BS + sh * 512 : b * BS + (sh + 1) * 512,
                                ],
                            )
                            nc.vector.tensor_add(out=scp, in0=scp, in1=mk)
                        ep = expp.tile(
                            [128, 512], dt_a, tag="exp",
                            name=f"ep_{b}_{h}_{sh}_{kb}_{t}",
                        )
                        nc.scalar.activation(
                            out=ep, in_=scp, func=EXP, scale=SCALING
                        )
                        return ep

                    def pvmm(i, ep):
                        kb, t = items[i]
                        nc.tensor.matmul(
                            out=pv,
                            lhsT=v_tiles[(kb, t)][:, h, :],
                            rhs=ep,
                            start=(i == 0),
                            stop=(i == nmm - 1),
                        )

                    # scores run two chunks ahead of pv in the (in-order) PE
                    # queue: pv_i waits on exp_i, and score_{i+2} waits on
                    # exp_i via the scps WAR, so the chain advances at the
                    # scalar exp rate with the scalar engine saturated.
                    # fillq items (deferred proj pairs) slot in after each
                    # pv to absorb the remaining exp slack.
                    pend = []
                    for i in range(len(items)):
                        ep = score(i)
                        pend.append((i, ep))
                        if i >= 2:
                            j, epj = pend.pop(0)
                            pvmm(j, epj)
                            if fillq:
                                fillq.pop(0)()
                    while pend:
                        j, epj = pend.pop(0)
                        pvmm(j, epj)
                        if fillq:
                            fillq.pop(0)()
                    # normalize: dd = denom row (scalar copy, psum->sbuf),
                    # broadcast across the 80 partitions on the (idle)
                    # gpsimd engine, invert with the fast custom-DVE
                    # reciprocal, ot = pv * rb.
                    dd = ddp.tile(
                        [1, 512], f32, tag="dd", name=f"dd_{b}_{h}_{sh}"
                    )
                    nc.scalar.copy(out=dd, in_=pv[96:97, :])
                    rbc = rbp.tile(
                        [D, 512], f32, tag="rbc", name=f"rbc_{b}_{h}_{sh}"
                    )
                    nc.gpsimd.partition_broadcast(rbc, dd)
                    rb = rbp.tile([D, 512], f32, tag="rb", name=f"rb_{b}_{h}_{sh}")
                    nc.vector.reciprocal_approx_fast(out=rb, in_=rbc)
                    nc.vector.tensor_mul(
                        out=ot_t[:, sh * 512 : (sh + 1) * 512],
                        in0=pv[0:D, :],
                        in1=rb,
                    )

                return run

            def emit_proj_o(sh, o, late, copy_scalar):
                ots = st["ots"]
                sts_t = stg.tile(
                    [128, 512], out_dt, tag="st", name=f"st_{b}_{sh}_{o}"
                )
                if copy_scalar and o % 2:
                    # scalar path (exp-idle stretches): fp32 accumulate +
                    # ACT copy, keeping the work off the busy DVE.
                    pp = qkvps.tile(
                        [128, 512], f32, tag="qkvps", name=f"pr_{b}_{sh}_{o}"
                    )
                    for hh in range(2):
                        nc.tensor.matmul(
                            out=pp,
                            lhsT=pw_sb[:, hh, o * 128 : (o + 1) * 128],
                            rhs=ots[hh][:, sh * 512 : (sh + 1) * 512],
                            start=(hh == 0),
                            stop=(hh == 1),
                        )
                    nc.scalar.copy(out=sts_t, in_=pp)
                    if b == NB - 1:
                        wp = pvps.tile(
                            [128, 128], f32, tag="pv", name=f"tw_{b}_{sh}_{o}"
                        )
                        nc.tensor.matmul(
                            out=wp,
                            lhsT=scratch[:, 0:128],
                            rhs=scratch[:, 0:128],
                        )
                else:
                    # late projs run when qkv is idle: alternate the idle
                    # qkv PSUM banks in so 4 matmul pairs can run ahead of
                    # the psum->sbuf copies.
                    if (late or b == NB - 1) and o % 2:
                        pp = qkvps.tile(
                            [128, 512], f32, tag="qkvps", name=f"pr_{b}_{sh}_{o}"
                        )
                    else:
                        pp = prps.tile(
                            [128, 512], f32, tag="pr", name=f"pr_{b}_{sh}_{o}"
                        )
                    for hh in range(2):
                        nc.tensor.matmul(
                            out=pp,
                            lhsT=pw_sb[:, hh, o * 128 : (o + 1) * 128],
                            rhs=ots[hh][:, sh * 512 : (sh + 1) * 512],
                            start=(hh == 0),
                            stop=(hh == 1),
                        )
                    nc.vector.tensor_copy(out=sts_t, in_=pp)
                if copy_scalar:
                    eng = (nc.sync, nc.gpsimd, nc.scalar)[o % 3]
                else:
                    eng = nc.gpsimd if o % 2 else nc.sync
                eng.dma_start(
                    out=outp[
                        o * 128 : (o + 1) * 128,
                        b * BS + sh * 512 : b * BS + (sh + 1) * 512,
                    ],
                    in_=sts_t,
                )

            def c_proj(sh, o_lo=0, o_hi=10, late=False, copy_scalar=False):
                def run():
                    for o in range(o_lo, o_hi):
                        emit_proj_o(sh, o, late, copy_scalar)

                return run

            def proj_pair(sh, o, late=False):
                def run():
                    emit_proj_o(sh, o, late, False)

                return run

            if b == NB - 1:
                # final block: deferred proj pairs of blocks NB-3/NB-2 are
                # consumed one pair per attention chunk inside the units
                # (fillq), absorbing the per-chunk exp slack; the block's
                # own sh0 proj pairs join the queue once both sh0 units are
                # done. sh1 proj runs at the very end with scalar copies
                # (exp is finished by then).
                chunks = [c_alloc]
                for h in range(NHL):
                    r = c_attn(0, h, fillq=deferred_pairs)
                    r.is_attn = True
                    chunks.append(r)
                chunks.append(
                    lambda: deferred_pairs.extend(
                        proj_pair(0, o) for o in range(10)
                    )
                )
                for h in range(NHL):
                    r = c_attn(1, h, fillq=deferred_pairs)
                    r.is_attn = True
                    chunks.append(r)
                chunks.append(lambda: [f() for f in deferred_pairs])
                chunks.append(lambda: deferred_pairs.clear())
                chunks.append(c_proj(1, copy_scalar=True))
                return chunks

            chunks = [c_alloc]
            for sh in range(2):
                for h in range(NHL):
                    r = c_attn(sh, h)
                    r.is_attn = True
                    chunks.append(r)
                # sh1 projs of blocks NB-4..NB-2 are deferred into the last
                # block's seam + attention phase: they are the only dense PE
                # filler available there (no next-block qkv remains), keeping
                # the HAM clock warm through the tail. Block NB-4's unit is
                # split in half to zip with the seam transposes; its copies
                # go to the scalar engine, which is exp-idle at the seam.
                if sh == 1 and b == NB - 4:
                    deferred.append(c_proj(sh, 0, 5, late=True, copy_scalar=True))
                    deferred.append(c_proj(sh, 5, 10, late=True, copy_scalar=True))
                elif sh == 1 and b in (NB - 3, NB - 2):
                    deferred_pairs.extend(
                        proj_pair(sh, o, late=True) for o in range(10)
                    )
                else:
                    # the last two o-chunks of each sh0 proj also go to the
                    # final block's fill queue: 30 pairs cover only 30 of
                    # its 32 attention-chunk slots, so the last unit runs
                    # dry without these.
                    chunks.append(c_proj(sh, 0, 8))
                    deferred_pairs.extend(
                        proj_pair(sh, o, late=True) for o in range(8, 10)
                    )
            return chunks

        # rotary unit order: k heads first (scores need the full kT block),
        # q heads after (only the matching sh-half of qT is needed early)
        ROT0 = [(0, 1, 0), (0, 1, 1), (0, 0, 0), (0, 0, 1)]
        ROT1 = [(1, 1, 0), (1, 1, 1), (1, 0, 0), (1, 0, 1)]

        def mid_list(q):
            # qkv mid-phase of a block: mm4..7 with the half-0 rotary units
            # woven in so each transpose lands behind a dense mm stretch.
            return [
                q["mm"][4], q["rdve"][ROT0[0]], q["mm"][5], q["rpe"][ROT0[0]],
                q["rdve"][ROT0[1]], q["mm"][6], q["rpe"][ROT0[1]],
                q["rdve"][ROT0[2]], q["mm"][7], q["rpe"][ROT0[2]],
                q["rdve"][ROT0[3]], q["rpe"][ROT0[3]],
            ]

        if interleave:
            # Software-pipelined emission. Per block b:
            #   seam:      half-1 rotary DVE queued first, its transposes
            #              zipped between block b+1's first qkv matmuls
            #              (in-order PE filler for the DVE wait)
            #   attention: zipped with block b+1's qkv mid-phase
            # The deferred sh1 projs of blocks NB-3/NB-2 play the role of
            # the "next qkv" for the final block.
            deferred = []
            deferred_pairs = []
            load_x(0)
            cur = qkv_chunks(0)
            c_wrest()
            for m in range(2):
                cur["mm"][m]()
            cur["load"]()
            c_const()
            for m in range(2, 4):
                cur["mm"][m]()
            load_x(1)
            for c in mid_list(cur):
                c()
            for b in range(NB):
                at = attn_chunks(b)
                if b + 1 < NB:
                    nxt = qkv_chunks(b + 1)
                    nxt["load"]()
                    if b + 2 < NB:
                        load_x(b + 2)
                    seam_fill = [nxt["mm"][m] for m in range(4)]
                else:
                    nxt = None
                    seam_fill = deferred[:2]
                for u in ROT1:
                    cur["rdve"][u]()
                if nxt is not None:
                    for f, u in zip(seam_fill, ROT1):
                        f()
                        cur["rpe"][u]()
                else:
                    seam_fill[0]()
                    cur["rpe"][ROT1[0]]()
                    cur["rpe"][ROT1[1]]()
                    seam_fill[1]()
                    cur["rpe"][ROT1[2]]()
                    cur["rpe"][ROT1[3]]()
                filler = mid_list(nxt) if nxt is not None else deferred[2:]
                # distribute filler only after the exp-bound attention
                # chunks (proj chunks are already tensor-dense)
                attn_pos = [
                    i for i, a in enumerate(at) if getattr(a, "is_attn", False)
                ]
                k = 0
                j = 0
                for i, a in enumerate(at):
                    a()
                    if i in attn_pos:
                        j += 1
                        take = (len(filler) * j) // max(1, len(attn_pos)) - k
                        for _ in range(take):
                            filler[k]()
                            k += 1
                while k < len(filler):
                    filler[k]()
                    k += 1
                cur = nxt
        else:
            deferred = []
            deferred_pairs = []
            c_wrest()
            c_const()
            load_x(0)
            for b in range(NB):
                if b + 1 < NB:
                    load_x(b + 1)
                q = qkv_chunks(b)
                q["load"]()
                for m in range(8):
                    q["mm"][m]()
                for u in ROT0 + ROT1:
                    q["rdve"][u]()
                    q["rpe"][u]()
            for b in range(NB):
                for c in attn_chunks(b):
                    c()
            for c in deferred:
                c()
            for c in deferred_pairs:
                c()

    nc.compile()
    return nc


def _build_v4(qkv_dt_name="bfloat16", attn_dt_name="bfloat16",
              out_dt_name="bfloat16"):
    """Block-sharded build: core c handles sequence block c//2 (1024
    tokens) and head-half c%2 (8 heads, 4 head-groups of 2). Attention is
    identical per head-group, but the output projection now contracts over
    640 dims (5 full 128-chunks) instead of 2x80 -> 10x2x5x512 cycles for
    a quarter of the output area, saving ~12.8us of PE streaming per core,
    and the kernel ends in a dense exp-free proj GEMM.
    Only valid for the exact block-diagonal mask (no mask adds).
    """
    import concourse.mybir as mybir
    import concourse.tile as tile
    from concourse import bacc
    from concourse.masks import make_identity

    f32 = mybir.dt.float32
    dt_qkv = getattr(mybir.dt, qkv_dt_name)
    dt_a = getattr(mybir.dt, attn_dt_name)
    out_dt = getattr(mybir.dt, out_dt_name)
    NHG = 4  # head groups of 2 per core

    nc = bacc.Bacc(
        "TRN2", target_bir_lowering=False, debug=False, num_devices=NCORES
    )
    xt = nc.dram_tensor("xt", [HID, BS], dt_qkv, kind="ExternalInput").ap()
    # host pre-arranged to the SBUF tile layout so every weight DMA is
    # contiguous per partition (strided weight loads run at ~50GB/s and
    # starve the x stream)
    wt = nc.dram_tensor(
        "wt", [NHG, 128, 10 * 480], dt_qkv, kind="ExternalInput"
    ).ap()
    bqkv = nc.dram_tensor("bqkv", [1, NHG * 480], f32, kind="ExternalInput").ap()
    cosd = nc.dram_tensor("cosd", [128, 8 * D], dt_a, kind="ExternalInput").ap()
    sind = nc.dram_tensor("sind", [128, 8 * D], dt_a, kind="ExternalInput").ap()
    pw = nc.dram_tensor("pw", [5, 128, HID], dt_a, kind="ExternalInput").ap()
    outp = nc.dram_tensor("outp", [HID, BS], out_dt, kind="ExternalOutput").ap()

    EXP = mybir.ActivationFunctionType.Exp

    with ExitStack() as ctx:
        tc = ctx.enter_context(tile.TileContext(nc))

        cpool = ctx.enter_context(tc.tile_pool(name="cpool", bufs=1))
        wt_sb = cpool.tile([128, NHG, 10, 480], dt_qkv, tag="wt_sb", name="wt_sb")
        # hg0 weights ahead of x on sync/scalar; hg1-3 behind on gpsimd
        wt_r0 = wt[0].rearrange("p (kk c) -> p kk c", kk=10)
        nc.sync.dma_start(out=wt_sb[:, 0, 0:2, :], in_=wt_r0[:, 0:2, :])
        bias_bc = cpool.tile([128, NHG * 480], f32, tag="bias_bc", name="bias_bc")
        ident = cpool.tile([128, 128], dt_a, tag="ident", name="ident")
        make_identity(nc, ident)
        pw_sb = cpool.tile([128, 5, HID], dt_a, tag="pw_sb", name="pw_sb")
        ot_stack = [
            cpool.tile([128, BS], dt_a, tag=f"ot{c}", name=f"ot{c}")
            for c in range(5)
        ]

        def c_wrest():
            nc.scalar.dma_start(out=wt_sb[:, 0, 2:10, :], in_=wt_r0[:, 2:10, :])
            nc.gpsimd.dma_start(
                out=bias_bc[:, 0:480],
                in_=bqkv[0:1, 0:480].to_broadcast((128, 480)),
            )

        def c_wnext(hg):
            # one head-group of weights (1.2MB), loaded one phase before
            # its qkv so the DMA never competes with block-0 x at startup
            def run():
                nc.gpsimd.dma_start(
                    out=wt_sb[:, hg, :, :],
                    in_=wt[hg].rearrange("p (kk c) -> p kk c", kk=10),
                )
                nc.gpsimd.dma_start(
                    out=bias_bc[:, hg * 480 : (hg + 1) * 480],
                    in_=bqkv[0:1, hg * 480 : (hg + 1) * 480].to_broadcast(
                        (128, 480)
                    ),
                )

            return run

        def c_const():
            nc.gpsimd.dma_start(out=pw_sb, in_=pw.rearrange("c p o -> p c o"))

        # ---- pools ----
        xtp = ctx.enter_context(tc.tile_pool(name="xtp", bufs=10))
        trig = ctx.enter_context(tc.tile_pool(name="trig", bufs=1))
        qkp = ctx.enter_context(tc.tile_pool(name="qkp", bufs=2))
        t2p = ctx.enter_context(tc.tile_pool(name="t2p", bufs=2))
        vp = ctx.enter_context(tc.tile_pool(name="vp", bufs=1))
        qtp = ctx.enter_context(tc.tile_pool(name="qtp", bufs=4))
        ktp = ctx.enter_context(tc.tile_pool(name="ktp", bufs=4))
        expp = ctx.enter_context(tc.tile_pool(name="expp", bufs=4))
        ddp = ctx.enter_context(tc.tile_pool(name="ddp", bufs=2))
        rbp = ctx.enter_context(tc.tile_pool(name="rbp", bufs=2))
        stg = ctx.enter_context(tc.tile_pool(name="stg", bufs=20))
        otmp = ctx.enter_context(tc.tile_pool(name="otmp", bufs=3))

        qkvps = ctx.enter_context(tc.tile_pool(name="qkvps", bufs=2, space="PSUM"))
        scps = ctx.enter_context(tc.tile_pool(name="scps", bufs=2, space="PSUM"))
        pvps = ctx.enter_context(tc.tile_pool(name="pvps", bufs=2, space="PSUM"))
        prps = ctx.enter_context(tc.tile_pool(name="prps", bufs=2, space="PSUM"))

        # PE warmup (see _build)
        scratch = cpool.tile([128, 512], dt_a, tag="scratch", name="scratch")
        nc.gpsimd.memset(scratch, 0.0)
        for w in range(6):
            wp = scps.tile([128, 128], f32, tag="sc", name=f"warm_{w}")
            nc.tensor.matmul(out=wp, lhsT=scratch[:, 0:128], rhs=scratch[:, 0:128])
        for w in range(12):
            wp = scps.tile([128, 512], f32, tag="sc", name=f"warmw_{w}")
            nc.tensor.matmul(out=wp, lhsT=scratch[:, 0:128], rhs=scratch)

        v_tiles = {}
        qT_tiles = {}
        kT_tiles = {}
        xts = []
        st_trig = {}

        def load_x():
            # startup is single-queue at full HBM share: hg0's remaining
            # weight chunks are woven into the x stream (splitting them to
            # other queues costs x two-thirds of the bandwidth).
            for k in range(10):
                xt_t = xtp.tile([128, BS], dt_qkv, tag="xt", name=f"xt_{k}")
                nc.sync.dma_start(out=xt_t, in_=xt[k * 128 : (k + 1) * 128, :])
                xts.append(xt_t)

        def c_trig():
            cos_t = trig.tile([128, 8, D], dt_a, tag="cos", name="cos")
            nc.sync.dma_start(
                out=cos_t, in_=cosd.rearrange("p (c d) -> p c d", c=8)
            )
            sin_t = trig.tile([128, 8, D], dt_a, tag="sin", name="sin")
            nc.sync.dma_start(
                out=sin_t, in_=sind.rearrange("p (c d) -> p c d", c=8)
            )
            st_trig["cos"] = cos_t
            st_trig["sin"] = sin_t

        def qkv_chunks(hg):
            st = {}
            st["qk"] = qkp.tile([128, 8, 320], dt_a, tag="qk", name=f"qk_{hg}")

            def c_mm(m):
                def run():
                    qk_blk = st["qk"]
                    ps = qkvps.tile(
                        [128, 480], f32, tag="qkvps", name=f"qkvps_{hg}_{m}"
                    )
                    for k in range(10):
                        nc.tensor.matmul(
                            out=ps,
                            lhsT=xts[k][:, m * 128 : (m + 1) * 128],
                            rhs=wt_sb[:, hg, k, :],
                            start=(k == 0),
                            stop=(k == 9),
                        )
                    bb = bias_bc[:, hg * 480 : (hg + 1) * 480]
                    nc.vector.tensor_add(
                        out=qk_blk[:, m, :], in0=ps[:, 0:320], in1=bb[:, 0:320]
                    )
                    v_t = vp.tile(
                        [128, 2, 97], dt_a, tag="v", name=f"v_{hg}_{m}", bufs=16
                    )
                    nc.vector.tensor_add(
                        out=v_t[:, :, 0:D],
                        in0=ps[:, 320:480].rearrange("p (h d) -> p h d", h=2),
                        in1=bb[:, 320:480].rearrange("p (h d) -> p h d", h=2),
                    )
                    nc.gpsimd.memset(v_t[:, :, D:97], 1.0)
                    v_tiles[(hg, m)] = v_t

                return run

            def rot_dve(half, tau, h):
                def run():
                    qk_blk = st["qk"]
                    sin_t = st_trig["sin"]
                    cos_t = st_trig["cos"]
                    base = tau * 160 + h * D
                    ms = slice(half * 4, half * 4 + 4)
                    sl = qk_blk[:, ms, base : base + D]
                    t2 = t2p.tile(
                        [128, 4, D], dt_a, tag="t2",
                        name=f"t2_{hg}_{half}_{tau}_{h}",
                    )
                    nc.vector.tensor_mul(
                        out=t2[:, :, 0:40],
                        in0=qk_blk[:, ms, base + 40 : base + D],
                        in1=sin_t[:, ms, 0:40],
                    )
                    nc.vector.tensor_mul(
                        out=t2[:, :, 40:D],
                        in0=qk_blk[:, ms, base : base + 40],
                        in1=sin_t[:, ms, 40:D],
                    )
                    nc.vector.tensor_mul(out=sl, in0=sl, in1=cos_t[:, ms, :])
                    nc.vector.tensor_add(out=sl, in0=sl, in1=t2)

                return run

            def rot_pe(half, tau, h):
                def run():
                    qk_blk = st["qk"]
                    base = tau * 160 + h * D
                    g = half
                    if half == 0:
                        if tau == 0:
                            qT_tiles[(hg, h)] = qtp.tile(
                                [D, BS], dt_a, tag="qt", name=f"qT_{hg}_{h}"
                            )
                        else:
                            kT_tiles[(hg, h)] = ktp.tile(
                                [D, BS], dt_a, tag="kt", name=f"kT_{hg}_{h}"
                            )
                    pool = qkvps if half == 0 else scps
                    tag = "qkvps" if half == 0 else "sc"
                    tp = pool.tile(
                        [D, 512], dt_a, tag=tag, name=f"tr_{hg}_{half}_{tau}_{h}"
                    )
                    for j in range(4):
                        m = g * 4 + j
                        nc.tensor.matmul(
                            out=tp[:, j * 128 : (j + 1) * 128],
                            lhsT=qk_blk[:, m, base : base + D],
                            rhs=ident,
                            is_transpose=True,
                            start=(j == 0),
                            stop=(j == 3),
                        )
                    tiles = qT_tiles if tau == 0 else kT_tiles
                    dst = tiles[(hg, h)][:, g * 512 : (g + 1) * 512]
                    nc.vector.tensor_copy(out=dst, in_=tp)

                return run

            return {
                "mm": [c_mm(m) for m in range(8)],
                "rdve": {
                    (hf, tau, h): rot_dve(hf, tau, h)
                    for hf in range(2)
                    for tau in range(2)
                    for h in range(NHL)
                },
                "rpe": {
                    (hf, tau, h): rot_pe(hf, tau, h)
                    for hf in range(2)
                    for tau in range(2)
                    for h in range(NHL)
                },
            }

        def emit_proj_o(sh, o, copy_scalar):
            sts_t = stg.tile([128, 512], out_dt, tag="st", name=f"st_{sh}_{o}")
            use_scalar = copy_scalar and o % 2
            pool, tag = (qkvps, "qkvps") if o % 2 else (prps, "pr")
            pp = pool.tile([128, 512], f32, tag=tag, name=f"pr_{sh}_{o}")
            for c in range(5):
                nc.tensor.matmul(
                    out=pp,
                    lhsT=pw_sb[:, c, o * 128 : (o + 1) * 128],
                    rhs=ot_stack[c][:, sh * 512 : (sh + 1) * 512],
                    start=(c == 0),
                    stop=(c == 4),
                )
            if use_scalar:
                nc.scalar.copy(out=sts_t, in_=pp)
            else:
                nc.vector.tensor_copy(out=sts_t, in_=pp)
            eng = (nc.sync, nc.gpsimd, nc.scalar)[o % 3] if copy_scalar else (
                nc.gpsimd if o % 2 else nc.sync
            )
            eng.dma_start(
                out=outp[
                    o * 128 : (o + 1) * 128, sh * 512 : (sh + 1) * 512
                ],
                in_=sts_t,
            )

        def proj_pair(sh, o, copy_scalar=False):
            def run():
                emit_proj_o(sh, o, copy_scalar)

            return run

        fillq = []

        def attn_chunks(hg):
            def c_attn(sh, h, use_fill=False):
                def run():
                    qT_t = qT_tiles[(hg, h)]
                    kT_t = kT_tiles[(hg, h)]
                    pv = pvps.tile(
                        [97, 512], f32, tag="pv", name=f"pv_{hg}_{h}_{sh}"
                    )

                    def score(i):
                        scp = scps.tile(
                            [128, 512], f32, tag="sc", name=f"sc_{hg}_{h}_{sh}_{i}"
                        )
                        nc.tensor.matmul(
                            out=scp,
                            lhsT=kT_t[:, i * 128 : (i + 1) * 128],
                            rhs=qT_t[:, sh * 512 : (sh + 1) * 512],
                            start=True,
                            stop=True,
                        )
                        ep = expp.tile(
                            [128, 512], dt_a, tag="exp",
                            name=f"ep_{hg}_{h}_{sh}_{i}",
                        )
                        nc.scalar.activation(
                            out=ep, in_=scp, func=EXP, scale=SCALING
                        )
                        return ep

                    def pvmm(i, ep):
                        nc.tensor.matmul(
                            out=pv,
                            lhsT=v_tiles[(hg, i)][:, h, :],
                            rhs=ep,
                            start=(i == 0),
                            stop=(i == 7),
                        )

                    pend = []
                    for i in range(8):
                        ep = score(i)
                        pend.append((i, ep))
                        if i >= 2:
                            j, epj = pend.pop(0)
                            pvmm(j, epj)
                            if use_fill and fillq:
                                fillq.pop(0)()
                    while pend:
                        j, epj = pend.pop(0)
                        pvmm(j, epj)
                        if use_fill and fillq:
                            fillq.pop(0)()
                    dd = ddp.tile([1, 512], f32, tag="dd", name=f"dd_{hg}_{h}_{sh}")
                    nc.scalar.copy(out=dd, in_=pv[96:97, :])
                    rbc = rbp.tile(
                        [D, 512], f32, tag="rbc", name=f"rbc_{hg}_{h}_{sh}"
                    )
                    nc.gpsimd.partition_broadcast(rbc, dd)
                    rb = rbp.tile([D, 512], f32, tag="rb", name=f"rb_{hg}_{h}_{sh}")
                    nc.vector.reciprocal_approx_fast(out=rb, in_=rbc)
                    # place into the stacked [128, BS] ot tiles at the head
                    # global row offset. DVE lanes are hard-wired to
                    # partitions, so a partition-shifted placement needs an
                    # SBUF->SBUF DMA hop (split across two tiles if the
                    # head straddles a 128 boundary).
                    g = hg * 2 + h
                    r0 = (g * D) % 128
                    t0 = (g * D) // 128
                    cs = slice(sh * 512, (sh + 1) * 512)
                    if r0 == 0:
                        nc.vector.tensor_mul(
                            out=ot_stack[t0][0:D, cs], in0=pv[0:D, :], in1=rb
                        )
                    else:
                        ott = otmp.tile(
                            [D, 512], dt_a, tag="otm", name=f"otm_{hg}_{h}_{sh}"
                        )
                        nc.vector.tensor_mul(out=ott, in0=pv[0:D, :], in1=rb)
                        n1 = min(128 - r0, D)
                        eng = nc.gpsimd if (g + sh) % 2 else nc.sync
                        eng.dma_start(
                            out=ot_stack[t0][r0 : r0 + n1, cs], in_=ott[0:n1, :]
                        )
                        if n1 < D:
                            eng.dma_start(
                                out=ot_stack[t0 + 1][0 : D - n1, cs],
                                in_=ott[n1:D, :],
                            )

                return run

            if hg == NHG - 1:
                chunks = []
                for h in range(NHL):
                    r = c_attn(0, h)
                    r.is_attn = True
                    chunks.append(r)
                chunks.append(
                    lambda: fillq.extend(proj_pair(0, o) for o in range(10))
                )
                for h in range(NHL):
                    r = c_attn(1, h, use_fill=True)
                    r.is_attn = True
                    chunks.append(r)
                chunks.append(lambda: [f() for f in fillq])
                chunks.append(lambda: fillq.clear())
                chunks.append(
                    lambda: [
                        emit_proj_o(1, o, copy_scalar=True) for o in range(10)
                    ]
                )
                return chunks
            chunks = []
            for sh in range(2):
                for h in range(NHL):
                    r = c_attn(sh, h)
                    r.is_attn = True
                    chunks.append(r)
            return chunks

        ROT0 = [(0, 1, 0), (0, 1, 1), (0, 0, 0), (0, 0, 1)]
        ROT1 = [(1, 1, 0), (1, 1, 1), (1, 0, 0), (1, 0, 1)]

        def mid_list(q):
            return [
                q["mm"][4], q["rdve"][ROT0[0]], q["mm"][5], q["rpe"][ROT0[0]],
                q["rdve"][ROT0[1]], q["mm"][6], q["rpe"][ROT0[1]],
                q["rdve"][ROT0[2]], q["mm"][7], q["rpe"][ROT0[2]],
                q["rdve"][ROT0[3]], q["rpe"][ROT0[3]],
            ]

        def seam_dummy():
            wp = prps.tile([128, 512], f32, tag="pr", name=f"sd_{len(sdn)}")
            sdn.append(0)
            nc.tensor.matmul(out=wp, lhsT=scratch[:, 0:128], rhs=scratch)

        sdn = []
        load_x()
        cur = qkv_chunks(0)
        c_wrest()
        for m in range(2):
            cur["mm"][m]()
        c_trig()
        for m in range(2, 4):
            cur["mm"][m]()
        mids = mid_list(cur)
        mids[0]()
        c_wnext(1)()
        for c in mids[1:]:
            c()
        for hg in range(NHG):
            at = attn_chunks(hg)
            if hg + 1 < NHG:
                nxt = qkv_chunks(hg + 1)
                seam_fill = [nxt["mm"][m] for m in range(4)]
            else:
                nxt = None
                seam_fill = [seam_dummy, seam_dummy, seam_dummy, seam_dummy]
            for u in ROT1:
                cur["rdve"][u]()
            for f, u in zip(seam_fill, ROT1):
                f()
                cur["rpe"][u]()
            filler = mid_list(nxt) if nxt is not None else []
            if hg == 0:
                filler = [c_wnext(2)] + filler
            elif hg == 1:
                filler = [c_wnext(3), c_const] + filler
            attn_pos = [
                i for i, a in enumerate(at) if getattr(a, "is_attn", False)
            ]
            k = 0
            j = 0
            for i, a in enumerate(at):
                a()
                if i in attn_pos:
                    j += 1
                    take = (len(filler) * j) // max(1, len(attn_pos)) - k
                    for _ in range(take):
                        filler[k]()
                        k += 1
            while k < len(filler):
                filler[k]()
                k += 1
            cur = nxt

    nc.compile()
    return nc


def _analyze_mask(mask):
    m = np.asarray(mask).reshape(S, S)
    allowed = []
    mask_add = set()
    for qb in range(NB):
        row = []
        for kb in range(NB):
            t = m[qb * BS : (qb + 1) * BS, kb * BS : (kb + 1) * BS]
            if np.all(t <= NEG_THRESH):
                continue
            row.append(kb)
            if not np.all(t == 0.0):
                mask_add.add((qb, kb))
        if not row:
            raise NotImplementedError("fully masked query block")
        allowed.append(tuple(row))
    return tuple(allowed), frozenset(mask_add)


def _np_dt(name):
    if name == "bfloat16":
        import ml_dtypes

        return ml_dtypes.bfloat16
    return np.float32


def kernel(
    hidden_states, attention_mask, cos, sin, qkv_w, qkv_b, proj_w, proj_b
):
    from concourse import bass_utils

    qkv_dt = os.environ.get("KERNEL_QKV_DT", "bfloat16")
    attn_dt = os.environ.get("KERNEL_ATTN_DT", "bfloat16")
    out_dt = os.environ.get("KERNEL_OUT_DT", "bfloat16")
    trace = bool(int(os.environ.get("KERNEL_TRACE", "0")))

    global LAST_RESULT

    X = np.ascontiguousarray(np.asarray(hidden_states, dtype=np.float32))
    allowed, mask_add = _analyze_mask(attention_mask)

    key = (qkv_dt, attn_dt, out_dt, allowed, mask_add)
    if key not in _CACHE:
        _CACHE[key] = _build(
            allowed, mask_add, qkv_dt_name=qkv_dt, attn_dt_name=attn_dt,
            out_dt_name=out_dt,
        )
    nc = _CACHE[key]

    np_qkv = _np_dt(qkv_dt)
    np_attn = _np_dt(attn_dt)

    XT = np.ascontiguousarray(X.T).astype(np_qkv)
    cos = np.ascontiguousarray(np.asarray(cos, dtype=np.float32))
    sin = np.asarray(sin, dtype=np.float32)
    sinh = np.ascontiguousarray(
        np.concatenate([-sin[:, : D // 2], sin[:, D // 2 :]], axis=1)
    )
    qkv_w = np.asarray(qkv_w, dtype=np.float32)

    def trig_prep(a):
        # [S, D] -> [NB*128, 8*D]: tile (b, p, c, d) = a[b*1024 + c*128 + p, d]
        return np.ascontiguousarray(
            a.reshape(NB, 8, 128, D).transpose(0, 2, 1, 3).reshape(NB * 128, 8 * D)
        )
    qkv_b = np.asarray(qkv_b, dtype=np.float32)
    proj_w = np.asarray(proj_w, dtype=np.float32)
    proj_b = np.asarray(proj_b, dtype=np.float32)

    diag = (
        all(tuple(allowed[b]) == (b,) for b in range(NB)) and not mask_add
    )
    if diag and os.environ.get("KERNEL_V4", "0") == "1":
        key = ("v4", qkv_dt, attn_dt, out_dt)
        if key not in _CACHE:
            _CACHE[key] = _build_v4(
                qkv_dt_name=qkv_dt, attn_dt_name=attn_dt, out_dt_name=out_dt
            )
        nc4 = _CACHE[key]
        cos_p = trig_prep(cos.astype(np_attn))
        sin_p = trig_prep(sinh.astype(np_attn))
        in_maps = []
        for c in range(NCORES):
            b = c // 2
            hh = c % 2
            wgs, bgs = [], []
            for hg in range(4):
                j0 = (hh * 8 + hg * 2) * D
                sl = slice(j0, j0 + 2 * D)
                Wg = np.concatenate(
                    [qkv_w[sl], qkv_w[HID:][sl], qkv_w[2 * HID :][sl]], axis=0
                )
                # [1280, 480] -> SBUF layout [128, 10*480]
                wgs.append(
                    Wg.T.reshape(10, 128, 480).transpose(1, 0, 2).reshape(
                        128, 4800
                    )
                )
                bgs.append(
                    np.concatenate(
                        [qkv_b[sl], qkv_b[HID:][sl], qkv_b[2 * HID :][sl]]
                    )
                )
            pwT = proj_w[:, hh * 640 : (hh + 1) * 640].T
            in_maps.append({
                "xt": np.ascontiguousarray(XT[:, b * BS : (b + 1) * BS]),
                "wt": np.ascontiguousarray(np.stack(wgs)).astype(np_qkv),
                "bqkv": np.ascontiguousarray(
                    np.concatenate(bgs)[None, :].astype(np.float32)
                ),
                "cosd": np.ascontiguousarray(cos_p[b * 128 : (b + 1) * 128]),
                "sind": np.ascontiguousarray(sin_p[b * 128 : (b + 1) * 128]),
                "pw": np.ascontiguousarray(pwT.reshape(5, 128, HID)).astype(
                    np_attn
                ),
            })
        res = bass_utils.run_bass_kernel_spmd(
            nc4, in_maps, core_ids=list(range(NCORES)), trace=trace
        )
        LAST_RESULT = res
        acc = np.zeros((HID, S), dtype=np.float64)
        for c in range(NCORES):
            b = c // 2
            acc[:, b * BS : (b + 1) * BS] += np.asarray(
                res.results[c]["outp"], dtype=np.float64
            )
        out = acc.T + proj_b.astype(np.float64)[None, :]
        return out.astype(np.float32)

    in_maps = []
    for c in range(NCORES):
        j0 = c * NHL * D
        sl = slice(j0, j0 + NHL * D)
        Wc = np.concatenate(
            [qkv_w[sl], qkv_w[HID:][sl], qkv_w[2 * HID :][sl]], axis=0
        )
        m = {
            "xt": XT,
            "wt": np.ascontiguousarray(Wc.T).astype(np_qkv),
            "bqkv": np.ascontiguousarray(
                np.concatenate([qkv_b[sl], qkv_b[HID:][sl], qkv_b[2 * HID :][sl]])[
                    None, :
                ]
            ),
            "cosd": trig_prep(cos.astype(np_attn)),
            "sind": trig_prep(sinh.astype(np_attn)),
            "pw": np.ascontiguousarray(
                np.stack(
                    [
                        proj_w[:, j0 : j0 + D].T,
                        proj_w[:, j0 + D : j0 + 2 * D].T,
                    ]
                )
            ).astype(np_attn),
        }
        if mask_add:
            m["maskt"] = np.ascontiguousarray(
                (np.asarray(attention_mask).reshape(S, S).T / SCALING).astype(
                    np.float32
                )
            )
        in_maps.append(m)

    res = bass_utils.run_bass_kernel_spmd(
        nc, in_maps, core_ids=list(range(NCORES)), trace=trace
    )
    LAST_RESULT = res

    acc = np.zeros((HID, S), dtype=np.float64)
    for c in range(NCORES):
        acc += np.asarray(res.results[c]["outp"], dtype=np.float64)
    out = acc.T + proj_b.astype(np.float64)[None, :]
    return out.astype(np.float32)


LAST_RESULT = None



# revision 61
# speedup vs baseline: 1.0215x; 1.0035x over previous
"""Qwen3.5 vision attention (S=4096, H=16x80, block-diag mask) on 8 trn2 cores.

Sharding: tensor-parallel over heads (2 heads/core). Each core computes
qkv projection for its heads, rotary, block-sparse attention, and a partial
output projection (RowParallel); the host sums the 8 partials (all-reduce)
and adds proj_b.

v2 (390 -> 238 us): bf16 everywhere off the PSUM paths, on-chip softmax
normalization (no DRAM round-trip), on-chip V ones-column, staged output
stores.

v3 (238 -> ~210 us) — PE (tensor engine) is the bottleneck at ~85% busy;
every change targets PE stalls and the HAM clock gate (PE runs at 1.2GHz
until ~3.4us of sustained activity, and re-throttles after idle windows):
  - rotary is split into per-(token-half, q/k, head) units; the half-0
    units interleave with qkv matmuls mid-phase, and at the block seam
    the half-1 DVE work is queued ahead of the next block's first qkv
    matmuls so the (in-order) PE always has filler in front of the
    rotary-dependent transposes. Seam transposes use the scores PSUM
    bank, which is idle exactly then.
  - x for block b+1 prefetches during block b's qkv phase (the 2.6MB
    DMA no longer starves the attention-phase filler).
  - attention chains run scores two chunks ahead of PV in the PE queue:
    PV_i waits exp_i while score_{i+2} is bounded by the scps WAR on
    exp_i, so the chain advances at the scalar exp rate with the scalar
    engine saturated instead of ping-ponging at ~2x that period.
  - sh1 projs of blocks 0-2 (plus the last two sh0 o-chunks of each) are
    deferred into the final block's seam and attention phase as per-chunk
    pair filler (the only dense PE work available there), and the final
    block's own sh0 proj pairs join the queue mid-phase. Late projs alternate between the proj and (idle)
    qkv PSUM pools so 4 matmul pairs run ahead of the psum->sbuf copies.
  - softmax denominator broadcast moved off the PE (rank-1 matmul) to
    gpsimd.partition_broadcast.
  - startup: weight chunks split across sync+scalar DMA queues ahead of /
    parallel with block-0 x; cos/sin host-pretransposed for contiguous
    DMA; self-pacing warmup matmuls cover the preamble+DMA window so the
    first real matmul runs at full clock.
"""

import os
from contextlib import ExitStack

import numpy as np

S = 4096
HID = 1280
D = 80
NB = 4
BS = 1024
NHL = 2  # heads per core
NCORES = 8
SCALING = float(D) ** -0.5
NEG_THRESH = -1e8

_CACHE = {}


def _build(allowed, mask_add, qkv_dt_name="bfloat16", attn_dt_name="bfloat16",
           out_dt_name="bfloat16"):
    """Build + compile the per-core bass module.

    allowed: tuple over qb of tuple of kb blocks attended to.
    mask_add: frozenset of (qb, kb) needing an additive mask tile.
    """
    import concourse.bass as bass
    import concourse.mybir as mybir
    import concourse.tile as tile
    from concourse import bacc
    from concourse.masks import make_identity

    f32 = mybir.dt.float32
    f32r = mybir.dt.float32r
    dt_qkv = getattr(mybir.dt, qkv_dt_name)
    dt_a = getattr(mybir.dt, attn_dt_name)
    out_dt = getattr(mybir.dt, out_dt_name)
    use_mask = len(mask_add) > 0

    nc = bacc.Bacc(
        "TRN2", target_bir_lowering=False, debug=False, num_devices=NCORES
    )
    xt = nc.dram_tensor("xt", [HID, S], dt_qkv, kind="ExternalInput").ap()
    wt = nc.dram_tensor("wt", [HID, 480], dt_qkv, kind="ExternalInput").ap()
    bqkv = nc.dram_tensor("bqkv", [1, 480], f32, kind="ExternalInput").ap()
    # host pre-rearranged to [p, c, d] tiles so the DMA is contiguous
    cosd = nc.dram_tensor("cosd", [NB * 128, 8 * D], dt_a, kind="ExternalInput").ap()
    sind = nc.dram_tensor("sind", [NB * 128, 8 * D], dt_a, kind="ExternalInput").ap()
    pw = nc.dram_tensor("pw", [2, D, HID], dt_a, kind="ExternalInput").ap()
    if use_mask:
        maskt = nc.dram_tensor("maskt", [S, S], f32, kind="ExternalInput").ap()
    outp = nc.dram_tensor("outp", [HID, S], out_dt, kind="ExternalOutput").ap()

    EXP = mybir.ActivationFunctionType.Exp
    interleave = all(tuple(allowed[b]) == (b,) for b in range(NB))

    with ExitStack() as ctx:
        tc = ctx.enter_context(tile.TileContext(nc))

        # ---- constants ----
        cpool = ctx.enter_context(tc.tile_pool(name="cpool", bufs=1))
        wt_sb = cpool.tile([128, 10, 480], dt_qkv, tag="wt_sb", name="wt_sb")
        wt_r = wt.rearrange("(kk p) c -> p kk c", p=128)
        # first two contraction chunks go ahead of everything else on the
        # sync queue; block 0's x tiles follow immediately so the first
        # qkv k-loop starts as soon as x_0 lands (~6.5us). The remaining
        # weight chunks, bias, and proj weights stream on the gpsimd queue
        # in parallel.
        nc.sync.dma_start(out=wt_sb[:, 0:2, :], in_=wt_r[:, 0:2, :])
        bias_bc = cpool.tile([128, 480], f32, tag="bias_bc", name="bias_bc")
        ident = cpool.tile([128, 128], dt_a, tag="ident", name="ident")
        make_identity(nc, ident)
        pw_sb = cpool.tile([D, 2, HID], dt_a, tag="pw_sb", name="pw_sb")

        def c_wrest():
            nc.scalar.dma_start(out=wt_sb[:, 2:6, :], in_=wt_r[:, 2:6, :])
            nc.scalar.dma_start(out=wt_sb[:, 6:10, :], in_=wt_r[:, 6:10, :])
            nc.gpsimd.dma_start(
                out=bias_bc, in_=bqkv[0:1, :].to_broadcast((128, 480))
            )

        def c_const():
            # deferred: not needed until the first attention/proj phase
            nc.gpsimd.dma_start(out=pw_sb, in_=pw.rearrange("h d o -> d h o"))

        kT_sb = [
            cpool.tile([D, S], dt_a, tag=f"kT{h}_sb", name=f"kT{h}_sb")
            for h in range(NHL)
        ]

        # ---- pools ----
        xtp = ctx.enter_context(tc.tile_pool(name="xtp", bufs=20))
        trig = ctx.enter_context(tc.tile_pool(name="trig", bufs=2))
        qkp = ctx.enter_context(tc.tile_pool(name="qkp", bufs=2))
        t2p = ctx.enter_context(tc.tile_pool(name="t2p", bufs=2))
        vp = ctx.enter_context(tc.tile_pool(name="vp", bufs=1))
        qtp = ctx.enter_context(
            tc.tile_pool(name="qtp", bufs=4 if interleave else 8)
        )
        expp = ctx.enter_context(tc.tile_pool(name="expp", bufs=4))
        # ot tiles of blocks NB-3/NB-2 stay alive until their deferred
        # sh1 proj runs during the final block -> all 4 blocks' ot tiles
        # can be live at once.
        otp = ctx.enter_context(tc.tile_pool(name="otp", bufs=8))
        ddp = ctx.enter_context(tc.tile_pool(name="ddp", bufs=2))
        rbp = ctx.enter_context(tc.tile_pool(name="rbp", bufs=2))
        stg = ctx.enter_context(tc.tile_pool(name="stg", bufs=20))
        if use_mask:
            mtp = ctx.enter_context(tc.tile_pool(name="mtp", bufs=4))

        # PSUM: 8 banks total. qkv accumulation and the rotary transposes
        # share one tag (they alternate within the qkv phase).
        qkvps = ctx.enter_context(tc.tile_pool(name="qkvps", bufs=2, space="PSUM"))
        scps = ctx.enter_context(tc.tile_pool(name="scps", bufs=2, space="PSUM"))
        pvps = ctx.enter_context(tc.tile_pool(name="pvps", bufs=2, space="PSUM"))
        prps = ctx.enter_context(tc.tile_pool(name="prps", bufs=2, space="PSUM"))

        # PE warmup: the HAM clock gate only releases full rate after ~3.4us
        # of sustained matmul activity, and the first real matmul waits on
        # the whole block-0 x DMA watermark (~13us). These throwaway
        # matmuls on a zeroed tile cover the 8..13us window so the real
        # qkv starts at full clock.
        scratch = cpool.tile([128, 512], dt_a, tag="scratch", name="scratch")
        nc.gpsimd.memset(scratch, 0.0)
        for w in range(6):
            wp = scps.tile([128, 128], f32, tag="sc", name=f"warm_{w}")
            nc.tensor.matmul(out=wp, lhsT=scratch[:, 0:128], rhs=scratch[:, 0:128])
        # wide warmups self-pace at ~640ns through the sem chain, covering
        # the stretch until the block-0 x watermark (~17us) with few
        # instructions.
        for w in range(12):
            wp = scps.tile([128, 512], f32, tag="sc", name=f"warmw_{w}")
            nc.tensor.matmul(out=wp, lhsT=scratch[:, 0:128], rhs=scratch)

        v_tiles = {}
        qT_tiles = {}
        xts_map = {}

        def load_x(b):
            # x of block b must be resident BEFORE block b-1's attention
            # begins (its qkv matmuls are the filler there), so this is
            # emitted at the start of block b-1's qkv phase: the 10 DMAs
            # (~7us) overlap that whole phase instead of starving the
            # attention filler. For block 0 the remaining weight chunks are
            # woven between the early x tiles on the same queue so the
            # first k-loop never stalls on a weight arriving behind all
            # of x.
            xts = []
            for k in range(10):
                xt_t = xtp.tile([128, BS], dt_qkv, tag="xt", name=f"xt_{b}_{k}")
                nc.sync.dma_start(
                    out=xt_t,
                    in_=xt[k * 128 : (k + 1) * 128, b * BS : (b + 1) * BS],
                )
                xts.append(xt_t)
            xts_map[b] = xts

        def qkv_chunks(b):
            """Emission chunks for the qkv+rotary+transpose phase of block b.

            Returned as closures so they can be zipped between the previous
            block's attention chunks: alternating bass_priority makes the
            tile scheduler fill the exp-bound attention stretches with qkv
            matmuls.
            """
            st = {}
            # qk tile allocated at qkv_chunks() call time (seam start) so
            # the first c_mm does not depend on the trig DMA emission.
            st["qk"] = qkp.tile([128, 8, 320], dt_a, tag="qk", name=f"qk_{b}")

            def c_load():
                cos_t = trig.tile([128, 8, D], dt_a, tag="cos", name=f"cos_{b}")
                nc.sync.dma_start(
                    out=cos_t,
                    in_=cosd[b * 128 : (b + 1) * 128, :].rearrange(
                        "p (c d) -> p c d", c=8
                    ),
                )
                sin_t = trig.tile([128, 8, D], dt_a, tag="sin", name=f"sin_{b}")
                nc.sync.dma_start(
                    out=sin_t,
                    in_=sind[b * 128 : (b + 1) * 128, :].rearrange(
                        "p (c d) -> p c d", c=8
                    ),
                )
                st["cos"] = cos_t
                st["sin"] = sin_t

            def c_mm(m):
                def run():
                    qk_blk = st["qk"]
                    ps = qkvps.tile(
                        [128, 480], f32, tag="qkvps", name=f"qkvps_{b}_{m}"
                    )
                    for k in range(10):
                        nc.tensor.matmul(
                            out=ps,
                            lhsT=xts_map[b][k][:, m * 128 : (m + 1) * 128],
                            rhs=wt_sb[:, k, :],
                            start=(k == 0),
                            stop=(k == 9),
                        )
                    nc.vector.tensor_add(
                        out=qk_blk[:, m, :], in0=ps[:, 0:320], in1=bias_bc[:, 0:320]
                    )
                    v_t = vp.tile(
                        [128, 2, 97], dt_a, tag="v", name=f"v_{b}_{m}",
                        bufs=16 if interleave else 32,
                    )
                    nc.vector.tensor_add(
                        out=v_t[:, :, 0:D],
                        in0=ps[:, 320:480].rearrange("p (h d) -> p h d", h=2),
                        in1=bias_bc[:, 320:480].rearrange("p (h d) -> p h d", h=2),
                    )
                    nc.gpsimd.memset(v_t[:, :, D:97], 1.0)
                    v_tiles[(b, m)] = v_t

                return run

            def rot_dve(half, tau, h):
                # rotary multiply-adds for one (token-half, q/k, head) unit.
                # Split from the transposes so the DVE work can be queued
                # ahead of PE filler matmuls at the block seam: the PE is
                # strictly in-order, so a transpose waiting on DVE rotary
                # blocks every later matmul in its queue.
                def run():
                    qk_blk = st["qk"]
                    sin_t = st["sin"]
                    cos_t = st["cos"]
                    base = tau * 160 + h * D
                    ms = slice(half * 4, half * 4 + 4)
                    sl = qk_blk[:, ms, base : base + D]
                    t2 = t2p.tile(
                        [128, 4, D], dt_a, tag="t2", name=f"t2_{b}_{half}_{tau}_{h}"
                    )
                    nc.vector.tensor_mul(
                        out=t2[:, :, 0:40],
                        in0=qk_blk[:, ms, base + 40 : base + D],
                        in1=sin_t[:, ms, 0:40],
                    )
                    nc.vector.tensor_mul(
                        out=t2[:, :, 40:D],
                        in0=qk_blk[:, ms, base : base + 40],
                        in1=sin_t[:, ms, 40:D],
                    )
                    nc.vector.tensor_mul(out=sl, in0=sl, in1=cos_t[:, ms, :])
                    nc.vector.tensor_add(out=sl, in0=sl, in1=t2)

                return run

            def rot_pe(half, tau, h):
                def run():
                    qk_blk = st["qk"]
                    base = tau * 160 + h * D
                    g = half
                    if tau == 0 and half == 0:
                        dst_t = qtp.tile([D, BS], dt_a, tag="qt", name=f"qT_{b}_{h}")
                        qT_tiles[(b, h)] = dst_t
                    # half-1 transposes run at the seam where the scores
                    # PSUM bank is idle; half-0 shares the qkv bank.
                    pool = qkvps if half == 0 else scps
                    tag = "qkvps" if half == 0 else "sc"
                    tp = pool.tile(
                        [D, 512], dt_a, tag=tag, name=f"tr_{b}_{half}_{tau}_{h}"
                    )
                    for j in range(4):
                        m = g * 4 + j
                        nc.tensor.matmul(
                            out=tp[:, j * 128 : (j + 1) * 128],
                            lhsT=qk_blk[:, m, base : base + D],
                            rhs=ident,
                            is_transpose=True,
                            start=(j == 0),
                            stop=(j == 3),
                        )
                    if tau == 0:
                        dst = qT_tiles[(b, h)][:, g * 512 : (g + 1) * 512]
                    else:
                        dst = kT_sb[h][
                            :, b * BS + g * 512 : b * BS + (g + 1) * 512
                        ]
                    nc.vector.tensor_copy(out=dst, in_=tp)

                return run

            return {
                "load": c_load,
                "mm": [c_mm(m) for m in range(8)],
                "rdve": {
                    (hf, tau, h): rot_dve(hf, tau, h)
                    for hf in range(2)
                    for tau in range(2)
                    for h in range(NHL)
                },
                "rpe": {
                    (hf, tau, h): rot_pe(hf, tau, h)
                    for hf in range(2)
                    for tau in range(2)
                    for h in range(NHL)
                },
            }

        def attn_chunks(b):
            kbs = allowed[b]
            nmm = len(kbs) * 8
            st = {}

            def c_alloc():
                st["ots"] = [
                    otp.tile([D, BS], dt_a, tag="ot", name=f"ot_{b}_{h}")
                    for h in range(NHL)
                ]

            def c_attn(sh, h, fillq=None):
                def run():
                    qT_t = qT_tiles[(b, h)]
                    ot_t = st["ots"][h]
                    pv = pvps.tile(
                        [97, 512], f32, tag="pv", name=f"pv_{b}_{h}_{sh}"
                    )
                    items = [(kb, t) for kb in kbs for t in range(8)]

                    def score(i):
                        kb, t = items[i]
                        scp = scps.tile(
                            [128, 512], f32, tag="sc",
                            name=f"sc_{b}_{h}_{sh}_{kb}_{t}",
                        )
                        nc.tensor.matmul(
                            out=scp,
                            lhsT=kT_sb[h][
                                :, kb * BS + t * 128 : kb * BS + (t + 1) * 128
                            ],
                            rhs=qT_t[:, sh * 512 : (sh + 1) * 512],
                            start=True,
                            stop=True,
                        )
                        if (b, kb) in mask_add:
                            mk = mtp.tile(
                                [128, 512], f32, tag="mk",
                                name=f"mk_{b}_{h}_{sh}_{kb}_{t}",
                            )
                            nc.sync.dma_start(
                                out=mk,
                                in_=maskt[
                                    kb * BS + t * 128 : kb * BS + (t + 1) * 128,
                                    b * BS + sh * 512 : b * BS + (sh + 1) * 512,
                                ],
                            )
                            nc.vector.tensor_add(out=scp, in0=scp, in1=mk)
                        ep = expp.tile(
                            [128, 512], dt_a, tag="exp",
                            name=f"ep_{b}_{h}_{sh}_{kb}_{t}",
                        )
                        nc.scalar.activation(
                            out=ep, in_=scp, func=EXP, scale=SCALING
                        )
                        return ep

                    def pvmm(i, ep):
                        kb, t = items[i]
                        nc.tensor.matmul(
                            out=pv,
                            lhsT=v_tiles[(kb, t)][:, h, :],
                            rhs=ep,
                            start=(i == 0),
                            stop=(i == nmm - 1),
                        )

                    # scores run two chunks ahead of pv in the (in-order) PE
                    # queue: pv_i waits on exp_i, and score_{i+2} waits on
                    # exp_i via the scps WAR, so the chain advances at the
                    # scalar exp rate with the scalar engine saturated.
                    # fillq items (deferred proj pairs) slot in after each
                    # pv to absorb the remaining exp slack.
                    pend = []
                    for i in range(len(items)):
                        ep = score(i)
                        pend.append((i, ep))
                        if i >= 2:
                            j, epj = pend.pop(0)
                            pvmm(j, epj)
                            if fillq:
                                fillq.pop(0)()
                    while pend:
                        j, epj = pend.pop(0)
                        pvmm(j, epj)
                        if fillq:
                            fillq.pop(0)()
                    # normalize: dd = denom row (scalar copy, psum->sbuf),
                    # broadcast across the 80 partitions on the (idle)
                    # gpsimd engine, invert with the fast custom-DVE
                    # reciprocal, ot = pv * rb.
                    dd = ddp.tile(
                        [1, 512], f32, tag="dd", name=f"dd_{b}_{h}_{sh}"
                    )
                    nc.scalar.copy(out=dd, in_=pv[96:97, :])
                    rbc = rbp.tile(
                        [D, 512], f32, tag="rbc", name=f"rbc_{b}_{h}_{sh}"
                    )
                    nc.gpsimd.partition_broadcast(rbc, dd)
                    rb = rbp.tile([D, 512], f32, tag="rb", name=f"rb_{b}_{h}_{sh}")
                    nc.vector.reciprocal_approx_fast(out=rb, in_=rbc)
                    nc.vector.tensor_mul(
                        out=ot_t[:, sh * 512 : (sh + 1) * 512],
                        in0=pv[0:D, :],
                        in1=rb,
                    )

                return run

            def emit_proj_o(sh, o, late, copy_scalar):
                ots = st["ots"]
                sts_t = stg.tile(
                    [128, 512], out_dt, tag="st", name=f"st_{b}_{sh}_{o}"
                )
                if copy_scalar and o % 2:
                    # scalar path (exp-idle stretches): fp32 accumulate +
                    # ACT copy, keeping the work off the busy DVE.
                    pp = qkvps.tile(
                        [128, 512], f32, tag="qkvps", name=f"pr_{b}_{sh}_{o}"
                    )
                    for hh in range(2):
                        nc.tensor.matmul(
                            out=pp,
                            lhsT=pw_sb[:, hh, o * 128 : (o + 1) * 128],
                            rhs=ots[hh][:, sh * 512 : (sh + 1) * 512],
                            start=(hh == 0),
                            stop=(hh == 1),
                        )
                    nc.scalar.copy(out=sts_t, in_=pp)
                    if b == NB - 1:
                        wp = pvps.tile(
                            [128, 128], f32, tag="pv", name=f"tw_{b}_{sh}_{o}"
                        )
                        nc.tensor.matmul(
                            out=wp,
                            lhsT=scratch[:, 0:128],
                            rhs=scratch[:, 0:128],
                        )
                else:
                    # late projs run when qkv is idle: alternate the idle
                    # qkv PSUM banks in so 4 matmul pairs can run ahead of
                    # the psum->sbuf copies.
                    if (late or b == NB - 1) and o % 2:
                        pp = qkvps.tile(
                            [128, 512], f32, tag="qkvps", name=f"pr_{b}_{sh}_{o}"
                        )
                    else:
                        pp = prps.tile(
                            [128, 512], f32, tag="pr", name=f"pr_{b}_{sh}_{o}"
                        )
                    for hh in range(2):
                        nc.tensor.matmul(
                            out=pp,
                            lhsT=pw_sb[:, hh, o * 128 : (o + 1) * 128],
                            rhs=ots[hh][:, sh * 512 : (sh + 1) * 512],
                            start=(hh == 0),
                            stop=(hh == 1),
                        )
                    nc.vector.tensor_copy(out=sts_t, in_=pp)
                if copy_scalar:
                    eng = (nc.sync, nc.gpsimd, nc.scalar)[o % 3]
                else:
                    eng = nc.gpsimd if o % 2 else nc.sync
                eng.dma_start(
                    out=outp[
                        o * 128 : (o + 1) * 128,
                        b * BS + sh * 512 : b * BS + (sh + 1) * 512,
                    ],
                    in_=sts_t,
                )

            def c_proj(sh, o_lo=0, o_hi=10, late=False, copy_scalar=False):
                def run():
                    for o in range(o_lo, o_hi):
                        emit_proj_o(sh, o, late, copy_scalar)

                return run

            def proj_pair(sh, o, late=False):
                def run():
                    emit_proj_o(sh, o, late, False)

                return run

            if b == NB - 1:
                # final block: deferred proj pairs of blocks NB-3/NB-2 are
                # consumed one pair per attention chunk inside the units
                # (fillq), absorbing the per-chunk exp slack; the block's
                # own sh0 proj pairs join the queue once both sh0 units are
                # done. sh1 proj runs at the very end with scalar copies
                # (exp is finished by then).
                chunks = [c_alloc]
                for h in range(NHL):
                    r = c_attn(0, h, fillq=deferred_pairs)
                    r.is_attn = True
                    chunks.append(r)
                chunks.append(
                    lambda: deferred_pairs.extend(
                        proj_pair(0, o) for o in range(10)
                    )
                )
                for h in range(NHL):
                    r = c_attn(1, h, fillq=deferred_pairs)
                    r.is_attn = True
                    chunks.append(r)
                chunks.append(lambda: [f() for f in deferred_pairs])
                chunks.append(lambda: deferred_pairs.clear())
                chunks.append(c_proj(1, copy_scalar=True))
                return chunks

            chunks = [c_alloc]
            for sh in range(2):
                for h in range(NHL):
                    r = c_attn(sh, h)
                    r.is_attn = True
                    chunks.append(r)
                # sh1 projs of blocks NB-4..NB-2 are deferred into the last
                # block's seam + attention phase: they are the only dense PE
                # filler available there (no next-block qkv remains), keeping
                # the HAM clock warm through the tail. Block NB-4's unit is
                # split in half to zip with the seam transposes; its copies
                # go to the scalar engine, which is exp-idle at the seam.
                if sh == 1 and b == NB - 4:
                    deferred.append(c_proj(sh, 0, 5, late=True, copy_scalar=True))
                    deferred.append(c_proj(sh, 5, 10, late=True, copy_scalar=True))
                elif sh == 1 and b in (NB - 3, NB - 2):
                    deferred_pairs.extend(
                        proj_pair(sh, o, late=True) for o in range(10)
                    )
                else:
                    # the last two o-chunks of each sh0 proj also go to the
                    # final block's fill queue: 30 pairs cover only 30 of
                    # its 32 attention-chunk slots, so the last unit runs
                    # dry without these.
                    chunks.append(c_proj(sh, 0, 8))
                    deferred_pairs.extend(
                        proj_pair(sh, o, late=True) for o in range(8, 10)
                    )
            return chunks

        # rotary unit order: k heads first (scores need the full kT block),
        # q heads after (only the matching sh-half of qT is needed early)
        ROT0 = [(0, 1, 0), (0, 1, 1), (0, 0, 0), (0, 0, 1)]
        ROT1 = [(1, 1, 0), (1, 1, 1), (1, 0, 0), (1, 0, 1)]

        def mid_list(q):
            # qkv mid-phase of a block: mm4..7 with the half-0 rotary units
            # woven in so each transpose lands behind a dense mm stretch.
            return [
                q["mm"][4], q["rdve"][ROT0[0]], q["mm"][5], q["rpe"][ROT0[0]],
                q["rdve"][ROT0[1]], q["mm"][6], q["rpe"][ROT0[1]],
                q["rdve"][ROT0[2]], q["mm"][7], q["rpe"][ROT0[2]],
                q["rdve"][ROT0[3]], q["rpe"][ROT0[3]],
            ]

        if interleave:
            # Software-pipelined emission. Per block b:
            #   seam:      half-1 rotary DVE queued first, its transposes
            #              zipped between block b+1's first qkv matmuls
            #              (in-order PE filler for the DVE wait)
            #   attention: zipped with block b+1's qkv mid-phase
            # The deferred sh1 projs of blocks NB-3/NB-2 play the role of
            # the "next qkv" for the final block.
            deferred = []
            deferred_pairs = []
            load_x(0)
            cur = qkv_chunks(0)
            c_wrest()
            for m in range(2):
                cur["mm"][m]()
            cur["load"]()
            c_const()
            for m in range(2, 4):
                cur["mm"][m]()
            load_x(1)
            for c in mid_list(cur):
                c()
            for b in range(NB):
                at = attn_chunks(b)
                if b + 1 < NB:
                    nxt = qkv_chunks(b + 1)
                    nxt["load"]()
                    if b + 2 < NB:
                        load_x(b + 2)
                    seam_fill = [nxt["mm"][m] for m in range(4)]
                else:
                    nxt = None
                    seam_fill = deferred[:2]
                for u in ROT1:
                    cur["rdve"][u]()
                if nxt is not None:
                    for f, u in zip(seam_fill, ROT1):
                        f()
                        cur["rpe"][u]()
                else:
                    seam_fill[0]()
                    cur["rpe"][ROT1[0]]()
                    cur["rpe"][ROT1[1]]()
                    seam_fill[1]()
                    cur["rpe"][ROT1[2]]()
                    cur["rpe"][ROT1[3]]()
                filler = mid_list(nxt) if nxt is not None else deferred[2:]
                # distribute filler only after the exp-bound attention
                # chunks (proj chunks are already tensor-dense)
                attn_pos = [
                    i for i, a in enumerate(at) if getattr(a, "is_attn", False)
                ]
                k = 0
                j = 0
                for i, a in enumerate(at):
                    a()
                    if i in attn_pos:
                        j += 1
                        take = (len(filler) * j) // max(1, len(attn_pos)) - k
                        for _ in range(take):
                            filler[k]()
                            k += 1
                while k < len(filler):
                    filler[k]()
                    k += 1
                cur = nxt
        else:
            deferred = []
            deferred_pairs = []
            c_wrest()
            c_const()
            load_x(0)
            for b in range(NB):
                if b + 1 < NB:
                    load_x(b + 1)
                q = qkv_chunks(b)
                q["load"]()
                for m in range(8):
                    q["mm"][m]()
                for u in ROT0 + ROT1:
                    q["rdve"][u]()
                    q["rpe"][u]()
            for b in range(NB):
                for c in attn_chunks(b):
                    c()
            for c in deferred:
                c()
            for c in deferred_pairs:
                c()

    nc.compile()
    return nc


def _build_v4(qkv_dt_name="bfloat16", attn_dt_name="bfloat16",
              out_dt_name="bfloat16"):
    """Block-sharded build: core c handles sequence block c//2 (1024
    tokens) and head-half c%2 (8 heads, 4 head-groups of 2). Attention is
    identical per head-group, but the output projection now contracts over
    640 dims (5 full 128-chunks) instead of 2x80 -> 10x2x5x512 cycles for
    a quarter of the output area, saving ~12.8us of PE streaming per core,
    and the kernel ends in a dense exp-free proj GEMM.
    Only valid for the exact block-diagonal mask (no mask adds).
    """
    import concourse.mybir as mybir
    import concourse.tile as tile
    from concourse import bacc
    from concourse.masks import make_identity

    f32 = mybir.dt.float32
    dt_qkv = getattr(mybir.dt, qkv_dt_name)
    dt_a = getattr(mybir.dt, attn_dt_name)
    out_dt = getattr(mybir.dt, out_dt_name)
    NHG = 4  # head groups of 2 per core

    nc = bacc.Bacc(
        "TRN2", target_bir_lowering=False, debug=False, num_devices=NCORES
    )
    xt = nc.dram_tensor("xt", [HID, BS], dt_qkv, kind="ExternalInput").ap()
    # host pre-arranged to the SBUF tile layout so every weight DMA is
    # contiguous per partition (strided weight loads run at ~50GB/s and
    # starve the x stream)
    wt = nc.dram_tensor(
        "wt", [NHG, 128, 10 * 480], dt_qkv, kind="ExternalInput"
    ).ap()
    bqkv = nc.dram_tensor("bqkv", [1, NHG * 480], f32, kind="ExternalInput").ap()
    cosd = nc.dram_tensor("cosd", [128, 8 * D], dt_a, kind="ExternalInput").ap()
    sind = nc.dram_tensor("sind", [128, 8 * D], dt_a, kind="ExternalInput").ap()
    pw = nc.dram_tensor("pw", [5, 128, HID], dt_a, kind="ExternalInput").ap()
    outp = nc.dram_tensor("outp", [HID, BS], out_dt, kind="ExternalOutput").ap()

    EXP = mybir.ActivationFunctionType.Exp

    with ExitStack() as ctx:
        tc = ctx.enter_context(tile.TileContext(nc))

        cpool = ctx.enter_context(tc.tile_pool(name="cpool", bufs=1))
        wt_sb = cpool.tile([128, NHG, 10, 480], dt_qkv, tag="wt_sb", name="wt_sb")
        # hg0 weights ahead of x on sync/scalar; hg1-3 behind on gpsimd
        wt_r0 = wt[0].rearrange("p (kk c) -> p kk c", kk=10)
        nc.sync.dma_start(out=wt_sb[:, 0, 0:2, :], in_=wt_r0[:, 0:2, :])
        bias_bc = cpool.tile([128, NHG * 480], f32, tag="bias_bc", name="bias_bc")
        ident = cpool.tile([128, 128], dt_a, tag="ident", name="ident")
        make_identity(nc, ident)
        pw_sb = cpool.tile([128, 5, HID], dt_a, tag="pw_sb", name="pw_sb")
        ot_stack = [
            cpool.tile([128, BS], dt_a, tag=f"ot{c}", name=f"ot{c}")
            for c in range(5)
        ]

        def c_wrest():
            nc.scalar.dma_start(out=wt_sb[:, 0, 2:10, :], in_=wt_r0[:, 2:10, :])
            nc.gpsimd.dma_start(
                out=bias_bc[:, 0:480],
                in_=bqkv[0:1, 0:480].to_broadcast((128, 480)),
            )

        def c_wnext(hg):
            # one head-group of weights (1.2MB), loaded one phase before
            # its qkv so the DMA never competes with block-0 x at startup
            def run():
                nc.gpsimd.dma_start(
                    out=wt_sb[:, hg, :, :],
                    in_=wt[hg].rearrange("p (kk c) -> p kk c", kk=10),
                )
                nc.gpsimd.dma_start(
                    out=bias_bc[:, hg * 480 : (hg + 1) * 480],
                    in_=bqkv[0:1, hg * 480 : (hg + 1) * 480].to_broadcast(
                        (128, 480)
                    ),
                )

            return run

        def c_const():
            nc.gpsimd.dma_start(out=pw_sb, in_=pw.rearrange("c p o -> p c o"))

        # ---- pools ----
        xtp = ctx.enter_context(tc.tile_pool(name="xtp", bufs=10))
        trig = ctx.enter_context(tc.tile_pool(name="trig", bufs=1))
        qkp = ctx.enter_context(tc.tile_pool(name="qkp", bufs=2))
        t2p = ctx.enter_context(tc.tile_pool(name="t2p", bufs=2))
        vp = ctx.enter_context(tc.tile_pool(name="vp", bufs=1))
        qtp = ctx.enter_context(tc.tile_pool(name="qtp", bufs=4))
        ktp = ctx.enter_context(tc.tile_pool(name="ktp", bufs=4))
        expp = ctx.enter_context(tc.tile_pool(name="expp", bufs=4))
        ddp = ctx.enter_context(tc.tile_pool(name="ddp", bufs=2))
        rbp = ctx.enter_context(tc.tile_pool(name="rbp", bufs=2))
        stg = ctx.enter_context(tc.tile_pool(name="stg", bufs=20))
        otmp = ctx.enter_context(tc.tile_pool(name="otmp", bufs=3))

        qkvps = ctx.enter_context(tc.tile_pool(name="qkvps", bufs=2, space="PSUM"))
        scps = ctx.enter_context(tc.tile_pool(name="scps", bufs=2, space="PSUM"))
        pvps = ctx.enter_context(tc.tile_pool(name="pvps", bufs=2, space="PSUM"))
        prps = ctx.enter_context(tc.tile_pool(name="prps", bufs=2, space="PSUM"))

        # PE warmup (see _build)
        scratch = cpool.tile([128, 512], dt_a, tag="scratch", name="scratch")
        nc.gpsimd.memset(scratch, 0.0)
        for w in range(6):
            wp = scps.tile([128, 128], f32, tag="sc", name=f"warm_{w}")
            nc.tensor.matmul(out=wp, lhsT=scratch[:, 0:128], rhs=scratch[:, 0:128])
        for w in range(12):
            wp = scps.tile([128, 512], f32, tag="sc", name=f"warmw_{w}")
            nc.tensor.matmul(out=wp, lhsT=scratch[:, 0:128], rhs=scratch)

        v_tiles = {}
        qT_tiles = {}
        kT_tiles = {}
        xts = []
        st_trig = {}

        def load_x():
            # startup is single-queue at full HBM share: hg0's remaining
            # weight chunks are woven into the x stream (splitting them to
            # other queues costs x two-thirds of the bandwidth).
            for k in range(10):
                xt_t = xtp.tile([128, BS], dt_qkv, tag="xt", name=f"xt_{k}")
                nc.sync.dma_start(out=xt_t, in_=xt[k * 128 : (k + 1) * 128, :])
                xts.append(xt_t)

        def c_trig():
            cos_t = trig.tile([128, 8, D], dt_a, tag="cos", name="cos")
            nc.sync.dma_start(
                out=cos_t, in_=cosd.rearrange("p (c d) -> p c d", c=8)
            )
            sin_t = trig.tile([128, 8, D], dt_a, tag="sin", name="sin")
            nc.sync.dma_start(
                out=sin_t, in_=sind.rearrange("p (c d) -> p c d", c=8)
            )
            st_trig["cos"] = cos_t
            st_trig["sin"] = sin_t

        def qkv_chunks(hg):
            st = {}
            st["qk"] = qkp.tile([128, 8, 320], dt_a, tag="qk", name=f"qk_{hg}")

            def c_mm(m):
                def run():
                    qk_blk = st["qk"]
                    ps = qkvps.tile(
                        [128, 480], f32, tag="qkvps", name=f"qkvps_{hg}_{m}"
                    )
                    for k in range(10):
                        nc.tensor.matmul(
                            out=ps,
                            lhsT=xts[k][:, m * 128 : (m + 1) * 128],
                            rhs=wt_sb[:, hg, k, :],
                            start=(k == 0),
                            stop=(k == 9),
                        )
                    bb = bias_bc[:, hg * 480 : (hg + 1) * 480]
                    nc.vector.tensor_add(
                        out=qk_blk[:, m, :], in0=ps[:, 0:320], in1=bb[:, 0:320]
                    )
                    v_t = vp.tile(
                        [128, 2, 97], dt_a, tag="v", name=f"v_{hg}_{m}", bufs=16
                    )
                    nc.vector.tensor_add(
                        out=v_t[:, :, 0:D],
                        in0=ps[:, 320:480].rearrange("p (h d) -> p h d", h=2),
                        in1=bb[:, 320:480].rearrange("p (h d) -> p h d", h=2),
                    )
                    nc.gpsimd.memset(v_t[:, :, D:97], 1.0)
                    v_tiles[(hg, m)] = v_t

                return run

            def rot_dve(half, tau, h):
                def run():
                    qk_blk = st["qk"]
                    sin_t = st_trig["sin"]
                    cos_t = st_trig["cos"]
                    base = tau * 160 + h * D
                    ms = slice(half * 4, half * 4 + 4)
                    sl = qk_blk[:, ms, base : base + D]
                    t2 = t2p.tile(
                        [128, 4, D], dt_a, tag="t2",
                        name=f"t2_{hg}_{half}_{tau}_{h}",
                    )
                    nc.vector.tensor_mul(
                        out=t2[:, :, 0:40],
                        in0=qk_blk[:, ms, base + 40 : base + D],
                        in1=sin_t[:, ms, 0:40],
                    )
                    nc.vector.tensor_mul(
                        out=t2[:, :, 40:D],
                        in0=qk_blk[:, ms, base : base + 40],
                        in1=sin_t[:, ms, 40:D],
                    )
                    nc.vector.tensor_mul(out=sl, in0=sl, in1=cos_t[:, ms, :])
                    nc.vector.tensor_add(out=sl, in0=sl, in1=t2)

                return run

            def rot_pe(half, tau, h):
                def run():
                    qk_blk = st["qk"]
                    base = tau * 160 + h * D
                    g = half
                    if half == 0:
                        if tau == 0:
                            qT_tiles[(hg, h)] = qtp.tile(
                                [D, BS], dt_a, tag="qt", name=f"qT_{hg}_{h}"
                            )
                        else:
                            kT_tiles[(hg, h)] = ktp.tile(
                                [D, BS], dt_a, tag="kt", name=f"kT_{hg}_{h}"
                            )
                    pool = qkvps if half == 0 else scps
                    tag = "qkvps" if half == 0 else "sc"
                    tp = pool.tile(
                        [D, 512], dt_a, tag=tag, name=f"tr_{hg}_{half}_{tau}_{h}"
                    )
                    for j in range(4):
                        m = g * 4 + j
                        nc.tensor.matmul(
                            out=tp[:, j * 128 : (j + 1) * 128],
                            lhsT=qk_blk[:, m, base : base + D],
                            rhs=ident,
                            is_transpose=True,
                            start=(j == 0),
                            stop=(j == 3),
                        )
                    tiles = qT_tiles if tau == 0 else kT_tiles
                    dst = tiles[(hg, h)][:, g * 512 : (g + 1) * 512]
                    nc.vector.tensor_copy(out=dst, in_=tp)

                return run

            return {
                "mm": [c_mm(m) for m in range(8)],
                "rdve": {
                    (hf, tau, h): rot_dve(hf, tau, h)
                    for hf in range(2)
                    for tau in range(2)
                    for h in range(NHL)
                },
                "rpe": {
                    (hf, tau, h): rot_pe(hf, tau, h)
                    for hf in range(2)
                    for tau in range(2)
                    for h in range(NHL)
                },
            }

        def emit_proj_o(sh, o, copy_scalar):
            sts_t = stg.tile([128, 512], out_dt, tag="st", name=f"st_{sh}_{o}")
            use_scalar = copy_scalar and o % 2
            pool, tag = (qkvps, "qkvps") if o % 2 else (prps, "pr")
            pp = pool.tile([128, 512], f32, tag=tag, name=f"pr_{sh}_{o}")
            for c in range(5):
                nc.tensor.matmul(
                    out=pp,
                    lhsT=pw_sb[:, c, o * 128 : (o + 1) * 128],
                    rhs=ot_stack[c][:, sh * 512 : (sh + 1) * 512],
                    start=(c == 0),
                    stop=(c == 4),
                )
            if use_scalar:
                nc.scalar.copy(out=sts_t, in_=pp)
            else:
                nc.vector.tensor_copy(out=sts_t, in_=pp)
            eng = (nc.sync, nc.gpsimd, nc.scalar)[o % 3] if copy_scalar else (
                nc.gpsimd if o % 2 else nc.sync
            )
            eng.dma_start(
                out=outp[
                    o * 128 : (o + 1) * 128, sh * 512 : (sh + 1) * 512
                ],
                in_=sts_t,
            )

        def proj_pair(sh, o, copy_scalar=False):
            def run():
                emit_proj_o(sh, o, copy_scalar)

            return run

        fillq = []

        def attn_chunks(hg):
            def c_attn(sh, h, use_fill=False):
                def run():
                    qT_t = qT_tiles[(hg, h)]
                    kT_t = kT_tiles[(hg, h)]
                    pv = pvps.tile(
                        [97, 512], f32, tag="pv", name=f"pv_{hg}_{h}_{sh}"
                    )

                    def score(i):
                        scp = scps.tile(
                            [128, 512], f32, tag="sc", name=f"sc_{hg}_{h}_{sh}_{i}"
                        )
                        nc.tensor.matmul(
                            out=scp,
                            lhsT=kT_t[:, i * 128 : (i + 1) * 128],
                            rhs=qT_t[:, sh * 512 : (sh + 1) * 512],
                            start=True,
                            stop=True,
                        )
                        ep = expp.tile(
                            [128, 512], dt_a, tag="exp",
                            name=f"ep_{hg}_{h}_{sh}_{i}",
                        )
                        nc.scalar.activation(
                            out=ep, in_=scp, func=EXP, scale=SCALING
                        )
                        return ep

                    def pvmm(i, ep):
                        nc.tensor.matmul(
                            out=pv,
                            lhsT=v_tiles[(hg, i)][:, h, :],
                            rhs=ep,
                            start=(i == 0),
                            stop=(i == 7),
                        )

                    pend = []
                    for i in range(8):
                        ep = score(i)
                        pend.append((i, ep))
                        if i >= 2:
                            j, epj = pend.pop(0)
                            pvmm(j, epj)
                            if use_fill and fillq:
                                fillq.pop(0)()
                    while pend:
                        j, epj = pend.pop(0)
                        pvmm(j, epj)
                        if use_fill and fillq:
                            fillq.pop(0)()
                    dd = ddp.tile([1, 512], f32, tag="dd", name=f"dd_{hg}_{h}_{sh}")
                    nc.scalar.copy(out=dd, in_=pv[96:97, :])
                    rbc = rbp.tile(
                        [D, 512], f32, tag="rbc", name=f"rbc_{hg}_{h}_{sh}"
                    )
                    nc.gpsimd.partition_broadcast(rbc, dd)
                    rb = rbp.tile([D, 512], f32, tag="rb", name=f"rb_{hg}_{h}_{sh}")
                    nc.vector.reciprocal_approx_fast(out=rb, in_=rbc)
                    # place into the stacked [128, BS] ot tiles at the head
                    # global row offset. DVE lanes are hard-wired to
                    # partitions, so a partition-shifted placement needs an
                    # SBUF->SBUF DMA hop (split across two tiles if the
                    # head straddles a 128 boundary).
                    g = hg * 2 + h
                    r0 = (g * D) % 128
                    t0 = (g * D) // 128
                    cs = slice(sh * 512, (sh + 1) * 512)
                    if r0 == 0:
                        nc.vector.tensor_mul(
                            out=ot_stack[t0][0:D, cs], in0=pv[0:D, :], in1=rb
                        )
                    else:
                        ott = otmp.tile(
                            [D, 512], dt_a, tag="otm", name=f"otm_{hg}_{h}_{sh}"
                        )
                        nc.vector.tensor_mul(out=ott, in0=pv[0:D, :], in1=rb)
                        n1 = min(128 - r0, D)
                        eng = nc.gpsimd if (g + sh) % 2 else nc.sync
                        eng.dma_start(
                            out=ot_stack[t0][r0 : r0 + n1, cs], in_=ott[0:n1, :]
                        )
                        if n1 < D:
                            eng.dma_start(
                                out=ot_stack[t0 + 1][0 : D - n1, cs],
                                in_=ott[n1:D, :],
                            )

                return run

            if hg == NHG - 1:
                chunks = []
                for h in range(NHL):
                    r = c_attn(0, h)
                    r.is_attn = True
                    chunks.append(r)
                chunks.append(
                    lambda: fillq.extend(proj_pair(0, o) for o in range(10))
                )
                for h in range(NHL):
                    r = c_attn(1, h, use_fill=True)
                    r.is_attn = True
                    chunks.append(r)
                chunks.append(lambda: [f() for f in fillq])
                chunks.append(lambda: fillq.clear())
                chunks.append(
                    lambda: [
                        emit_proj_o(1, o, copy_scalar=True) for o in range(10)
                    ]
                )
                return chunks
            chunks = []
            for sh in range(2):
                for h in range(NHL):
                    r = c_attn(sh, h)
                    r.is_attn = True
                    chunks.append(r)
            return chunks

        ROT0 = [(0, 1, 0), (0, 1, 1), (0, 0, 0), (0, 0, 1)]
        ROT1 = [(1, 1, 0), (1, 1, 1), (1, 0, 0), (1, 0, 1)]

        def mid_list(q):
            return [
                q["mm"][4], q["rdve"][ROT0[0]], q["mm"][5], q["rpe"][ROT0[0]],
                q["rdve"][ROT0[1]], q["mm"][6], q["rpe"][ROT0[1]],
                q["rdve"][ROT0[2]], q["mm"][7], q["rpe"][ROT0[2]],
                q["rdve"][ROT0[3]], q["rpe"][ROT0[3]],
            ]

        def seam_dummy():
            wp = prps.tile([128, 512], f32, tag="pr", name=f"sd_{len(sdn)}")
            sdn.append(0)
            nc.tensor.matmul(out=wp, lhsT=scratch[:, 0:128], rhs=scratch)

        sdn = []
        load_x()
        cur = qkv_chunks(0)
        c_wrest()
        for m in range(2):
            cur["mm"][m]()
        c_trig()
        for m in range(2, 4):
            cur["mm"][m]()
        mids = mid_list(cur)
        mids[0]()
        c_wnext(1)()
        for c in mids[1:]:
            c()
        for hg in range(NHG):
            at = attn_chunks(hg)
            if hg + 1 < NHG:
                nxt = qkv_chunks(hg + 1)
                seam_fill = [nxt["mm"][m] for m in range(4)]
            else:
                nxt = None
                seam_fill = [seam_dummy, seam_dummy, seam_dummy, seam_dummy]
            for u in ROT1:
                cur["rdve"][u]()
            for f, u in zip(seam_fill, ROT1):
                f()
                cur["rpe"][u]()
            filler = mid_list(nxt) if nxt is not None else []
            if hg == 0:
                filler = [c_wnext(2)] + filler
            elif hg == 1:
                filler = [c_wnext(3), c_const] + filler
            attn_pos = [
                i for i, a in enumerate(at) if getattr(a, "is_attn", False)
            ]
            k = 0
            j = 0
            for i, a in enumerate(at):
                a()
                if i in attn_pos:
                    j += 1
                    take = (len(filler) * j) // max(1, len(attn_pos)) - k
                    for _ in range(take):
                        filler[k]()
                        k += 1
            while k < len(filler):
                filler[k]()
                k += 1
            cur = nxt

    nc.compile()
    return nc


def _analyze_mask(mask):
    m = np.asarray(mask).reshape(S, S)
    allowed = []
    mask_add = set()
    for qb in range(NB):
        row = []
        for kb in range(NB):
            t = m[qb * BS : (qb + 1) * BS, kb * BS : (kb + 1) * BS]
            if np.all(t <= NEG_THRESH):
                continue
            row.append(kb)
            if not np.all(t == 0.0):
                mask_add.add((qb, kb))
        if not row:
            raise NotImplementedError("fully masked query block")
        allowed.append(tuple(row))
    return tuple(allowed), frozenset(mask_add)


def _np_dt(name):
    if name == "bfloat16":
        import ml_dtypes

        return ml_dtypes.bfloat16
    return np.float32


def kernel(
    hidden_states, attention_mask, cos, sin, qkv_w, qkv_b, proj_w, proj_b
):
    from concourse import bass_utils

    qkv_dt = os.environ.get("KERNEL_QKV_DT", "bfloat16")
    attn_dt = os.environ.get("KERNEL_ATTN_DT", "bfloat16")
    out_dt = os.environ.get("KERNEL_OUT_DT", "bfloat16")
    trace = bool(int(os.environ.get("KERNEL_TRACE", "0")))

    global LAST_RESULT

    X = np.ascontiguousarray(np.asarray(hidden_states, dtype=np.float32))
    allowed, mask_add = _analyze_mask(attention_mask)

    key = (qkv_dt, attn_dt, out_dt, allowed, mask_add)
    if key not in _CACHE:
        _CACHE[key] = _build(
            allowed, mask_add, qkv_dt_name=qkv_dt, attn_dt_name=attn_dt,
            out_dt_name=out_dt,
        )
    nc = _CACHE[key]

    np_qkv = _np_dt(qkv_dt)
    np_attn = _np_dt(attn_dt)

    XT = np.ascontiguousarray(X.T).astype(np_qkv)
    cos = np.ascontiguousarray(np.asarray(cos, dtype=np.float32))
    sin = np.asarray(sin, dtype=np.float32)
    sinh = np.ascontiguousarray(
        np.concatenate([-sin[:, : D // 2], sin[:, D // 2 :]], axis=1)
    )
    qkv_w = np.asarray(qkv_w, dtype=np.float32)

    def trig_prep(a):
        # [S, D] -> [NB*128, 8*D]: tile (b, p, c, d) = a[b*1024 + c*128 + p, d]
        return np.ascontiguousarray(
            a.reshape(NB, 8, 128, D).transpose(0, 2, 1, 3).reshape(NB * 128, 8 * D)
        )
    qkv_b = np.asarray(qkv_b, dtype=np.float32)
    proj_w = np.asarray(proj_w, dtype=np.float32)
    proj_b = np.asarray(proj_b, dtype=np.float32)

    diag = (
        all(tuple(allowed[b]) == (b,) for b in range(NB)) and not mask_add
    )
    if diag and os.environ.get("KERNEL_V4", "0") == "1":
        key = ("v4", qkv_dt, attn_dt, out_dt)
        if key not in _CACHE:
            _CACHE[key] = _build_v4(
                qkv_dt_name=qkv_dt, attn_dt_name=attn_dt, out_dt_name=out_dt
            )
        nc4 = _CACHE[key]
        cos_p = trig_prep(cos.astype(np_attn))
        sin_p = trig_prep(sinh.astype(np_attn))
        in_maps = []
        for c in range(NCORES):
            b = c // 2
            hh = c % 2
            wgs, bgs = [], []
            for hg in range(4):
                j0 = (hh * 8 + hg * 2) * D
                sl = slice(j0, j0 + 2 * D)
                Wg = np.concatenate(
                    [qkv_w[sl], qkv_w[HID:][sl], qkv_w[2 * HID :][sl]], axis=0
                )
                # [1280, 480] -> SBUF layout [128, 10*480]
                wgs.append(
                    Wg.T.reshape(10, 128, 480).transpose(1, 0, 2).reshape(
                        128, 4800
                    )
                )
                bgs.append(
                    np.concatenate(
                        [qkv_b[sl], qkv_b[HID:][sl], qkv_b[2 * HID :][sl]]
                    )
                )
            pwT = proj_w[:, hh * 640 : (hh + 1) * 640].T
            in_maps.append({
                "xt": np.ascontiguousarray(XT[:, b * BS : (b + 1) * BS]),
                "wt": np.ascontiguousarray(np.stack(wgs)).astype(np_qkv),
                "bqkv": np.ascontiguousarray(
                    np.concatenate(bgs)[None, :].astype(np.float32)
                ),
                "cosd": np.ascontiguousarray(cos_p[b * 128 : (b + 1) * 128]),
                "sind": np.ascontiguousarray(sin_p[b * 128 : (b + 1) * 128]),
                "pw": np.ascontiguousarray(pwT.reshape(5, 128, HID)).astype(
                    np_attn
                ),
            })
        res = bass_utils.run_bass_kernel_spmd(
            nc4, in_maps, core_ids=list(range(NCORES)), trace=trace
        )
        LAST_RESULT = res
        acc = np.zeros((HID, S), dtype=np.float64)
        for c in range(NCORES):
            b = c // 2
            acc[:, b * BS : (b + 1) * BS] += np.asarray(
                res.results[c]["outp"], dtype=np.float64
            )
        out = acc.T + proj_b.astype(np.float64)[None, :]
        return out.astype(np.float32)

    in_maps = []
    for c in range(NCORES):
        j0 = c * NHL * D
        sl = slice(j0, j0 + NHL * D)
        Wc = np.concatenate(
            [qkv_w[sl], qkv_w[HID:][sl], qkv_w[2 * HID :][sl]], axis=0
        )
        m = {
            "xt": XT,
            "wt": np.ascontiguousarray(Wc.T).astype(np_qkv),
            "bqkv": np.ascontiguousarray(
                np.concatenate([qkv_b[sl], qkv_b[HID:][sl], qkv_b[2 * HID :][sl]])[
                    None, :
                ]
            ),
            "cosd": trig_prep(cos.astype(np_attn)),
            "sind": trig_prep(sinh.astype(np_attn)),
            "pw": np.ascontiguousarray(
                np.stack(
                    [
                        proj_w[:, j0 : j0 + D].T,
                        proj_w[:, j0 + D : j0 + 2 * D].T,
                    ]
                )
            ).astype(np_attn),
        }
        if mask_add:
            m["maskt"] = np.ascontiguousarray(
                (np.asarray(attention_mask).reshape(S, S).T / SCALING).astype(
                    np.float32
                )
            )
        in_maps.append(m)

    res = bass_utils.run_bass_kernel_spmd(
        nc, in_maps, core_ids=list(range(NCORES)), trace=trace
    )
    LAST_RESULT = res

    acc = np.zeros((HID, S), dtype=np.float64)
    for c in range(NCORES):
        acc += np.asarray(res.results[c]["outp"], dtype=np.float64)
    out = acc.T + proj_b.astype(np.float64)[None, :]
    return out.astype(np.float32)


LAST_RESULT = None



# revision 62
# speedup vs baseline: 1.0235x; 1.0019x over previous
"""Qwen3.5 vision attention (S=4096, H=16x80, block-diag mask) on 8 trn2 cores.

Sharding: tensor-parallel over heads (2 heads/core). Each core computes
qkv projection for its heads, rotary, block-sparse attention, and a partial
output projection (RowParallel); the host sums the 8 partials (all-reduce)
and adds proj_b.

v2 (390 -> 238 us): bf16 everywhere off the PSUM paths, on-chip softmax
normalization (no DRAM round-trip), on-chip V ones-column, staged output
stores.

v3 (238 -> ~210 us) — PE (tensor engine) is the bottleneck at ~85% busy;
every change targets PE stalls and the HAM clock gate (PE runs at 1.2GHz
until ~3.4us of sustained activity, and re-throttles after idle windows):
  - rotary is split into per-(token-half, q/k, head) units; the half-0
    units interleave with qkv matmuls mid-phase, and at the block seam
    the half-1 DVE work is queued ahead of the next block's first qkv
    matmuls so the (in-order) PE always has filler in front of the
    rotary-dependent transposes. Seam transposes use the scores PSUM
    bank, which is idle exactly then.
  - x for block b+1 prefetches during block b's qkv phase (the 2.6MB
    DMA no longer starves the attention-phase filler).
  - attention chains run scores two chunks ahead of PV in the PE queue:
    PV_i waits exp_i while score_{i+2} is bounded by the scps WAR on
    exp_i, so the chain advances at the scalar exp rate with the scalar
    engine saturated instead of ping-ponging at ~2x that period.
  - sh1 projs of blocks 0-2 (plus the last two sh0 o-chunks of each) are
    deferred into the final block's seam and attention phase as per-chunk
    pair filler (the only dense PE work available there), and the final
    block's own sh0 proj pairs join the queue mid-phase. Late projs alternate between the proj and (idle)
    qkv PSUM pools so 4 matmul pairs run ahead of the psum->sbuf copies.
  - softmax denominator broadcast moved off the PE (rank-1 matmul) to
    gpsimd.partition_broadcast.
  - startup: weight chunks split across sync+scalar DMA queues ahead of /
    parallel with block-0 x; cos/sin host-pretransposed for contiguous
    DMA; self-pacing warmup matmuls cover the preamble+DMA window so the
    first real matmul runs at full clock.
"""

import os
from contextlib import ExitStack

import numpy as np

S = 4096
HID = 1280
D = 80
NB = 4
BS = 1024
NHL = 2  # heads per core
NCORES = 8
SCALING = float(D) ** -0.5
NEG_THRESH = -1e8

_CACHE = {}


def _build(allowed, mask_add, qkv_dt_name="bfloat16", attn_dt_name="bfloat16",
           out_dt_name="bfloat16"):
    """Build + compile the per-core bass module.

    allowed: tuple over qb of tuple of kb blocks attended to.
    mask_add: frozenset of (qb, kb) needing an additive mask tile.
    """
    import concourse.bass as bass
    import concourse.mybir as mybir
    import concourse.tile as tile
    from concourse import bacc
    from concourse.masks import make_identity

    f32 = mybir.dt.float32
    f32r = mybir.dt.float32r
    dt_qkv = getattr(mybir.dt, qkv_dt_name)
    dt_a = getattr(mybir.dt, attn_dt_name)
    out_dt = getattr(mybir.dt, out_dt_name)
    use_mask = len(mask_add) > 0

    nc = bacc.Bacc(
        "TRN2", target_bir_lowering=False, debug=False, num_devices=NCORES
    )
    xt = nc.dram_tensor("xt", [HID, S], dt_qkv, kind="ExternalInput").ap()
    # host pre-arranged to the SBUF tile layout so weight DMAs are
    # contiguous per partition (strided loads run at ~50GB/s and delay
    # the block-0 x stream behind them on the same queue)
    wt = nc.dram_tensor("wt", [128, 10 * 480], dt_qkv, kind="ExternalInput").ap()
    bqkv = nc.dram_tensor("bqkv", [1, 480], f32, kind="ExternalInput").ap()
    # host pre-rearranged to [p, c, d] tiles so the DMA is contiguous
    cosd = nc.dram_tensor("cosd", [NB * 128, 8 * D], dt_a, kind="ExternalInput").ap()
    sind = nc.dram_tensor("sind", [NB * 128, 8 * D], dt_a, kind="ExternalInput").ap()
    pw = nc.dram_tensor("pw", [2, D, HID], dt_a, kind="ExternalInput").ap()
    if use_mask:
        maskt = nc.dram_tensor("maskt", [S, S], f32, kind="ExternalInput").ap()
    outp = nc.dram_tensor("outp", [HID, S], out_dt, kind="ExternalOutput").ap()

    EXP = mybir.ActivationFunctionType.Exp
    interleave = all(tuple(allowed[b]) == (b,) for b in range(NB))

    with ExitStack() as ctx:
        tc = ctx.enter_context(tile.TileContext(nc))

        # ---- constants ----
        cpool = ctx.enter_context(tc.tile_pool(name="cpool", bufs=1))
        wt_sb = cpool.tile([128, 10, 480], dt_qkv, tag="wt_sb", name="wt_sb")
        wt_r = wt.rearrange("p (kk c) -> p kk c", kk=10)
        # first two contraction chunks go ahead of everything else on the
        # sync queue; block 0's x tiles follow immediately so the first
        # qkv k-loop starts as soon as x_0 lands (~6.5us). The remaining
        # weight chunks, bias, and proj weights stream on the gpsimd queue
        # in parallel.
        nc.sync.dma_start(out=wt_sb[:, 0:2, :], in_=wt_r[:, 0:2, :])
        bias_bc = cpool.tile([128, 480], f32, tag="bias_bc", name="bias_bc")
        ident = cpool.tile([128, 128], dt_a, tag="ident", name="ident")
        make_identity(nc, ident)
        pw_sb = cpool.tile([D, 2, HID], dt_a, tag="pw_sb", name="pw_sb")

        def c_wrest():
            nc.scalar.dma_start(out=wt_sb[:, 2:6, :], in_=wt_r[:, 2:6, :])
            nc.scalar.dma_start(out=wt_sb[:, 6:10, :], in_=wt_r[:, 6:10, :])
            nc.gpsimd.dma_start(
                out=bias_bc, in_=bqkv[0:1, :].to_broadcast((128, 480))
            )

        def c_const():
            # deferred: not needed until the first attention/proj phase
            nc.gpsimd.dma_start(out=pw_sb, in_=pw.rearrange("h d o -> d h o"))

        kT_sb = [
            cpool.tile([D, S], dt_a, tag=f"kT{h}_sb", name=f"kT{h}_sb")
            for h in range(NHL)
        ]

        # ---- pools ----
        xtp = ctx.enter_context(tc.tile_pool(name="xtp", bufs=20))
        trig = ctx.enter_context(tc.tile_pool(name="trig", bufs=2))
        qkp = ctx.enter_context(tc.tile_pool(name="qkp", bufs=2))
        t2p = ctx.enter_context(tc.tile_pool(name="t2p", bufs=2))
        vp = ctx.enter_context(tc.tile_pool(name="vp", bufs=1))
        qtp = ctx.enter_context(
            tc.tile_pool(name="qtp", bufs=4 if interleave else 8)
        )
        expp = ctx.enter_context(tc.tile_pool(name="expp", bufs=4))
        # ot tiles of blocks NB-3/NB-2 stay alive until their deferred
        # sh1 proj runs during the final block -> all 4 blocks' ot tiles
        # can be live at once.
        otp = ctx.enter_context(tc.tile_pool(name="otp", bufs=8))
        ddp = ctx.enter_context(tc.tile_pool(name="ddp", bufs=2))
        rbp = ctx.enter_context(tc.tile_pool(name="rbp", bufs=2))
        stg = ctx.enter_context(tc.tile_pool(name="stg", bufs=20))
        if use_mask:
            mtp = ctx.enter_context(tc.tile_pool(name="mtp", bufs=4))

        # PSUM: 8 banks total. qkv accumulation and the rotary transposes
        # share one tag (they alternate within the qkv phase).
        qkvps = ctx.enter_context(tc.tile_pool(name="qkvps", bufs=2, space="PSUM"))
        scps = ctx.enter_context(tc.tile_pool(name="scps", bufs=2, space="PSUM"))
        pvps = ctx.enter_context(tc.tile_pool(name="pvps", bufs=2, space="PSUM"))
        prps = ctx.enter_context(tc.tile_pool(name="prps", bufs=2, space="PSUM"))

        # PE warmup: the HAM clock gate only releases full rate after ~3.4us
        # of sustained matmul activity, and the first real matmul waits on
        # the whole block-0 x DMA watermark (~13us). These throwaway
        # matmuls on a zeroed tile cover the 8..13us window so the real
        # qkv starts at full clock.
        scratch = cpool.tile([128, 512], dt_a, tag="scratch", name="scratch")
        nc.gpsimd.memset(scratch, 0.0)
        for w in range(6):
            wp = scps.tile([128, 128], f32, tag="sc", name=f"warm_{w}")
            nc.tensor.matmul(out=wp, lhsT=scratch[:, 0:128], rhs=scratch[:, 0:128])
        # wide warmups self-pace at ~640ns through the sem chain, covering
        # the stretch until the block-0 x watermark (~17us) with few
        # instructions.
        for w in range(12):
            wp = scps.tile([128, 512], f32, tag="sc", name=f"warmw_{w}")
            nc.tensor.matmul(out=wp, lhsT=scratch[:, 0:128], rhs=scratch)

        v_tiles = {}
        qT_tiles = {}
        xts_map = {}

        def load_x(b):
            # x of block b must be resident BEFORE block b-1's attention
            # begins (its qkv matmuls are the filler there), so this is
            # emitted at the start of block b-1's qkv phase: the 10 DMAs
            # (~7us) overlap that whole phase instead of starving the
            # attention filler. For block 0 the remaining weight chunks are
            # woven between the early x tiles on the same queue so the
            # first k-loop never stalls on a weight arriving behind all
            # of x.
            xts = []
            for k in range(10):
                xt_t = xtp.tile([128, BS], dt_qkv, tag="xt", name=f"xt_{b}_{k}")
                nc.sync.dma_start(
                    out=xt_t,
                    in_=xt[k * 128 : (k + 1) * 128, b * BS : (b + 1) * BS],
                )
                xts.append(xt_t)
            xts_map[b] = xts

        def qkv_chunks(b):
            """Emission chunks for the qkv+rotary+transpose phase of block b.

            Returned as closures so they can be zipped between the previous
            block's attention chunks: alternating bass_priority makes the
            tile scheduler fill the exp-bound attention stretches with qkv
            matmuls.
            """
            st = {}
            # qk tile allocated at qkv_chunks() call time (seam start) so
            # the first c_mm does not depend on the trig DMA emission.
            st["qk"] = qkp.tile([128, 8, 320], dt_a, tag="qk", name=f"qk_{b}")

            def c_load():
                cos_t = trig.tile([128, 8, D], dt_a, tag="cos", name=f"cos_{b}")
                nc.sync.dma_start(
                    out=cos_t,
                    in_=cosd[b * 128 : (b + 1) * 128, :].rearrange(
                        "p (c d) -> p c d", c=8
                    ),
                )
                sin_t = trig.tile([128, 8, D], dt_a, tag="sin", name=f"sin_{b}")
                nc.sync.dma_start(
                    out=sin_t,
                    in_=sind[b * 128 : (b + 1) * 128, :].rearrange(
                        "p (c d) -> p c d", c=8
                    ),
                )
                st["cos"] = cos_t
                st["sin"] = sin_t

            def c_mm(m):
                def run():
                    qk_blk = st["qk"]
                    ps = qkvps.tile(
                        [128, 480], f32, tag="qkvps", name=f"qkvps_{b}_{m}"
                    )
                    for k in range(10):
                        nc.tensor.matmul(
                            out=ps,
                            lhsT=xts_map[b][k][:, m * 128 : (m + 1) * 128],
                            rhs=wt_sb[:, k, :],
                            start=(k == 0),
                            stop=(k == 9),
                        )
                    nc.vector.tensor_add(
                        out=qk_blk[:, m, :], in0=ps[:, 0:320], in1=bias_bc[:, 0:320]
                    )
                    v_t = vp.tile(
                        [128, 2, 97], dt_a, tag="v", name=f"v_{b}_{m}",
                        bufs=16 if interleave else 32,
                    )
                    nc.vector.tensor_add(
                        out=v_t[:, :, 0:D],
                        in0=ps[:, 320:480].rearrange("p (h d) -> p h d", h=2),
                        in1=bias_bc[:, 320:480].rearrange("p (h d) -> p h d", h=2),
                    )
                    nc.gpsimd.memset(v_t[:, :, D:97], 1.0)
                    v_tiles[(b, m)] = v_t

                return run

            def rot_dve(half, tau, h):
                # rotary multiply-adds for one (token-half, q/k, head) unit.
                # Split from the transposes so the DVE work can be queued
                # ahead of PE filler matmuls at the block seam: the PE is
                # strictly in-order, so a transpose waiting on DVE rotary
                # blocks every later matmul in its queue.
                def run():
                    qk_blk = st["qk"]
                    sin_t = st["sin"]
                    cos_t = st["cos"]
                    base = tau * 160 + h * D
                    ms = slice(half * 4, half * 4 + 4)
                    sl = qk_blk[:, ms, base : base + D]
                    t2 = t2p.tile(
                        [128, 4, D], dt_a, tag="t2", name=f"t2_{b}_{half}_{tau}_{h}"
                    )
                    nc.vector.tensor_mul(
                        out=t2[:, :, 0:40],
                        in0=qk_blk[:, ms, base + 40 : base + D],
                        in1=sin_t[:, ms, 0:40],
                    )
                    nc.vector.tensor_mul(
                        out=t2[:, :, 40:D],
                        in0=qk_blk[:, ms, base : base + 40],
                        in1=sin_t[:, ms, 40:D],
                    )
                    nc.vector.tensor_mul(out=sl, in0=sl, in1=cos_t[:, ms, :])
                    nc.vector.tensor_add(out=sl, in0=sl, in1=t2)

                return run

            def rot_pe(half, tau, h):
                def run():
                    qk_blk = st["qk"]
                    base = tau * 160 + h * D
                    g = half
                    if tau == 0 and half == 0:
                        dst_t = qtp.tile([D, BS], dt_a, tag="qt", name=f"qT_{b}_{h}")
                        qT_tiles[(b, h)] = dst_t
                    # half-1 transposes run at the seam where the scores
                    # PSUM bank is idle; half-0 shares the qkv bank.
                    pool = qkvps if half == 0 else scps
                    tag = "qkvps" if half == 0 else "sc"
                    tp = pool.tile(
                        [D, 512], dt_a, tag=tag, name=f"tr_{b}_{half}_{tau}_{h}"
                    )
                    for j in range(4):
                        m = g * 4 + j
                        nc.tensor.matmul(
                            out=tp[:, j * 128 : (j + 1) * 128],
                            lhsT=qk_blk[:, m, base : base + D],
                            rhs=ident,
                            is_transpose=True,
                            start=(j == 0),
                            stop=(j == 3),
                        )
                    if tau == 0:
                        dst = qT_tiles[(b, h)][:, g * 512 : (g + 1) * 512]
                    else:
                        dst = kT_sb[h][
                            :, b * BS + g * 512 : b * BS + (g + 1) * 512
                        ]
                    nc.vector.tensor_copy(out=dst, in_=tp)

                return run

            return {
                "load": c_load,
                "mm": [c_mm(m) for m in range(8)],
                "rdve": {
                    (hf, tau, h): rot_dve(hf, tau, h)
                    for hf in range(2)
                    for tau in range(2)
                    for h in range(NHL)
                },
                "rpe": {
                    (hf, tau, h): rot_pe(hf, tau, h)
                    for hf in range(2)
                    for tau in range(2)
                    for h in range(NHL)
                },
            }

        def attn_chunks(b):
            kbs = allowed[b]
            nmm = len(kbs) * 8
            st = {}

            def c_alloc():
                st["ots"] = [
                    otp.tile([D, BS], dt_a, tag="ot", name=f"ot_{b}_{h}")
                    for h in range(NHL)
                ]

            def c_attn(sh, h, fillq=None):
                def run():
                    qT_t = qT_tiles[(b, h)]
                    ot_t = st["ots"][h]
                    pv = pvps.tile(
                        [97, 512], f32, tag="pv", name=f"pv_{b}_{h}_{sh}"
                    )
                    items = [(kb, t) for kb in kbs for t in range(8)]

                    def score(i):
                        kb, t = items[i]
                        scp = scps.tile(
                            [128, 512], f32, tag="sc",
                            name=f"sc_{b}_{h}_{sh}_{kb}_{t}",
                        )
                        nc.tensor.matmul(
                            out=scp,
                            lhsT=kT_sb[h][
                                :, kb * BS + t * 128 : kb * BS + (t + 1) * 128
                            ],
                            rhs=qT_t[:, sh * 512 : (sh + 1) * 512],
                            start=True,
                            stop=True,
                        )
                        if (b, kb) in mask_add:
                            mk = mtp.tile(
                                [128, 512], f32, tag="mk",
                                name=f"mk_{b}_{h}_{sh}_{kb}_{t}",
                            )
                            nc.sync.dma_start(
                                out=mk,
                                in_=maskt[
                                    kb * BS + t * 128 : kb * BS + (t + 1) * 128,
                                    b * BS + sh * 512 : b * BS + (sh + 1) * 512,
                                ],
                            )
                            nc.vector.tensor_add(out=scp, in0=scp, in1=mk)
                        ep = expp.tile(
                            [128, 512], dt_a, tag="exp",
                            name=f"ep_{b}_{h}_{sh}_{kb}_{t}",
                        )
                        nc.scalar.activation(
                            out=ep, in_=scp, func=EXP, scale=SCALING
                        )
                        return ep

                    def pvmm(i, ep):
                        kb, t = items[i]
                        nc.tensor.matmul(
                            out=pv,
                            lhsT=v_tiles[(kb, t)][:, h, :],
                            rhs=ep,
                            start=(i == 0),
                            stop=(i == nmm - 1),
                        )

                    # scores run two chunks ahead of pv in the (in-order) PE
                    # queue: pv_i waits on exp_i, and score_{i+2} waits on
                    # exp_i via the scps WAR, so the chain advances at the
                    # scalar exp rate with the scalar engine saturated.
                    # fillq items (deferred proj pairs) slot in after each
                    # pv to absorb the remaining exp slack.
                    pend = []
                    for i in range(len(items)):
                        ep = score(i)
                        pend.append((i, ep))
                        if i >= 2:
                            j, epj = pend.pop(0)
                            pvmm(j, epj)
                            if fillq:
                                fillq.pop(0)()
                    while pend:
                        j, epj = pend.pop(0)
                        pvmm(j, epj)
                        if fillq:
                            fillq.pop(0)()
                    # normalize: dd = denom row (scalar copy, psum->sbuf),
                    # broadcast across the 80 partitions on the (idle)
                    # gpsimd engine, invert with the fast custom-DVE
                    # reciprocal, ot = pv * rb.
                    dd = ddp.tile(
                        [1, 512], f32, tag="dd", name=f"dd_{b}_{h}_{sh}"
                    )
                    nc.scalar.copy(out=dd, in_=pv[96:97, :])
                    rbc = rbp.tile(
                        [D, 512], f32, tag="rbc", name=f"rbc_{b}_{h}_{sh}"
                    )
                    nc.gpsimd.partition_broadcast(rbc, dd)
                    rb = rbp.tile([D, 512], f32, tag="rb", name=f"rb_{b}_{h}_{sh}")
                    nc.vector.reciprocal_approx_fast(out=rb, in_=rbc)
                    nc.vector.tensor_mul(
                        out=ot_t[:, sh * 512 : (sh + 1) * 512],
                        in0=pv[0:D, :],
                        in1=rb,
                    )

                return run

            def emit_proj_o(sh, o, late, copy_scalar):
                ots = st["ots"]
                sts_t = stg.tile(
                    [128, 512], out_dt, tag="st", name=f"st_{b}_{sh}_{o}"
                )
                if copy_scalar and o % 2:
                    # scalar path (exp-idle stretches): fp32 accumulate +
                    # ACT copy, keeping the work off the busy DVE.
                    pp = qkvps.tile(
                        [128, 512], f32, tag="qkvps", name=f"pr_{b}_{sh}_{o}"
                    )
                    for hh in range(2):
                        nc.tensor.matmul(
                            out=pp,
                            lhsT=pw_sb[:, hh, o * 128 : (o + 1) * 128],
                            rhs=ots[hh][:, sh * 512 : (sh + 1) * 512],
                            start=(hh == 0),
                            stop=(hh == 1),
                        )
                    nc.scalar.copy(out=sts_t, in_=pp)
                    if b == NB - 1:
                        wp = pvps.tile(
                            [128, 128], f32, tag="pv", name=f"tw_{b}_{sh}_{o}"
                        )
                        nc.tensor.matmul(
                            out=wp,
                            lhsT=scratch[:, 0:128],
                            rhs=scratch[:, 0:128],
                        )
                else:
                    # late projs run when qkv is idle: alternate the idle
                    # qkv PSUM banks in so 4 matmul pairs can run ahead of
                    # the psum->sbuf copies.
                    if (late or b == NB - 1) and o % 2:
                        pp = qkvps.tile(
                            [128, 512], f32, tag="qkvps", name=f"pr_{b}_{sh}_{o}"
                        )
                    else:
                        pp = prps.tile(
                            [128, 512], f32, tag="pr", name=f"pr_{b}_{sh}_{o}"
                        )
                    for hh in range(2):
                        nc.tensor.matmul(
                            out=pp,
                            lhsT=pw_sb[:, hh, o * 128 : (o + 1) * 128],
                            rhs=ots[hh][:, sh * 512 : (sh + 1) * 512],
                            start=(hh == 0),
                            stop=(hh == 1),
                        )
                    nc.vector.tensor_copy(out=sts_t, in_=pp)
                if copy_scalar:
                    eng = (nc.sync, nc.gpsimd, nc.scalar)[o % 3]
                else:
                    eng = nc.gpsimd if o % 2 else nc.sync
                eng.dma_start(
                    out=outp[
                        o * 128 : (o + 1) * 128,
                        b * BS + sh * 512 : b * BS + (sh + 1) * 512,
                    ],
                    in_=sts_t,
                )

            def c_proj(sh, o_lo=0, o_hi=10, late=False, copy_scalar=False):
                def run():
                    for o in range(o_lo, o_hi):
                        emit_proj_o(sh, o, late, copy_scalar)

                return run

            def proj_pair(sh, o, late=False):
                def run():
                    emit_proj_o(sh, o, late, False)

                return run

            if b == NB - 1:
                # final block: deferred proj pairs of blocks NB-3/NB-2 are
                # consumed one pair per attention chunk inside the units
                # (fillq), absorbing the per-chunk exp slack; the block's
                # own sh0 proj pairs join the queue once both sh0 units are
                # done. sh1 proj runs at the very end with scalar copies
                # (exp is finished by then).
                chunks = [c_alloc]
                for h in range(NHL):
                    r = c_attn(0, h, fillq=deferred_pairs)
                    r.is_attn = True
                    chunks.append(r)
                chunks.append(
                    lambda: deferred_pairs.extend(
                        proj_pair(0, o) for o in range(10)
                    )
                )
                for h in range(NHL):
                    r = c_attn(1, h, fillq=deferred_pairs)
                    r.is_attn = True
                    chunks.append(r)
                chunks.append(lambda: [f() for f in deferred_pairs])
                chunks.append(lambda: deferred_pairs.clear())
                chunks.append(c_proj(1, copy_scalar=True))
                return chunks

            chunks = [c_alloc]
            for sh in range(2):
                for h in range(NHL):
                    r = c_attn(sh, h)
                    r.is_attn = True
                    chunks.append(r)
                # sh1 projs of blocks NB-4..NB-2 are deferred into the last
                # block's seam + attention phase: they are the only dense PE
                # filler available there (no next-block qkv remains), keeping
                # the HAM clock warm through the tail. Block NB-4's unit is
                # split in half to zip with the seam transposes; its copies
                # go to the scalar engine, which is exp-idle at the seam.
                if sh == 1 and b == NB - 4:
                    deferred.append(c_proj(sh, 0, 5, late=True, copy_scalar=True))
                    deferred.append(c_proj(sh, 5, 10, late=True, copy_scalar=True))
                elif sh == 1 and b in (NB - 3, NB - 2):
                    deferred_pairs.extend(
                        proj_pair(sh, o, late=True) for o in range(10)
                    )
                else:
                    # the last two o-chunks of each sh0 proj also go to the
                    # final block's fill queue: 30 pairs cover only 30 of
                    # its 32 attention-chunk slots, so the last unit runs
                    # dry without these.
                    chunks.append(c_proj(sh, 0, 8))
                    deferred_pairs.extend(
                        proj_pair(sh, o, late=True) for o in range(8, 10)
                    )
            return chunks

        # rotary unit order: k heads first (scores need the full kT block),
        # q heads after (only the matching sh-half of qT is needed early)
        ROT0 = [(0, 1, 0), (0, 1, 1), (0, 0, 0), (0, 0, 1)]
        ROT1 = [(1, 1, 0), (1, 1, 1), (1, 0, 0), (1, 0, 1)]

        def mid_list(q):
            # qkv mid-phase of a block: mm4..7 with the half-0 rotary units
            # woven in so each transpose lands behind a dense mm stretch.
            return [
                q["mm"][4], q["rdve"][ROT0[0]], q["mm"][5], q["rpe"][ROT0[0]],
                q["rdve"][ROT0[1]], q["mm"][6], q["rpe"][ROT0[1]],
                q["rdve"][ROT0[2]], q["mm"][7], q["rpe"][ROT0[2]],
                q["rdve"][ROT0[3]], q["rpe"][ROT0[3]],
            ]

        if interleave:
            # Software-pipelined emission. Per block b:
            #   seam:      half-1 rotary DVE queued first, its transposes
            #              zipped between block b+1's first qkv matmuls
            #              (in-order PE filler for the DVE wait)
            #   attention: zipped with block b+1's qkv mid-phase
            # The deferred sh1 projs of blocks NB-3/NB-2 play the role of
            # the "next qkv" for the final block.
            deferred = []
            deferred_pairs = []
            load_x(0)
            cur = qkv_chunks(0)
            c_wrest()
            for m in range(2):
                cur["mm"][m]()
            cur["load"]()
            c_const()
            for m in range(2, 4):
                cur["mm"][m]()
            load_x(1)
            for c in mid_list(cur):
                c()
            for b in range(NB):
                at = attn_chunks(b)
                if b + 1 < NB:
                    nxt = qkv_chunks(b + 1)
                    nxt["load"]()
                    if b + 2 < NB:
                        load_x(b + 2)
                    seam_fill = [nxt["mm"][m] for m in range(4)]
                else:
                    nxt = None
                    seam_fill = deferred[:2]
                for u in ROT1:
                    cur["rdve"][u]()
                if nxt is not None:
                    for f, u in zip(seam_fill, ROT1):
                        f()
                        cur["rpe"][u]()
                else:
                    seam_fill[0]()
                    cur["rpe"][ROT1[0]]()
                    cur["rpe"][ROT1[1]]()
                    seam_fill[1]()
                    cur["rpe"][ROT1[2]]()
                    cur["rpe"][ROT1[3]]()
                filler = mid_list(nxt) if nxt is not None else deferred[2:]
                # distribute filler only after the exp-bound attention
                # chunks (proj chunks are already tensor-dense)
                attn_pos = [
                    i for i, a in enumerate(at) if getattr(a, "is_attn", False)
                ]
                k = 0
                j = 0
                for i, a in enumerate(at):
                    a()
                    if i in attn_pos:
                        j += 1
                        take = (len(filler) * j) // max(1, len(attn_pos)) - k
                        for _ in range(take):
                            filler[k]()
                            k += 1
                while k < len(filler):
                    filler[k]()
                    k += 1
                cur = nxt
        else:
            deferred = []
            deferred_pairs = []
            c_wrest()
            c_const()
            load_x(0)
            for b in range(NB):
                if b + 1 < NB:
                    load_x(b + 1)
                q = qkv_chunks(b)
                q["load"]()
                for m in range(8):
                    q["mm"][m]()
                for u in ROT0 + ROT1:
                    q["rdve"][u]()
                    q["rpe"][u]()
            for b in range(NB):
                for c in attn_chunks(b):
                    c()
            for c in deferred:
                c()
            for c in deferred_pairs:
                c()

    nc.compile()
    return nc


def _build_v4(qkv_dt_name="bfloat16", attn_dt_name="bfloat16",
              out_dt_name="bfloat16"):
    """Block-sharded build: core c handles sequence block c//2 (1024
    tokens) and head-half c%2 (8 heads, 4 head-groups of 2). Attention is
    identical per head-group, but the output projection now contracts over
    640 dims (5 full 128-chunks) instead of 2x80 -> 10x2x5x512 cycles for
    a quarter of the output area, saving ~12.8us of PE streaming per core,
    and the kernel ends in a dense exp-free proj GEMM.
    Only valid for the exact block-diagonal mask (no mask adds).
    """
    import concourse.mybir as mybir
    import concourse.tile as tile
    from concourse import bacc
    from concourse.masks import make_identity

    f32 = mybir.dt.float32
    dt_qkv = getattr(mybir.dt, qkv_dt_name)
    dt_a = getattr(mybir.dt, attn_dt_name)
    out_dt = getattr(mybir.dt, out_dt_name)
    NHG = 4  # head groups of 2 per core

    nc = bacc.Bacc(
        "TRN2", target_bir_lowering=False, debug=False, num_devices=NCORES
    )
    xt = nc.dram_tensor("xt", [HID, BS], dt_qkv, kind="ExternalInput").ap()
    # host pre-arranged to the SBUF tile layout so every weight DMA is
    # contiguous per partition (strided weight loads run at ~50GB/s and
    # starve the x stream)
    wt = nc.dram_tensor(
        "wt", [NHG, 128, 10 * 480], dt_qkv, kind="ExternalInput"
    ).ap()
    bqkv = nc.dram_tensor("bqkv", [1, NHG * 480], f32, kind="ExternalInput").ap()
    cosd = nc.dram_tensor("cosd", [128, 8 * D], dt_a, kind="ExternalInput").ap()
    sind = nc.dram_tensor("sind", [128, 8 * D], dt_a, kind="ExternalInput").ap()
    pw = nc.dram_tensor("pw", [5, 128, HID], dt_a, kind="ExternalInput").ap()
    outp = nc.dram_tensor("outp", [HID, BS], out_dt, kind="ExternalOutput").ap()

    EXP = mybir.ActivationFunctionType.Exp

    with ExitStack() as ctx:
        tc = ctx.enter_context(tile.TileContext(nc))

        cpool = ctx.enter_context(tc.tile_pool(name="cpool", bufs=1))
        wt_sb = cpool.tile([128, NHG, 10, 480], dt_qkv, tag="wt_sb", name="wt_sb")
        # hg0 weights ahead of x on sync/scalar; hg1-3 behind on gpsimd
        wt_r0 = wt[0].rearrange("p (kk c) -> p kk c", kk=10)
        nc.sync.dma_start(out=wt_sb[:, 0, 0:2, :], in_=wt_r0[:, 0:2, :])
        bias_bc = cpool.tile([128, NHG * 480], f32, tag="bias_bc", name="bias_bc")
        ident = cpool.tile([128, 128], dt_a, tag="ident", name="ident")
        make_identity(nc, ident)
        pw_sb = cpool.tile([128, 5, HID], dt_a, tag="pw_sb", name="pw_sb")
        ot_stack = [
            cpool.tile([128, BS], dt_a, tag=f"ot{c}", name=f"ot{c}")
            for c in range(5)
        ]

        def c_wrest():
            nc.scalar.dma_start(out=wt_sb[:, 0, 2:10, :], in_=wt_r0[:, 2:10, :])
            nc.gpsimd.dma_start(
                out=bias_bc[:, 0:480],
                in_=bqkv[0:1, 0:480].to_broadcast((128, 480)),
            )

        def c_wnext(hg):
            # one head-group of weights (1.2MB), loaded one phase before
            # its qkv so the DMA never competes with block-0 x at startup
            def run():
                nc.gpsimd.dma_start(
                    out=wt_sb[:, hg, :, :],
                    in_=wt[hg].rearrange("p (kk c) -> p kk c", kk=10),
                )
                nc.gpsimd.dma_start(
                    out=bias_bc[:, hg * 480 : (hg + 1) * 480],
                    in_=bqkv[0:1, hg * 480 : (hg + 1) * 480].to_broadcast(
                        (128, 480)
                    ),
                )

            return run

        def c_const():
            nc.gpsimd.dma_start(out=pw_sb, in_=pw.rearrange("c p o -> p c o"))

        # ---- pools ----
        xtp = ctx.enter_context(tc.tile_pool(name="xtp", bufs=10))
        trig = ctx.enter_context(tc.tile_pool(name="trig", bufs=1))
        qkp = ctx.enter_context(tc.tile_pool(name="qkp", bufs=2))
        t2p = ctx.enter_context(tc.tile_pool(name="t2p", bufs=2))
        vp = ctx.enter_context(tc.tile_pool(name="vp", bufs=1))
        qtp = ctx.enter_context(tc.tile_pool(name="qtp", bufs=4))
        ktp = ctx.enter_context(tc.tile_pool(name="ktp", bufs=4))
        expp = ctx.enter_context(tc.tile_pool(name="expp", bufs=4))
        ddp = ctx.enter_context(tc.tile_pool(name="ddp", bufs=2))
        rbp = ctx.enter_context(tc.tile_pool(name="rbp", bufs=2))
        stg = ctx.enter_context(tc.tile_pool(name="stg", bufs=20))
        otmp = ctx.enter_context(tc.tile_pool(name="otmp", bufs=3))

        qkvps = ctx.enter_context(tc.tile_pool(name="qkvps", bufs=2, space="PSUM"))
        scps = ctx.enter_context(tc.tile_pool(name="scps", bufs=2, space="PSUM"))
        pvps = ctx.enter_context(tc.tile_pool(name="pvps", bufs=2, space="PSUM"))
        prps = ctx.enter_context(tc.tile_pool(name="prps", bufs=2, space="PSUM"))

        # PE warmup (see _build)
        scratch = cpool.tile([128, 512], dt_a, tag="scratch", name="scratch")
        nc.gpsimd.memset(scratch, 0.0)
        for w in range(6):
            wp = scps.tile([128, 128], f32, tag="sc", name=f"warm_{w}")
            nc.tensor.matmul(out=wp, lhsT=scratch[:, 0:128], rhs=scratch[:, 0:128])
        for w in range(12):
            wp = scps.tile([128, 512], f32, tag="sc", name=f"warmw_{w}")
            nc.tensor.matmul(out=wp, lhsT=scratch[:, 0:128], rhs=scratch)

        v_tiles = {}
        qT_tiles = {}
        kT_tiles = {}
        xts = []
        st_trig = {}

        def load_x():
            # startup is single-queue at full HBM share: hg0's remaining
            # weight chunks are woven into the x stream (splitting them to
            # other queues costs x two-thirds of the bandwidth).
            for k in range(10):
                xt_t = xtp.tile([128, BS], dt_qkv, tag="xt", name=f"xt_{k}")
                nc.sync.dma_start(out=xt_t, in_=xt[k * 128 : (k + 1) * 128, :])
                xts.append(xt_t)

        def c_trig():
            cos_t = trig.tile([128, 8, D], dt_a, tag="cos", name="cos")
            nc.sync.dma_start(
                out=cos_t, in_=cosd.rearrange("p (c d) -> p c d", c=8)
            )
            sin_t = trig.tile([128, 8, D], dt_a, tag="sin", name="sin")
            nc.sync.dma_start(
                out=sin_t, in_=sind.rearrange("p (c d) -> p c d", c=8)
            )
            st_trig["cos"] = cos_t
            st_trig["sin"] = sin_t

        def qkv_chunks(hg):
            st = {}
            st["qk"] = qkp.tile([128, 8, 320], dt_a, tag="qk", name=f"qk_{hg}")

            def c_mm(m):
                def run():
                    qk_blk = st["qk"]
                    ps = qkvps.tile(
                        [128, 480], f32, tag="qkvps", name=f"qkvps_{hg}_{m}"
                    )
                    for k in range(10):
                        nc.tensor.matmul(
                            out=ps,
                            lhsT=xts[k][:, m * 128 : (m + 1) * 128],
                            rhs=wt_sb[:, hg, k, :],
                            start=(k == 0),
                            stop=(k == 9),
                        )
                    bb = bias_bc[:, hg * 480 : (hg + 1) * 480]
                    nc.vector.tensor_add(
                        out=qk_blk[:, m, :], in0=ps[:, 0:320], in1=bb[:, 0:320]
                    )
                    v_t = vp.tile(
                        [128, 2, 97], dt_a, tag="v", name=f"v_{hg}_{m}", bufs=16
                    )
                    nc.vector.tensor_add(
                        out=v_t[:, :, 0:D],
                        in0=ps[:, 320:480].rearrange("p (h d) -> p h d", h=2),
                        in1=bb[:, 320:480].rearrange("p (h d) -> p h d", h=2),
                    )
                    nc.gpsimd.memset(v_t[:, :, D:97], 1.0)
                    v_tiles[(hg, m)] = v_t

                return run

            def rot_dve(half, tau, h):
                def run():
                    qk_blk = st["qk"]
                    sin_t = st_trig["sin"]
                    cos_t = st_trig["cos"]
                    base = tau * 160 + h * D
                    ms = slice(half * 4, half * 4 + 4)
                    sl = qk_blk[:, ms, base : base + D]
                    t2 = t2p.tile(
                        [128, 4, D], dt_a, tag="t2",
                        name=f"t2_{hg}_{half}_{tau}_{h}",
                    )
                    nc.vector.tensor_mul(
                        out=t2[:, :, 0:40],
                        in0=qk_blk[:, ms, base + 40 : base + D],
                        in1=sin_t[:, ms, 0:40],
                    )
                    nc.vector.tensor_mul(
                        out=t2[:, :, 40:D],
                        in0=qk_blk[:, ms, base : base + 40],
                        in1=sin_t[:, ms, 40:D],
                    )
                    nc.vector.tensor_mul(out=sl, in0=sl, in1=cos_t[:, ms, :])
                    nc.vector.tensor_add(out=sl, in0=sl, in1=t2)

                return run

            def rot_pe(half, tau, h):
                def run():
                    qk_blk = st["qk"]
                    base = tau * 160 + h * D
                    g = half
                    if half == 0:
                        if tau == 0:
                            qT_tiles[(hg, h)] = qtp.tile(
                                [D, BS], dt_a, tag="qt", name=f"qT_{hg}_{h}"
                            )
                        else:
                            kT_tiles[(hg, h)] = ktp.tile(
                                [D, BS], dt_a, tag="kt", name=f"kT_{hg}_{h}"
                            )
                    pool = qkvps if half == 0 else scps
                    tag = "qkvps" if half == 0 else "sc"
                    tp = pool.tile(
                        [D, 512], dt_a, tag=tag, name=f"tr_{hg}_{half}_{tau}_{h}"
                    )
                    for j in range(4):
                        m = g * 4 + j
                        nc.tensor.matmul(
                            out=tp[:, j * 128 : (j + 1) * 128],
                            lhsT=qk_blk[:, m, base : base + D],
                            rhs=ident,
                            is_transpose=True,
                            start=(j == 0),
                            stop=(j == 3),
                        )
                    tiles = qT_tiles if tau == 0 else kT_tiles
                    dst = tiles[(hg, h)][:, g * 512 : (g + 1) * 512]
                    nc.vector.tensor_copy(out=dst, in_=tp)

                return run

            return {
                "mm": [c_mm(m) for m in range(8)],
                "rdve": {
                    (hf, tau, h): rot_dve(hf, tau, h)
                    for hf in range(2)
                    for tau in range(2)
                    for h in range(NHL)
                },
                "rpe": {
                    (hf, tau, h): rot_pe(hf, tau, h)
                    for hf in range(2)
                    for tau in range(2)
                    for h in range(NHL)
                },
            }

        def emit_proj_o(sh, o, copy_scalar):
            sts_t = stg.tile([128, 512], out_dt, tag="st", name=f"st_{sh}_{o}")
            use_scalar = copy_scalar and o % 2
            pool, tag = (qkvps, "qkvps") if o % 2 else (prps, "pr")
            pp = pool.tile([128, 512], f32, tag=tag, name=f"pr_{sh}_{o}")
            for c in range(5):
                nc.tensor.matmul(
                    out=pp,
                    lhsT=pw_sb[:, c, o * 128 : (o + 1) * 128],
                    rhs=ot_stack[c][:, sh * 512 : (sh + 1) * 512],
                    start=(c == 0),
                    stop=(c == 4),
                )
            if use_scalar:
                nc.scalar.copy(out=sts_t, in_=pp)
            else:
                nc.vector.tensor_copy(out=sts_t, in_=pp)
            eng = (nc.sync, nc.gpsimd, nc.scalar)[o % 3] if copy_scalar else (
                nc.gpsimd if o % 2 else nc.sync
            )
            eng.dma_start(
                out=outp[
                    o * 128 : (o + 1) * 128, sh * 512 : (sh + 1) * 512
                ],
                in_=sts_t,
            )

        def proj_pair(sh, o, copy_scalar=False):
            def run():
                emit_proj_o(sh, o, copy_scalar)

            return run

        fillq = []

        def attn_chunks(hg):
            def c_attn(sh, h, use_fill=False):
                def run():
                    qT_t = qT_tiles[(hg, h)]
                    kT_t = kT_tiles[(hg, h)]
                    pv = pvps.tile(
                        [97, 512], f32, tag="pv", name=f"pv_{hg}_{h}_{sh}"
                    )

                    def score(i):
                        scp = scps.tile(
                            [128, 512], f32, tag="sc", name=f"sc_{hg}_{h}_{sh}_{i}"
                        )
                        nc.tensor.matmul(
                            out=scp,
                            lhsT=kT_t[:, i * 128 : (i + 1) * 128],
                            rhs=qT_t[:, sh * 512 : (sh + 1) * 512],
                            start=True,
                            stop=True,
                        )
                        ep = expp.tile(
                            [128, 512], dt_a, tag="exp",
                            name=f"ep_{hg}_{h}_{sh}_{i}",
                        )
                        nc.scalar.activation(
                            out=ep, in_=scp, func=EXP, scale=SCALING
                        )
                        return ep

                    def pvmm(i, ep):
                        nc.tensor.matmul(
                            out=pv,
                            lhsT=v_tiles[(hg, i)][:, h, :],
                            rhs=ep,
                            start=(i == 0),
                            stop=(i == 7),
                        )

                    pend = []
                    for i in range(8):
                        ep = score(i)
                        pend.append((i, ep))
                        if i >= 2:
                            j, epj = pend.pop(0)
                            pvmm(j, epj)
                            if use_fill and fillq:
                                fillq.pop(0)()
                    while pend:
                        j, epj = pend.pop(0)
                        pvmm(j, epj)
                        if use_fill and fillq:
                            fillq.pop(0)()
                    dd = ddp.tile([1, 512], f32, tag="dd", name=f"dd_{hg}_{h}_{sh}")
                    nc.scalar.copy(out=dd, in_=pv[96:97, :])
                    rbc = rbp.tile(
                        [D, 512], f32, tag="rbc", name=f"rbc_{hg}_{h}_{sh}"
                    )
                    nc.gpsimd.partition_broadcast(rbc, dd)
                    rb = rbp.tile([D, 512], f32, tag="rb", name=f"rb_{hg}_{h}_{sh}")
                    nc.vector.reciprocal_approx_fast(out=rb, in_=rbc)
                    # place into the stacked [128, BS] ot tiles at the head
                    # global row offset. DVE lanes are hard-wired to
                    # partitions, so a partition-shifted placement needs an
                    # SBUF->SBUF DMA hop (split across two tiles if the
                    # head straddles a 128 boundary).
                    g = hg * 2 + h
                    r0 = (g * D) % 128
                    t0 = (g * D) // 128
                    cs = slice(sh * 512, (sh + 1) * 512)
                    if r0 == 0:
                        nc.vector.tensor_mul(
                            out=ot_stack[t0][0:D, cs], in0=pv[0:D, :], in1=rb
                        )
                    else:
                        ott = otmp.tile(
                            [D, 512], dt_a, tag="otm", name=f"otm_{hg}_{h}_{sh}"
                        )
                        nc.vector.tensor_mul(out=ott, in0=pv[0:D, :], in1=rb)
                        n1 = min(128 - r0, D)
                        eng = nc.gpsimd if (g + sh) % 2 else nc.sync
                        eng.dma_start(
                            out=ot_stack[t0][r0 : r0 + n1, cs], in_=ott[0:n1, :]
                        )
                        if n1 < D:
                            eng.dma_start(
                                out=ot_stack[t0 + 1][0 : D - n1, cs],
                                in_=ott[n1:D, :],
                            )

                return run

            if hg == NHG - 1:
                chunks = []
                for h in range(NHL):
                    r = c_attn(0, h)
                    r.is_attn = True
                    chunks.append(r)
                chunks.append(
                    lambda: fillq.extend(proj_pair(0, o) for o in range(10))
                )
                for h in range(NHL):
                    r = c_attn(1, h, use_fill=True)
                    r.is_attn = True
                    chunks.append(r)
                chunks.append(lambda: [f() for f in fillq])
                chunks.append(lambda: fillq.clear())
                chunks.append(
                    lambda: [
                        emit_proj_o(1, o, copy_scalar=True) for o in range(10)
                    ]
                )
                return chunks
            chunks = []
            for sh in range(2):
                for h in range(NHL):
                    r = c_attn(sh, h)
                    r.is_attn = True
                    chunks.append(r)
            return chunks

        ROT0 = [(0, 1, 0), (0, 1, 1), (0, 0, 0), (0, 0, 1)]
        ROT1 = [(1, 1, 0), (1, 1, 1), (1, 0, 0), (1, 0, 1)]

        def mid_list(q):
            return [
                q["mm"][4], q["rdve"][ROT0[0]], q["mm"][5], q["rpe"][ROT0[0]],
                q["rdve"][ROT0[1]], q["mm"][6], q["rpe"][ROT0[1]],
                q["rdve"][ROT0[2]], q["mm"][7], q["rpe"][ROT0[2]],
                q["rdve"][ROT0[3]], q["rpe"][ROT0[3]],
            ]

        def seam_dummy():
            wp = prps.tile([128, 512], f32, tag="pr", name=f"sd_{len(sdn)}")
            sdn.append(0)
            nc.tensor.matmul(out=wp, lhsT=scratch[:, 0:128], rhs=scratch)

        sdn = []
        load_x()
        cur = qkv_chunks(0)
        c_wrest()
        for m in range(2):
            cur["mm"][m]()
        c_trig()
        for m in range(2, 4):
            cur["mm"][m]()
        mids = mid_list(cur)
        mids[0]()
        c_wnext(1)()
        for c in mids[1:]:
            c()
        for hg in range(NHG):
            at = attn_chunks(hg)
            if hg + 1 < NHG:
                nxt = qkv_chunks(hg + 1)
                seam_fill = [nxt["mm"][m] for m in range(4)]
            else:
                nxt = None
                seam_fill = [seam_dummy, seam_dummy, seam_dummy, seam_dummy]
            for u in ROT1:
                cur["rdve"][u]()
            for f, u in zip(seam_fill, ROT1):
                f()
                cur["rpe"][u]()
            filler = mid_list(nxt) if nxt is not None else []
            if hg == 0:
                filler = [c_wnext(2)] + filler
            elif hg == 1:
                filler = [c_wnext(3), c_const] + filler
            attn_pos = [
                i for i, a in enumerate(at) if getattr(a, "is_attn", False)
            ]
            k = 0
            j = 0
            for i, a in enumerate(at):
                a()
                if i in attn_pos:
                    j += 1
                    take = (len(filler) * j) // max(1, len(attn_pos)) - k
                    for _ in range(take):
                        filler[k]()
                        k += 1
            while k < len(filler):
                filler[k]()
                k += 1
            cur = nxt

    nc.compile()
    return nc


def _analyze_mask(mask):
    m = np.asarray(mask).reshape(S, S)
    allowed = []
    mask_add = set()
    for qb in range(NB):
        row = []
        for kb in range(NB):
            t = m[qb * BS : (qb + 1) * BS, kb * BS : (kb + 1) * BS]
            if np.all(t <= NEG_THRESH):
                continue
            row.append(kb)
            if not np.all(t == 0.0):
                mask_add.add((qb, kb))
        if not row:
            raise NotImplementedError("fully masked query block")
        allowed.append(tuple(row))
    return tuple(allowed), frozenset(mask_add)


def _np_dt(name):
    if name == "bfloat16":
        import ml_dtypes

        return ml_dtypes.bfloat16
    return np.float32


def kernel(
    hidden_states, attention_mask, cos, sin, qkv_w, qkv_b, proj_w, proj_b
):
    from concourse import bass_utils

    qkv_dt = os.environ.get("KERNEL_QKV_DT", "bfloat16")
    attn_dt = os.environ.get("KERNEL_ATTN_DT", "bfloat16")
    out_dt = os.environ.get("KERNEL_OUT_DT", "bfloat16")
    trace = bool(int(os.environ.get("KERNEL_TRACE", "0")))

    global LAST_RESULT

    X = np.ascontiguousarray(np.asarray(hidden_states, dtype=np.float32))
    allowed, mask_add = _analyze_mask(attention_mask)

    key = (qkv_dt, attn_dt, out_dt, allowed, mask_add)
    if key not in _CACHE:
        _CACHE[key] = _build(
            allowed, mask_add, qkv_dt_name=qkv_dt, attn_dt_name=attn_dt,
            out_dt_name=out_dt,
        )
    nc = _CACHE[key]

    np_qkv = _np_dt(qkv_dt)
    np_attn = _np_dt(attn_dt)

    XT = np.ascontiguousarray(X.T).astype(np_qkv)
    cos = np.ascontiguousarray(np.asarray(cos, dtype=np.float32))
    sin = np.asarray(sin, dtype=np.float32)
    sinh = np.ascontiguousarray(
        np.concatenate([-sin[:, : D // 2], sin[:, D // 2 :]], axis=1)
    )
    qkv_w = np.asarray(qkv_w, dtype=np.float32)

    def trig_prep(a):
        # [S, D] -> [NB*128, 8*D]: tile (b, p, c, d) = a[b*1024 + c*128 + p, d]
        return np.ascontiguousarray(
            a.reshape(NB, 8, 128, D).transpose(0, 2, 1, 3).reshape(NB * 128, 8 * D)
        )
    qkv_b = np.asarray(qkv_b, dtype=np.float32)
    proj_w = np.asarray(proj_w, dtype=np.float32)
    proj_b = np.asarray(proj_b, dtype=np.float32)

    diag = (
        all(tuple(allowed[b]) == (b,) for b in range(NB)) and not mask_add
    )
    if diag and os.environ.get("KERNEL_V4", "0") == "1":
        key = ("v4", qkv_dt, attn_dt, out_dt)
        if key not in _CACHE:
            _CACHE[key] = _build_v4(
                qkv_dt_name=qkv_dt, attn_dt_name=attn_dt, out_dt_name=out_dt
            )
        nc4 = _CACHE[key]
        cos_p = trig_prep(cos.astype(np_attn))
        sin_p = trig_prep(sinh.astype(np_attn))
        in_maps = []
        for c in range(NCORES):
            b = c // 2
            hh = c % 2
            wgs, bgs = [], []
            for hg in range(4):
                j0 = (hh * 8 + hg * 2) * D
                sl = slice(j0, j0 + 2 * D)
                Wg = np.concatenate(
                    [qkv_w[sl], qkv_w[HID:][sl], qkv_w[2 * HID :][sl]], axis=0
                )
                # [1280, 480] -> SBUF layout [128, 10*480]
                wgs.append(
                    Wg.T.reshape(10, 128, 480).transpose(1, 0, 2).reshape(
                        128, 4800
                    )
                )
                bgs.append(
                    np.concatenate(
                        [qkv_b[sl], qkv_b[HID:][sl], qkv_b[2 * HID :][sl]]
                    )
                )
            pwT = proj_w[:, hh * 640 : (hh + 1) * 640].T
            in_maps.append({
                "xt": np.ascontiguousarray(XT[:, b * BS : (b + 1) * BS]),
                "wt": np.ascontiguousarray(np.stack(wgs)).astype(np_qkv),
                "bqkv": np.ascontiguousarray(
                    np.concatenate(bgs)[None, :].astype(np.float32)
                ),
                "cosd": np.ascontiguousarray(cos_p[b * 128 : (b + 1) * 128]),
                "sind": np.ascontiguousarray(sin_p[b * 128 : (b + 1) * 128]),
                "pw": np.ascontiguousarray(pwT.reshape(5, 128, HID)).astype(
                    np_attn
                ),
            })
        res = bass_utils.run_bass_kernel_spmd(
            nc4, in_maps, core_ids=list(range(NCORES)), trace=trace
        )
        LAST_RESULT = res
        acc = np.zeros((HID, S), dtype=np.float64)
        for c in range(NCORES):
            b = c // 2
            acc[:, b * BS : (b + 1) * BS] += np.asarray(
                res.results[c]["outp"], dtype=np.float64
            )
        out = acc.T + proj_b.astype(np.float64)[None, :]
        return out.astype(np.float32)

    in_maps = []
    for c in range(NCORES):
        j0 = c * NHL * D
        sl = slice(j0, j0 + NHL * D)
        Wc = np.concatenate(
            [qkv_w[sl], qkv_w[HID:][sl], qkv_w[2 * HID :][sl]], axis=0
        )
        m = {
            "xt": XT,
            "wt": np.ascontiguousarray(
                Wc.T.reshape(10, 128, 480).transpose(1, 0, 2).reshape(128, 4800)
            ).astype(np_qkv),
            "bqkv": np.ascontiguousarray(
                np.concatenate([qkv_b[sl], qkv_b[HID:][sl], qkv_b[2 * HID :][sl]])[
                    None, :
                ]
            ),
            "cosd": trig_prep(cos.astype(np_attn)),
            "sind": trig_prep(sinh.astype(np_attn)),
            "pw": np.ascontiguousarray(
                np.stack(
                    [
                        proj_w[:, j0 : j0 + D].T,
                        proj_w[:, j0 + D : j0 + 2 * D].T,
                    ]
                )
            ).astype(np_attn),
        }
        if mask_add:
            m["maskt"] = np.ascontiguousarray(
                (np.asarray(attention_mask).reshape(S, S).T / SCALING).astype(
                    np.float32
                )
            )
        in_maps.append(m)

    res = bass_utils.run_bass_kernel_spmd(
        nc, in_maps, core_ids=list(range(NCORES)), trace=trace
    )
    LAST_RESULT = res

    acc = np.zeros((HID, S), dtype=np.float64)
    for c in range(NCORES):
        acc += np.asarray(res.results[c]["outp"], dtype=np.float64)
    out = acc.T + proj_b.astype(np.float64)[None, :]
    return out.astype(np.float32)


LAST_RESULT = None

